# revision 23
# baseline (speedup 1.0000x reference)
"""Trainium2 Bass kernel for nn_Attention_72438918414857.

Reference computation (B=8, N=1024, C=768, H=12, D=64):
    qkv = (x @ qkv_w.T + qkv_b) -> q, k, v per head
    attn = softmax(q @ k.T / sqrt(D)) + static_a   (bias added AFTER softmax)
    out = (attn @ v) merged-heads @ proj_w.T + proj_b

Sharding: data-parallel over batch -- one batch element per NeuronCore,
weights + static_a replicated. No collectives needed.

Math used on-chip (per batch, per head), everything transposed so each
matmul gets its contraction dim on partitions with no on-chip transposes:
    qkT = [Wq;Wk]^T-proj of x  ->  [cout, t] layout
    E^T = exp(K_h^T.T @ Q_h^T * D^-0.5)           [k, q] strips
    out_h^T = ([V_h|1].T @ E^T) -> rows 0..63 = E@v, row 64 = rowsum(E)
    attn_h^T = (E@v) * (1/rowsum) + V_h.T @ A_h^T
where static_a is pre-transposed on host to A^T[h, k, q].  The softmax
normalization is applied to the [64, q] output instead of the [k, q]
matrix; no max-subtraction is needed (|scores*scale| < ~3).

Matmuls run in bf16 (fp32 PE matmul is 4x slower); PSUM accumulation is
fp32.  bf16 rounding of operands keeps rel-err ~4e-3, well under the
2e-2 gate.

v2 scheduling changes (over the first working version):
  - warm-up matmuls on const data at t=0 so the PE HAM clock-gate
    reaches 8/8 before the real work starts (first ~20us of v1 ran at
    1.2 GHz)
  - V projection runs kc-outer in two tt-quads so the first x/vw DMA
    chunk immediately yields dense PE work
  - qkT projections for pairs 2..5 are emitted as per-item filler
    bursts inside the attention loop, filling the exp-gated PE bubbles
    at item boundaries
  - softmax reciprocal on DVE (reciprocal_approx_fast) instead of the
    ACT Ln/Exp chain (saves ~33us of ACT time; ACT runs only the big
    exps)
  - drain: proj partial bursts interleaved into the last item's
    out-steps; output stored/DMA'd as bf16
"""

import os
import sys

import numpy as np

B, N, C = 8, 1024, 768
H, D = 12, 64
NCORES = 8
P = 128
QW = 512          # q tile width (PSUM bank = 512 f32)
NQT = N // QW     # 2 q tiles
NKT = N // P      # 8 k tiles
NCIN = C // P     # 6 c_in chunks
NPAIR = H // 2    # 6 head pairs
SCALE = float(D) ** -0.5

_REPO = "/opt/trn_rl_repo"


def _ensure_paths():
    if _REPO not in sys.path:
        sys.path.insert(0, _REPO)


def _dedup_ldweights(nc):
    """Delete an Ldweights whose weights AP + tile geometry match the
    immediately preceding Ldweights on the PE stream (the weights are
    still resident in the array); its waits/updates move to the next
    instruction."""
    import concourse.mybir as mybir

    def sig(inst):
        ap = inst.ins[0]
        return (str(ap), str(getattr(inst, "tile_position", None)),
                str(getattr(inst, "tile_size", None)))

    for fn in nc.m.functions:
        for blk in fn.blocks:
            out = []
            last_sig = None
            pend_w, pend_u = [], []
            changed = False
            for inst in blk.instructions:
                op = str(inst.opcode)
                if op == "Ldweights":
                    s_ = sig(inst)
                    if s_ == last_sig:
                        si = inst.sync_info
                        if si:
                            pend_w.extend(si.on_wait or [])
                            pend_u.extend(si.on_update or [])
                        changed = True
                        continue
                    last_sig = s_
                elif op == "Matmult":
                    pass          # matmuls don't disturb loaded weights
                elif op in ("NoOp", "EventSemaphore"):
                    pass
                else:
                    last_sig = None
                if pend_w or pend_u:
                    si = inst.sync_info
                    ow = list(si.on_wait or []) if si else []
                    ou = list(si.on_update or []) if si else []
                    inst.sync_info = mybir.SyncInfo(
                        on_wait=pend_w + ow, on_update=pend_u + ou)
                    pend_w, pend_u = [], []
                out.append(inst)
            assert not pend_w and not pend_u
            if changed:
                blk.instructions = out


def _split_excess_waits(nc):
    """The TRN2 walrus codegen allows only 1 sem-wait command per
    instruction.  Tile's sem-assigner can emit more (one per logical
    proc a tile depends on).
    Move the excess onto freshly inserted same-engine NoOps placed just
    before the instruction -- engines execute in order, so waiting on a
    preceding NoOp is equivalent."""
    import concourse.mybir as mybir
    from bass_rust import InstNoOp

    nid = [0]
    for fn in nc.m.functions:
        for blk in fn.blocks:
            out = []
            changed = False
            for inst in blk.instructions:
                si = inst.sync_info
                waits = list(si.on_wait) if si and si.on_wait else []
                limit = 1
                if len(waits) > limit:
                    extra, keep = waits[:-limit], waits[-limit:]
                    inst.sync_info = si.__replace__(on_wait=keep)
                    for w in extra:
                        nop = InstNoOp(
                            name=f"{inst.name}-wsplit{nid[0]}", ins=[], outs=[])
                        nid[0] += 1
                        nop.engine = inst.engine
                        nop.sync_info = mybir.SyncInfo(
                            on_wait=[w], on_update=[])
                        out.append(nop)
                    changed = True
                out.append(inst)
            if changed:
                blk.instructions = out


def _patch_act_tables():
    """Force Bacc's activation-table chooser to the single set that
    contains every function this kernel uses (exp, identity, copy),
    so only one ACT_TABLE_LOAD (~2.7us each) is emitted."""
    import concourse.hw_specs as hw_specs
    import concourse.mybir as mybir
    if getattr(hw_specs.get_activation_tables, "_attn_patched", False):
        return
    orig = hw_specs.get_activation_tables
    keep = {mybir.ActivationFunctionType.Exp, mybir.ActivationFunctionType.Ln,
            mybir.ActivationFunctionType.Identity,
            mybir.ActivationFunctionType.Copy}

    import functools

    @functools.cache
    def patched(module_arch):
        tables = dict(orig(module_arch))
        out = {}
        for name, fns in tables.items():
            if name == "natural_log_exp_and_others":
                out[name] = fns
            else:
                out[name] = fns - keep
        return out

    patched._attn_patched = True
    hw_specs.get_activation_tables = patched
    import concourse.bacc as bacc_mod
    bacc_mod.get_activation_tables = patched


def build_nc():
    """Build the per-core Bass/Tile program."""
    _ensure_paths()
    _patch_act_tables()
    import concourse.bass as bass
    import concourse.mybir as mybir
    import concourse.tile as tile
    from concourse import bacc
    from contextlib import ExitStack

    f32 = mybir.dt.float32
    bf16 = mybir.dt.bfloat16
    f8e4 = mybir.dt.float8e4

    epi_mode = os.environ.get("ATTN_EPI", "pe")   # 'pe' | 'gps'
    nwarm = int(os.environ.get("ATTN_WARM", "16"))
    tailwarm = int(os.environ.get("ATTN_TAILWARM", "10"))
    itemwarm = int(os.environ.get("ATTN_ITEMWARM", "6"))
    at_bufs = int(os.environ.get("ATTN_AT_BUFS", "12"))
    ev8 = os.environ.get("ATTN_EV8", "1") == "1"   # DoubleRow fp8 E@v

    nc = bacc.Bacc("TRN2", target_bir_lowering=False, debug=False,
                   num_devices=NCORES)

    xT_ext = nc.declare_dram_parameter("xT", [C, N], bf16, isOutput=False)
    qkwT_ext = nc.declare_dram_parameter("qkwT", [C, 2 * C], bf16, isOutput=False)
    qkb_ext = nc.declare_dram_parameter("qkb", [P, 2 * C // P], f32, isOutput=False)
    vwT_ext = nc.declare_dram_parameter("vwT", [C, C], bf16, isOutput=False)
    vb_ext = nc.declare_dram_parameter("vb", [1, C], bf16, isOutput=False)
    at_ext = nc.declare_dram_parameter(
        "at", [NPAIR, NQT, NKT, P, 2 * QW], bf16, isOutput=False)
    pwT_ext = nc.declare_dram_parameter("pwT", [C, C], bf16, isOutput=False)
    pb_ext = nc.declare_dram_parameter("pb", [P, C // P], f32, isOutput=False)
    out_ext = nc.declare_dram_parameter("out", [C, N], bf16, isOutput=True)

    with tile.TileContext(nc, num_cores=NCORES) as tc, ExitStack() as ctx:
        consts = ctx.enter_context(tc.tile_pool(name="consts", bufs=1))
        persist = ctx.enter_context(tc.tile_pool(name="persist", bufs=1))
        attn_pool = ctx.enter_context(tc.tile_pool(name="attnout", bufs=1))
        epool = ctx.enter_context(tc.tile_pool(name="epool", bufs=2))
        atbf = ctx.enter_context(tc.tile_pool(name="atbf", bufs=at_bufs))
        small = ctx.enter_context(tc.tile_pool(name="small", bufs=2))

        qkb_sb = consts.tile([P, 2 * C // P], f32)
        pb_sb = consts.tile([P, NCIN], f32)
        vb_sb = consts.tile([1, C], bf16)
        # memsets for the warmup constants go on DVE: the gpsimd queue is
        # busy with the Tile prologue (sem clears) for the first ~3us and
        # would delay the HAM warmup matmuls
        ones_sb = consts.tile([1, P], bf16)
        nc.vector.memset(ones_sb[:], 1.0)
        ones64_sb = consts.tile([1, 64], bf16)
        nc.vector.memset(ones64_sb[:], 1.0)
        warm_sb = consts.tile([1, 512], bf16)
        nc.vector.memset(warm_sb[:], 0.0)

        # persistent activations (bf16 matmul operands)
        qkT_prs = [persist.tile([P, 2, N], bf16, tag=f"qkt{p}",
                                name=f"qkt{p}")
                   for p in range(NPAIR)]
        vp_sb = persist.tile([P, H, NKT, 65], bf16)   # [V_h | 1] stationary
        nc.any.memset(vp_sb[:, :, :, 64:65], 1.0)
        if ev8:
            # fp8 copy of [V_h | 1] with k-tile pairs interleaved along
            # the free axis for DoubleRow E@v (ko stride 80: %16 rule)
            vp8_sb = persist.tile([P, H, NKT // 2, 2, 80], f8e4)
            nc.any.memset(vp8_sb[:, :, :, :, 64:65], 1.0)
        pw_sb = persist.tile([P, NCIN, C], bf16)      # proj weights
        attn_sb = attn_pool.tile([P, NCIN, N], bf16)  # attention out^T

        if epi_mode == "gps":
            from concourse import library_config
            nc.gpsimd.load_library(library_config.attn)

        with tc.tile_pool(name="ph1", bufs=1) as ph1:
            xT_sb = ph1.tile([P, NCIN, N], bf16)
            qkw_sb = ph1.tile([P, NCIN, 2 * C], bf16)
            vw_sb = ph1.tile([P, NCIN, C], bf16)
            # direct bf16 DMA loads (host pre-casts).  Emission order on
            # the sync queue == descriptor order per DMA queue, so x/vw
            # stream first, then qkw; pw + at tiles are emitted later.
            xT_r = xT_ext.rearrange("(c p) t -> p c t", p=P)
            qkw_r = qkwT_ext.rearrange("(c p) n -> p c n", p=P)
            vw_r = vwT_ext.rearrange("(c p) n -> p c n", p=P)
            pw_r = pwT_ext.rearrange("(c p) n -> p c n", p=P)
            for kc in range(NCIN):
                nc.sync.dma_start(xT_sb[:, kc, :], xT_r[:, kc, :])
                nc.sync.dma_start(vw_sb[:, kc, :], vw_r[:, kc, :])
                if kc == 0:
                    nc.sync.dma_start(vb_sb[:], vb_ext[:])
                    nc.sync.dma_start(qkb_sb[:], qkb_ext[:])
                    nc.sync.dma_start(pb_sb[:], pb_ext[:])
            for kc in range(NCIN):
                nc.sync.dma_start(qkw_sb[:, kc, :], qkw_r[:, kc, :])

            # ---- HAM warm-up + V projection.  Const matmuls keep the PE
            # busy while the first x/vw chunks stream in, so the clock
            # gate opens to 8/8 (~3.4us sustained) before real work; a
            # few more const matmuls pad the DMA-paced first group so
            # the busy window stays unbroken. ----
            with tc.tile_pool(name="pp_w", bufs=1, space="PSUM") as pp_w, \
                 tc.tile_pool(name="pp_v", bufs=3, space="PSUM") as pp_v:
                wps = pp_w.tile([64, 512], f32)

                def warm_mm(n):
                    nc.tensor.matmul(
                        wps[:, 0:n], ones_sb[0:1, 0:64], warm_sb[0:1, 0:n],
                        start=True, stop=True, skip_group_check=True)

                for _ in range(nwarm):
                    warm_mm(256)

                for gi, tts in enumerate(((0, 1, 2), (3, 4, 5), (6, 7))):
                    pss = {tt: pp_v.tile([P, C], f32, tag="v",
                                         name=f"vps{tt}") for tt in tts}
                    if gi:
                        # pad the group seam (psum rotation wait on the
                        # previous group's DVE copies)
                        for _ in range(3):
                            warm_mm(512)
                    for kc in range(NCIN):
                        for tt in tts:
                            for (n0, nw) in ((0, QW), (QW, C - QW)):
                                nc.tensor.matmul(
                                    pss[tt][:, n0:n0 + nw],
                                    xT_sb[:, kc, tt * P:(tt + 1) * P],
                                    vw_sb[:, kc, n0:n0 + nw],
                                    start=(kc == 0), stop=False,
                                    skip_group_check=True)
                        if gi == 0 and kc < 5:
                            # absorb per-chunk DMA lateness so the HAM
                            # busy window is not broken
                            warm_mm(256)
                            warm_mm(256)
                    for tt in tts:
                        for (n0, nw) in ((0, QW), (QW, C - QW)):
                            nc.tensor.matmul(
                                pss[tt][:, n0:n0 + nw],
                                ones_sb[0:1, 0:P],
                                vb_sb[0:1, n0:n0 + nw],
                                start=False, stop=True,
                                skip_group_check=True)
                        nc.vector.tensor_copy(
                            vp_sb[:, :, tt, 0:64],
                            pss[tt].rearrange("p (h d) -> p h d", d=64))
                        if ev8:
                            nc.vector.tensor_copy(
                                vp8_sb[:, :, tt // 2, tt % 2, 0:64],
                                pss[tt].rearrange("p (h d) -> p h d", d=64))

            # ---- attention (+ interleaved qkT / proj work) ----
            with tc.tile_pool(name="pp_st", bufs=2, space="PSUM") as pp_st, \
                 tc.tile_pool(name="pp_ev", bufs=2, space="PSUM") as pp_ev, \
                     tc.tile_pool(name="pp_av", bufs=2, space="PSUM") as pp_av:

                def qkt_ct(ct, pr_dst, qki):
                    """One qkT output tile: 12 matmuls + DVE bias-add."""
                    ps = pp_st.tile([P, N], f32, tag="st", name=f"qk{ct}")
                    for kc in range(NCIN):
                        for qh in range(NQT):
                            nc.tensor.matmul(
                                ps[:, qh * QW:(qh + 1) * QW],
                                qkw_sb[:, kc, ct * P:(ct + 1) * P],
                                xT_sb[:, kc, qh * QW:(qh + 1) * QW],
                                start=(kc == 0), stop=(kc == NCIN - 1),
                                skip_group_check=True)
                    nc.vector.tensor_scalar_add(
                        qkT_prs[pr_dst][:, qki, :], ps[:, :],
                        qkb_sb[:, ct:ct + 1])

                def qkt_group(pr):
                    qkt_ct(pr, pr, 0)
                    qkt_ct(NPAIR + pr, pr, 1)

                def warm_into(ps, n=QW):
                    # const matmul into a psum region whose next real
                    # matmul is start=True (overwrites the garbage):
                    # keeps the PE busy so the HAM clock stays at 8/8
                    nc.tensor.matmul(
                        ps[0:64, 0:n], ones_sb[0:1, 0:64],
                        warm_sb[0:1, 0:n], start=True, stop=True,
                        skip_group_check=True)

                # dummy av-tag tile: a scratch psum bank for padding the
                # qkT-upfront stretch (paced by the qkw DMA stream)
                avw = pp_av.tile([P, QW], f32, tag="av", name="phasewarm")
                for ct_i, (ct, prd, qki) in enumerate(
                        ((0, 0, 0), (NPAIR, 0, 1), (1, 1, 0),
                         (NPAIR + 1, 1, 1))):
                    qkt_ct(ct, prd, qki)
                    for _ in range(3):
                        warm_into(avw)

                # proj weights stream after x/qkw/vw, before the at tiles
                for kc in range(NCIN):
                    nc.sync.dma_start(pw_sb[:, kc, :], pw_r[:, kc, :])

                def emit_st_step(pr, qt, e_sb, kt):
                    q0 = qt * QW
                    st = pp_st.tile([P, 2 * QW], f32, tag="st",
                                    name=f"st{pr}_{qt}_{kt}")
                    k0 = kt * P
                    nc.tensor.matmul(
                        st[:, 0:QW],
                        qkT_prs[pr][0:64, 1, k0:k0 + P],
                        qkT_prs[pr][0:64, 0, q0:q0 + QW],
                        start=True, stop=True)
                    nc.tensor.matmul(
                        st[:, QW:2 * QW],
                        qkT_prs[pr][64:128, 1, k0:k0 + P],
                        qkT_prs[pr][64:128, 0, q0:q0 + QW],
                        start=True, stop=True)
                    nc.scalar.activation(
                        e_sb[:, kt, :], st[:, :],
                        mybir.ActivationFunctionType.Exp, scale=SCALE)

                def emit_out_step(item, kt):
                    pr, qt, e_sb, psE1, psE2, psA = item
                    h1, h2 = 2 * pr, 2 * pr + 1
                    at = atbf.tile([P, 2 * QW], bf16, tag="atb",
                                   name=f"atb{pr}_{qt}_{kt}")
                    nc.sync.dma_start(at[:], at_ext[pr, qt, kt])
                    st_flags = dict(start=(kt == 0), stop=(kt == NKT - 1),
                                    skip_group_check=True)
                    nc.tensor.matmul(
                        psA[0:64, :], vp_sb[:, h1, kt, 0:64],
                        at[:, 0:QW], **st_flags)
                    nc.tensor.matmul(
                        psA[64:128, :], vp_sb[:, h2, kt, 0:64],
                        at[:, QW:2 * QW], **st_flags)
                    if ev8:
                        # DoubleRow fp8: one matmul covers a k-tile pair
                        if kt % 2 == 1:
                            g = kt // 2
                            dr_flags = dict(
                                start=(g == 0), stop=(g == NKT // 2 - 1),
                                perf_mode=mybir.MatmulPerfMode.DoubleRow,
                                skip_group_check=True)
                            nc.tensor.matmul(
                                psE1[0:65, :],
                                vp8_sb[:, h1, g, :, 0:65],
                                e_sb[:, 2 * g:2 * g + 2, 0:QW], **dr_flags)
                            nc.tensor.matmul(
                                psE2[0:65, :],
                                vp8_sb[:, h2, g, :, 0:65],
                                e_sb[:, 2 * g:2 * g + 2, QW:2 * QW],
                                **dr_flags)
                    else:
                        nc.tensor.matmul(
                            psE1[0:65, :], vp_sb[:, h1, kt, :],
                            e_sb[:, kt, 0:QW], **st_flags)
                        nc.tensor.matmul(
                            psE2[0:65, :], vp_sb[:, h2, kt, :],
                            e_sb[:, kt, QW:2 * QW], **st_flags)

                recip_mode = os.environ.get("ATTN_RECIP", "dve")

                def emit_epilogue_recip(item):
                    # 1/rowsum; runs while the next block's score
                    # matmuls keep the PE busy
                    pr, qt, e_sb, psE1, psE2, psA = item
                    rs = []
                    for hi, psE in ((0, psE1), (1, psE2)):
                        if recip_mode == "act":
                            lns = small.tile([1, QW], f32, tag="lns",
                                             name=f"ln{pr}_{qt}_{hi}")
                            nc.scalar.activation(
                                lns[:], psE[64:65, :],
                                mybir.ActivationFunctionType.Ln)
                            r16 = small.tile([1, QW], bf16, tag="r16",
                                             name=f"r16_{pr}_{qt}_{hi}")
                            nc.scalar.activation(
                                r16[:], lns[:],
                                mybir.ActivationFunctionType.Exp,
                                scale=-1.0)
                            rs.append(r16)
                            continue
                        # copy the rowsum to a partition-0 SBUF tile first:
                        # the custom-DVE recip mis-reads a partition-64
                        # PSUM operand (standard ops handle it fine)
                        rsum = small.tile([1, QW], f32, tag="rsum",
                                          name=f"rs_{pr}_{qt}_{hi}")
                        nc.vector.tensor_copy(rsum[:], psE[64:65, :])
                        r32 = small.tile([1, QW], f32, tag="r32",
                                         name=f"r32_{pr}_{qt}_{hi}")
                        nc.vector.reciprocal_approx_fast(r32[:], rsum[:])
                        if epi_mode == "gps":
                            rs.append(r32)
                        else:
                            r16 = small.tile([1, QW], bf16, tag="r16",
                                             name=f"r16_{pr}_{qt}_{hi}")
                            nc.vector.tensor_copy(r16[:], r32[:])
                            rs.append(r16)
                    return rs

                def emit_epilogue_apply(item, rs):
                    pr, qt, e_sb, psE1, psE2, psA = item
                    q0 = qt * QW
                    for hi, psE in ((0, psE1), (1, psE2)):
                        pa, pz = hi * 64, hi * 64 + 64
                        dst = attn_sb[pa:pz, pr, q0:q0 + QW]
                        if epi_mode == "gps":
                            rb = small.tile([64, QW], f32, tag="rb",
                                            name=f"rb{pr}_{qt}_{hi}")
                            nc.gpsimd.partition_broadcast(
                                rb[:], rs[hi][:], channels=64)
                            nc.vector.tensor_mul(dst, psE[0:64, :], rb[:])
                        else:
                            nc.tensor.matmul(psE[64:128, :],
                                             ones64_sb[0:1, :],
                                             rs[hi][:, :], start=True,
                                             stop=True,
                                             skip_group_check=True)
                            rb = small.tile([64, QW], f32, tag="rb",
                                            name=f"rb{pr}_{qt}_{hi}")
                            nc.vector.tensor_copy(rb[:], psE[64:128, :])
                            nc.vector.tensor_mul(dst, psE[0:64, :], rb[:])
                        nc.vector.tensor_add(dst, dst, psA[pa:pz, :])

                # qkT filler bursts inside the item loops: pairs 2..5,
                # each ct one-to-two items before its first use.  Item
                # (0,0) has no out-step work (pipeline fill), so it gets
                # two bursts.
                filler = {
                    (0, 0): [(2, 2, 0), (NPAIR + 2, 2, 1)],
                    (0, 1): [(3, 3, 0)],
                    (1, 0): [(NPAIR + 3, 3, 1)],
                    (1, 1): [(4, 4, 0)],
                    (2, 0): [(NPAIR + 4, 4, 1)],
                    (2, 1): [(5, 5, 0)],
                    (3, 0): [(NPAIR + 5, 5, 1)],
                }

                # software-pipelined emission: item i's ST/exp stream is
                # interleaved kt-by-kt with item i-1's E@v/A@v matmuls, so
                # the PE has dense work while ACT drains the score tiles
                items = [(pr, qt) for pr in range(NPAIR)
                         for qt in range(NQT)]
                prev = None        # item whose OUT runs in the current block
                pend = None        # pe-mode: (item, rs) awaiting PE/DVE apply
                e_dt = f8e4 if ev8 else bf16
                for pr, qt in items:
                    e_sb = epool.tile([P, NKT, 2 * QW], e_dt, tag="e",
                                      name=f"e{pr}_{qt}")
                    # two score steps up front cover the pending
                    # epilogue's DVE reciprocal latency
                    emit_st_step(pr, qt, e_sb, 0)
                    emit_st_step(pr, qt, e_sb, 1)
                    if pend is not None:
                        emit_epilogue_apply(*pend)
                        pend = None
                    psE1 = pp_ev.tile([P, QW], f32, tag="ev",
                                      name=f"ev1_{pr}_{qt}")
                    psE2 = pp_ev.tile([P, QW], f32, tag="ev",
                                      name=f"ev2_{pr}_{qt}")
                    psA = pp_av.tile([P, QW], f32, tag="av",
                                     name=f"av{pr}_{qt}")
                    cur = (pr, qt, e_sb, psE1, psE2, psA)
                    fill = list(filler.get((pr, qt), ()))
                    for kt in range(NKT):
                        if kt + 2 < NKT:
                            emit_st_step(pr, qt, e_sb, kt + 2)
                        if prev is not None:
                            emit_out_step(prev, kt)
                        if fill and kt in (2, 4):
                            qkt_ct(*fill.pop(0))
                    # item-boundary padding: the next item's first score
                    # steps wait on this item's last exps (ACT is the
                    # pacer once E@v runs DoubleRow); garbage written here
                    # is overwritten by this item's start=True A@v in the
                    # next block
                    i_next = items.index((pr, qt)) + 1
                    if i_next < len(items):
                        nwm = 2 if filler.get(items[i_next]) else itemwarm
                        for _ in range(nwm):
                            warm_into(psA)
                    if prev is not None:
                        rs = emit_epilogue_recip(prev)
                        if epi_mode == "gps":
                            emit_epilogue_apply(prev, rs)
                        else:
                            pend = (prev, rs)
                    prev = cur

                # ---- drain: last item's outs with proj partial bursts
                # interleaved, then the final epilogue and the output
                # projection ----
                def proj_partial(ps, ct, kcs, start, stop):
                    for kc in kcs:
                        for qh in range(NQT):
                            nc.tensor.matmul(
                                ps[:, qh * QW:(qh + 1) * QW],
                                pw_sb[:, kc, ct * P:(ct + 1) * P],
                                attn_sb[:, kc, qh * QW:(qh + 1) * QW],
                                start=(start and kc == kcs[0]),
                                stop=(stop and kc == kcs[-1]),
                                skip_group_check=True)

                with tc.tile_pool(name="ph3o", bufs=2) as ph3o:
                    out_r = out_ext.rearrange("(c p) t -> p c t", p=P)
                    pjs = {}

                    def proj_finish(ct, kcs, start):
                        ps = pjs[ct]
                        proj_partial(ps, ct, kcs, start, True)
                        o_sb = ph3o.tile([P, N], bf16, tag="o",
                                         name=f"o{ct}")
                        # alternate the bias-add between ACT and DVE so
                        # the six tail bias-adds run two-wide
                        if ct % 2 == 0:
                            nc.scalar.activation(
                                o_sb[:], ps[:],
                                mybir.ActivationFunctionType.Identity,
                                bias=pb_sb[:, ct:ct + 1])
                        else:
                            nc.vector.tensor_scalar_add(
                                o_sb[:], ps[:], pb_sb[:, ct:ct + 1])
                        nc.sync.dma_start(out_r[:, ct, :], o_sb[:])

                    for kt in range(NKT):
                        emit_out_step(prev, kt)
                        if kt == 0 and pend is not None:
                            emit_epilogue_apply(*pend)
                            pend = None
                        # the proj bursts recycle the st-score psum bufs;
                        # placed where the bufs actually free (exp kt6/kt7)
                        # so they don't stall the out-step stream
                        if kt == 5:
                            pjs[0] = pp_st.tile([P, N], f32, tag="st",
                                                name="proj0")
                            proj_partial(pjs[0], 0, list(range(NCIN - 1)),
                                         True, False)
                        if kt == 6:
                            pjs[1] = pp_st.tile([P, N], f32, tag="st",
                                                name="proj1")
                            proj_partial(pjs[1], 1, list(range(NCIN - 1)),
                                         True, False)
                    # const matmuls keep the PE busy (and the HAM clock
                    # warm) while the final epilogue chain runs on
                    # ACT/DVE; the proj finishes wait on it anyway
                    if tailwarm:
                        fav = pp_av.tile([P, QW], f32, tag="av",
                                         name="tailwarm")
                        for _ in range(tailwarm):
                            nc.tensor.matmul(
                                fav[0:64, :], ones_sb[0:1, 0:64],
                                warm_sb[0:1, :], start=True, stop=True,
                                skip_group_check=True)
                    rs = emit_epilogue_recip(prev)
                    emit_epilogue_apply(prev, rs)
                    proj_finish(0, [NCIN - 1], False)
                    proj_finish(1, [NCIN - 1], False)
                    for ct in range(2, NCIN):
                        pjs[ct] = pp_st.tile([P, N], f32, tag="st",
                                             name=f"proj{ct}")
                        proj_finish(ct, list(range(NCIN)), True)

    if os.environ.get("ATTN_DEDUP_LDW", "1") == "1":
        _dedup_ldweights(nc)
    if os.environ.get("ATTN_SPLIT_WAITS", "1") == "1":
        _split_excess_waits(nc)
    if not nc.is_finalized():
        nc.finalize()   # Bacc: move_matmul_waits + generate_event_semaphores
    return nc


def make_in_maps(x, qkv_w, qkv_b, static_a, proj_w, proj_b):
    """Host-side sharding / layout prep. One batch element per core."""
    x = np.asarray(x, dtype=np.float32)
    qkv_w = np.asarray(qkv_w, dtype=np.float32)
    qkv_b = np.asarray(qkv_b, dtype=np.float32)
    static_a = np.asarray(static_a, dtype=np.float32)
    proj_w = np.asarray(proj_w, dtype=np.float32)
    proj_b = np.asarray(proj_b, dtype=np.float32)

    import ml_dtypes
    bf16 = ml_dtypes.bfloat16

    qkwT = np.ascontiguousarray(qkv_w[0:2 * C].T).astype(bf16)  # [768, 1536]
    qkb = np.ascontiguousarray(
        qkv_b[0:2 * C].reshape(2 * C // P, P).T).astype(np.float32)
    vwT = np.ascontiguousarray(qkv_w[2 * C:3 * C].T).astype(bf16)
    vb = np.ascontiguousarray(
        qkv_b[2 * C:3 * C].reshape(1, C)).astype(bf16)
    # A^T strips, contiguous per (pair, qtile, ktile): [6, 2, 8, 128, 1024]
    # at[pr, qt, kt, :, 0:512] = A^T[2pr][kt tile, qt tile], [..., 512:] = head 2pr+1
    atT = static_a[0].transpose(0, 2, 1)                      # [H, k, q]
    at = np.ascontiguousarray(
        atT.reshape(NPAIR, 2, NKT, P, NQT, QW).transpose(0, 4, 2, 3, 1, 5)
        .reshape(NPAIR, NQT, NKT, P, 2 * QW)).astype(bf16)
    pwT = np.ascontiguousarray(proj_w.T).astype(bf16)
    pb = np.ascontiguousarray(
        proj_b.reshape(C // P, P).T).astype(np.float32)

    shared = {"qkwT": qkwT, "qkb": qkb, "vwT": vwT, "vb": vb,
              "at": at, "pwT": pwT, "pb": pb}
    in_maps = []
    for b in range(B):
        m = dict(shared)
        m["xT"] = np.ascontiguousarray(x[b].T).astype(bf16)
        in_maps.append(m)
    return in_maps


_NC_CACHE = {}


def _get_nc():
    if "nc" not in _NC_CACHE:
        _NC_CACHE["nc"] = build_nc()
    return _NC_CACHE["nc"]


def kernel(x, qkv_w, qkv_b, static_a, proj_w, proj_b):
    _ensure_paths()
    from concourse.bass_utils import run_bass_kernel_spmd

    nc = _get_nc()
    in_maps = make_in_maps(x, qkv_w, qkv_b, static_a, proj_w, proj_b)
    res = run_bass_kernel_spmd(nc, in_maps, core_ids=list(range(NCORES)))
    out = np.empty((B, N, C), dtype=np.float32)
    for b in range(B):
        out[b] = np.asarray(res.results[b]["out"], dtype=np.float32).T
    return out


# revision 27
# speedup vs baseline: 1.0366x; 1.0366x over previous
"""Trainium2 Bass kernel for nn_Attention_72438918414857.

Reference computation (B=8, N=1024, C=768, H=12, D=64):
    qkv = (x @ qkv_w.T + qkv_b) -> q, k, v per head
    attn = softmax(q @ k.T / sqrt(D)) + static_a   (bias added AFTER softmax)
    out = (attn @ v) merged-heads @ proj_w.T + proj_b

Sharding: data-parallel over batch -- one batch element per NeuronCore,
weights + static_a replicated. No collectives needed.

Math used on-chip (per batch, per head), everything transposed so each
matmul gets its contraction dim on partitions with no on-chip transposes:
    qkT = [Wq;Wk]^T-proj of x  ->  [cout, t] layout
    E^T = exp(K_h^T.T @ Q_h^T * D^-0.5)           [k, q] strips
    out_h^T = ([V_h|1].T @ E^T) -> rows 0..63 = E@v, row 64 = rowsum(E)
    attn_h^T = (E@v) * (1/rowsum) + V_h.T @ A_h^T
where static_a is pre-transposed on host to A^T[h, k, q].  The softmax
normalization is applied to the [64, q] output instead of the [k, q]
matrix; no max-subtraction is needed (|scores*scale| < ~3).

Matmuls run in bf16 (fp32 PE matmul is 4x slower); PSUM accumulation is
fp32.  bf16 rounding of operands keeps rel-err ~4e-3, well under the
2e-2 gate.

v2 scheduling changes (over the first working version):
  - warm-up matmuls on const data at t=0 so the PE HAM clock-gate
    reaches 8/8 before the real work starts (first ~20us of v1 ran at
    1.2 GHz)
  - V projection runs kc-outer in two tt-quads so the first x/vw DMA
    chunk immediately yields dense PE work
  - qkT projections for pairs 2..5 are emitted as per-item filler
    bursts inside the attention loop, filling the exp-gated PE bubbles
    at item boundaries
  - softmax reciprocal on DVE (reciprocal_approx_fast) instead of the
    ACT Ln/Exp chain (saves ~33us of ACT time; ACT runs only the big
    exps)
  - drain: proj partial bursts interleaved into the last item's
    out-steps; output stored/DMA'd as bf16
"""

import os
import sys

import numpy as np

B, N, C = 8, 1024, 768
H, D = 12, 64
NCORES = 8
P = 128
QW = 512          # q tile width (PSUM bank = 512 f32)
NQT = N // QW     # 2 q tiles
NKT = N // P      # 8 k tiles
NCIN = C // P     # 6 c_in chunks
NPAIR = H // 2    # 6 head pairs
SCALE = float(D) ** -0.5

_REPO = "/opt/trn_rl_repo"


def _ensure_paths():
    if _REPO not in sys.path:
        sys.path.insert(0, _REPO)


def _dedup_ldweights(nc):
    """Delete an Ldweights whose weights AP + tile geometry match the
    immediately preceding Ldweights on the PE stream (the weights are
    still resident in the array); its waits/updates move to the next
    instruction."""
    import concourse.mybir as mybir

    def sig(inst):
        ap = inst.ins[0]
        return (str(ap), str(getattr(inst, "tile_position", None)),
                str(getattr(inst, "tile_size", None)))

    for fn in nc.m.functions:
        for blk in fn.blocks:
            out = []
            last_sig = None
            pend_w, pend_u = [], []
            changed = False
            for inst in blk.instructions:
                op = str(inst.opcode)
                if op == "Ldweights":
                    s_ = sig(inst)
                    if s_ == last_sig:
                        si = inst.sync_info
                        if si:
                            pend_w.extend(si.on_wait or [])
                            pend_u.extend(si.on_update or [])
                        changed = True
                        continue
                    last_sig = s_
                elif op == "Matmult":
                    pass          # matmuls don't disturb loaded weights
                elif op in ("NoOp", "EventSemaphore"):
                    pass
                else:
                    last_sig = None
                if pend_w or pend_u:
                    si = inst.sync_info
                    ow = list(si.on_wait or []) if si else []
                    ou = list(si.on_update or []) if si else []
                    inst.sync_info = mybir.SyncInfo(
                        on_wait=pend_w + ow, on_update=pend_u + ou)
                    pend_w, pend_u = [], []
                out.append(inst)
            assert not pend_w and not pend_u
            if changed:
                blk.instructions = out


def _split_excess_waits(nc):
    """The TRN2 walrus codegen allows only 1 sem-wait command per
    instruction.  Tile's sem-assigner can emit more (one per logical
    proc a tile depends on).
    Move the excess onto freshly inserted same-engine NoOps placed just
    before the instruction -- engines execute in order, so waiting on a
    preceding NoOp is equivalent."""
    import concourse.mybir as mybir
    from bass_rust import InstNoOp

    nid = [0]
    for fn in nc.m.functions:
        for blk in fn.blocks:
            out = []
            changed = False
            for inst in blk.instructions:
                si = inst.sync_info
                waits = list(si.on_wait) if si and si.on_wait else []
                limit = 1
                if len(waits) > limit:
                    extra, keep = waits[:-limit], waits[-limit:]
                    inst.sync_info = si.__replace__(on_wait=keep)
                    for w in extra:
                        nop = InstNoOp(
                            name=f"{inst.name}-wsplit{nid[0]}", ins=[], outs=[])
                        nid[0] += 1
                        nop.engine = inst.engine
                        nop.sync_info = mybir.SyncInfo(
                            on_wait=[w], on_update=[])
                        out.append(nop)
                    changed = True
                out.append(inst)
            if changed:
                blk.instructions = out


def _patch_act_tables():
    """Force Bacc's activation-table chooser to the single set that
    contains every function this kernel uses (exp, identity, copy),
    so only one ACT_TABLE_LOAD (~2.7us each) is emitted."""
    import concourse.hw_specs as hw_specs
    import concourse.mybir as mybir
    if getattr(hw_specs.get_activation_tables, "_attn_patched", False):
        return
    orig = hw_specs.get_activation_tables
    keep = {mybir.ActivationFunctionType.Exp, mybir.ActivationFunctionType.Ln,
            mybir.ActivationFunctionType.Identity,
            mybir.ActivationFunctionType.Copy}

    import functools

    @functools.cache
    def patched(module_arch):
        tables = dict(orig(module_arch))
        out = {}
        for name, fns in tables.items():
            if name == "natural_log_exp_and_others":
                out[name] = fns
            else:
                out[name] = fns - keep
        return out

    patched._attn_patched = True
    hw_specs.get_activation_tables = patched
    import concourse.bacc as bacc_mod
    bacc_mod.get_activation_tables = patched


def build_nc():
    """Build the per-core Bass/Tile program."""
    _ensure_paths()
    _patch_act_tables()
    import concourse.bass as bass
    import concourse.mybir as mybir
    import concourse.tile as tile
    from concourse import bacc
    from contextlib import ExitStack

    f32 = mybir.dt.float32
    bf16 = mybir.dt.bfloat16
    f8e4 = mybir.dt.float8e4

    epi_mode = os.environ.get("ATTN_EPI", "pe")   # 'pe' | 'gps'
    nwarm = int(os.environ.get("ATTN_WARM", "16"))
    tailwarm = int(os.environ.get("ATTN_TAILWARM", "10"))
    itemwarm = int(os.environ.get("ATTN_ITEMWARM", "6"))
    at_bufs = int(os.environ.get("ATTN_AT_BUFS", "12"))
    ev8 = os.environ.get("ATTN_EV8", "1") == "1"   # DoubleRow fp8 E@v

    nc = bacc.Bacc("TRN2", target_bir_lowering=False, debug=False,
                   num_devices=NCORES)

    xT_ext = nc.declare_dram_parameter("xT", [C, N], bf16, isOutput=False)
    qkwT_ext = nc.declare_dram_parameter("qkwT", [C, 2 * C], bf16, isOutput=False)
    qkb_ext = nc.declare_dram_parameter("qkb", [P, 2 * C // P], f32, isOutput=False)
    vwT_ext = nc.declare_dram_parameter("vwT", [C, C], bf16, isOutput=False)
    vb_ext = nc.declare_dram_parameter("vb", [1, C], bf16, isOutput=False)
    at_ext = nc.declare_dram_parameter(
        "at", [NPAIR, NQT, NKT, P, 2 * QW], bf16, isOutput=False)
    pwT_ext = nc.declare_dram_parameter("pwT", [C, C], bf16, isOutput=False)
    pb_ext = nc.declare_dram_parameter("pb", [P, C // P], f32, isOutput=False)
    out_ext = nc.declare_dram_parameter("out", [C, N], bf16, isOutput=True)

    with tile.TileContext(nc, num_cores=NCORES) as tc, ExitStack() as ctx:
        consts = ctx.enter_context(tc.tile_pool(name="consts", bufs=1))
        persist = ctx.enter_context(tc.tile_pool(name="persist", bufs=1))
        attn_pool = ctx.enter_context(tc.tile_pool(name="attnout", bufs=1))
        epool = ctx.enter_context(tc.tile_pool(name="epool", bufs=2))
        atbf = ctx.enter_context(tc.tile_pool(name="atbf", bufs=at_bufs))
        small = ctx.enter_context(tc.tile_pool(name="small", bufs=2))

        qkb_sb = consts.tile([P, 2 * C // P], f32)
        pb_sb = consts.tile([P, NCIN], f32)
        vb_sb = consts.tile([1, C], bf16)
        # memsets for the warmup constants go on DVE: the gpsimd queue is
        # busy with the Tile prologue (sem clears) for the first ~3us and
        # would delay the HAM warmup matmuls
        ones_sb = consts.tile([1, P], bf16)
        nc.vector.memset(ones_sb[:], 1.0)
        ones64_sb = consts.tile([1, 64], bf16)
        nc.vector.memset(ones64_sb[:], 1.0)
        # warm matmuls must engage the full 128-row array: the HAM
        # activity monitor does not register K=1 matmuls as PE-busy
        ww_sb = consts.tile([P, 64], bf16)
        nc.vector.memset(ww_sb[:], 0.0)
        wr_sb = consts.tile([P, 512], bf16)
        nc.vector.memset(wr_sb[:], 0.0)

        # persistent activations (bf16 matmul operands)
        qkT_prs = [persist.tile([P, 2, N], bf16, tag=f"qkt{p}",
                                name=f"qkt{p}")
                   for p in range(NPAIR)]
        vp_sb = persist.tile([P, H, NKT, 65], bf16)   # [V_h | 1] stationary
        nc.any.memset(vp_sb[:, :, :, 64:65], 1.0)
        if ev8:
            # fp8 copy of [V_h | 1] with k-tile pairs interleaved along
            # the free axis for DoubleRow E@v (ko stride 80: %16 rule)
            vp8_sb = persist.tile([P, H, NKT // 2, 2, 80], f8e4)
            nc.any.memset(vp8_sb[:, :, :, :, 64:65], 1.0)
        pw_sb = persist.tile([P, NCIN, C], bf16)      # proj weights
        attn_sb = attn_pool.tile([P, NCIN, N], bf16)  # attention out^T

        if epi_mode == "gps":
            from concourse import library_config
            nc.gpsimd.load_library(library_config.attn)

        with tc.tile_pool(name="ph1", bufs=1) as ph1:
            xT_sb = ph1.tile([P, NCIN, N], bf16)
            qkw_sb = ph1.tile([P, NCIN, 2 * C], bf16)
            vw_sb = ph1.tile([P, NCIN, C], bf16)
            # direct bf16 DMA loads (host pre-casts).  Emission order on
            # the sync queue == descriptor order per DMA queue, so x/vw
            # stream first, then qkw; pw + at tiles are emitted later.
            xT_r = xT_ext.rearrange("(c p) t -> p c t", p=P)
            qkw_r = qkwT_ext.rearrange("(c p) n -> p c n", p=P)
            vw_r = vwT_ext.rearrange("(c p) n -> p c n", p=P)
            pw_r = pwT_ext.rearrange("(c p) n -> p c n", p=P)
            for kc in range(NCIN):
                nc.sync.dma_start(xT_sb[:, kc, :], xT_r[:, kc, :])
                nc.sync.dma_start(vw_sb[:, kc, :], vw_r[:, kc, :])
                if kc == 0:
                    nc.sync.dma_start(vb_sb[:], vb_ext[:])
                    nc.sync.dma_start(qkb_sb[:], qkb_ext[:])
                    nc.sync.dma_start(pb_sb[:], pb_ext[:])
            for kc in range(NCIN):
                nc.sync.dma_start(qkw_sb[:, kc, :], qkw_r[:, kc, :])

            # ---- HAM warm-up + V projection.  Const matmuls keep the PE
            # busy while the first x/vw chunks stream in, so the clock
            # gate opens to 8/8 (~3.4us sustained) before real work; a
            # few more const matmuls pad the DMA-paced first group so
            # the busy window stays unbroken. ----
            with tc.tile_pool(name="pp_w", bufs=1, space="PSUM") as pp_w, \
                 tc.tile_pool(name="pp_v", bufs=3, space="PSUM") as pp_v:
                wps = pp_w.tile([64, 512], f32)

                def warm_mm(n):
                    nc.tensor.matmul(
                        wps[:, 0:n], ww_sb[:, 0:64], wr_sb[:, 0:n],
                        start=True, stop=True, skip_group_check=True)

                for _ in range(nwarm):
                    warm_mm(256)

                for gi, tts in enumerate(((0, 1, 2), (3, 4, 5), (6, 7))):
                    pss = {tt: pp_v.tile([P, C], f32, tag="v",
                                         name=f"vps{tt}") for tt in tts}
                    if gi:
                        # pad the group seam (psum rotation wait on the
                        # previous group's DVE copies)
                        for _ in range(3):
                            warm_mm(512)
                    for kc in range(NCIN):
                        for tt in tts:
                            for (n0, nw) in ((0, QW), (QW, C - QW)):
                                nc.tensor.matmul(
                                    pss[tt][:, n0:n0 + nw],
                                    xT_sb[:, kc, tt * P:(tt + 1) * P],
                                    vw_sb[:, kc, n0:n0 + nw],
                                    start=(kc == 0), stop=False,
                                    skip_group_check=True)
                        if gi == 0 and kc < 5:
                            # absorb per-chunk DMA lateness so the HAM
                            # busy window is not broken
                            warm_mm(256)
                            warm_mm(256)
                    for tt in tts:
                        for (n0, nw) in ((0, QW), (QW, C - QW)):
                            nc.tensor.matmul(
                                pss[tt][:, n0:n0 + nw],
                                ones_sb[0:1, 0:P],
                                vb_sb[0:1, n0:n0 + nw],
                                start=False, stop=True,
                                skip_group_check=True)
                        nc.vector.tensor_copy(
                            vp_sb[:, :, tt, 0:64],
                            pss[tt].rearrange("p (h d) -> p h d", d=64))
                        if ev8:
                            nc.vector.tensor_copy(
                                vp8_sb[:, :, tt // 2, tt % 2, 0:64],
                                pss[tt].rearrange("p (h d) -> p h d", d=64))

            # ---- attention (+ interleaved qkT / proj work) ----
            with tc.tile_pool(name="pp_st", bufs=2, space="PSUM") as pp_st, \
                 tc.tile_pool(name="pp_ev", bufs=2, space="PSUM") as pp_ev, \
                     tc.tile_pool(name="pp_av", bufs=2, space="PSUM") as pp_av:

                def qkt_ct(ct, pr_dst, qki):
                    """One qkT output tile: 12 matmuls + DVE bias-add."""
                    ps = pp_st.tile([P, N], f32, tag="st", name=f"qk{ct}")
                    for kc in range(NCIN):
                        for qh in range(NQT):
                            nc.tensor.matmul(
                                ps[:, qh * QW:(qh + 1) * QW],
                                qkw_sb[:, kc, ct * P:(ct + 1) * P],
                                xT_sb[:, kc, qh * QW:(qh + 1) * QW],
                                start=(kc == 0), stop=(kc == NCIN - 1),
                                skip_group_check=True)
                    nc.vector.tensor_scalar_add(
                        qkT_prs[pr_dst][:, qki, :], ps[:, :],
                        qkb_sb[:, ct:ct + 1])

                def qkt_group(pr):
                    qkt_ct(pr, pr, 0)
                    qkt_ct(NPAIR + pr, pr, 1)

                def warm_into(ps, n=QW):
                    # full-K const matmul into a psum region whose next
                    # real matmul is start=True (overwrites the garbage):
                    # keeps the PE busy so the HAM clock stays at 8/8
                    nc.tensor.matmul(
                        ps[0:64, 0:n], ww_sb[:, 0:64],
                        wr_sb[:, 0:n], start=True, stop=True,
                        skip_group_check=True)

                # dummy av-tag tile: a scratch psum bank for padding the
                # qkT-upfront stretch (paced by the qkw DMA stream)
                avw = pp_av.tile([P, QW], f32, tag="av", name="phasewarm")
                for ct_i, (ct, prd, qki) in enumerate(
                        ((0, 0, 0), (NPAIR, 0, 1), (1, 1, 0),
                         (NPAIR + 1, 1, 1))):
                    qkt_ct(ct, prd, qki)
                    for _ in range(3):
                        warm_into(avw)

                # proj weights stream after x/qkw/vw, before the at tiles
                for kc in range(NCIN):
                    nc.sync.dma_start(pw_sb[:, kc, :], pw_r[:, kc, :])

                def emit_st_step(pr, qt, e_sb, kt):
                    q0 = qt * QW
                    st = pp_st.tile([P, 2 * QW], f32, tag="st",
                                    name=f"st{pr}_{qt}_{kt}")
                    k0 = kt * P
                    nc.tensor.matmul(
                        st[:, 0:QW],
                        qkT_prs[pr][0:64, 1, k0:k0 + P],
                        qkT_prs[pr][0:64, 0, q0:q0 + QW],
                        start=True, stop=True)
                    nc.tensor.matmul(
                        st[:, QW:2 * QW],
                        qkT_prs[pr][64:128, 1, k0:k0 + P],
                        qkT_prs[pr][64:128, 0, q0:q0 + QW],
                        start=True, stop=True)
                    nc.scalar.activation(
                        e_sb[:, kt, :], st[:, :],
                        mybir.ActivationFunctionType.Exp, scale=SCALE)

                def emit_out_step(item, kt):
                    pr, qt, e_sb, psE1, psE2, psA = item
                    h1, h2 = 2 * pr, 2 * pr + 1
                    at = atbf.tile([P, 2 * QW], bf16, tag="atb",
                                   name=f"atb{pr}_{qt}_{kt}")
                    nc.sync.dma_start(at[:], at_ext[pr, qt, kt])
                    st_flags = dict(start=(kt == 0), stop=(kt == NKT - 1),
                                    skip_group_check=True)
                    nc.tensor.matmul(
                        psA[0:64, :], vp_sb[:, h1, kt, 0:64],
                        at[:, 0:QW], **st_flags)
                    nc.tensor.matmul(
                        psA[64:128, :], vp_sb[:, h2, kt, 0:64],
                        at[:, QW:2 * QW], **st_flags)
                    if ev8:
                        # DoubleRow fp8: one matmul covers a k-tile pair
                        if kt % 2 == 1:
                            g = kt // 2
                            dr_flags = dict(
                                start=(g == 0), stop=(g == NKT // 2 - 1),
                                perf_mode=mybir.MatmulPerfMode.DoubleRow,
                                skip_group_check=True)
                            nc.tensor.matmul(
                                psE1[0:65, :],
                                vp8_sb[:, h1, g, :, 0:65],
                                e_sb[:, 2 * g:2 * g + 2, 0:QW], **dr_flags)
                            nc.tensor.matmul(
                                psE2[0:65, :],
                                vp8_sb[:, h2, g, :, 0:65],
                                e_sb[:, 2 * g:2 * g + 2, QW:2 * QW],
                                **dr_flags)
                    else:
                        nc.tensor.matmul(
                            psE1[0:65, :], vp_sb[:, h1, kt, :],
                            e_sb[:, kt, 0:QW], **st_flags)
                        nc.tensor.matmul(
                            psE2[0:65, :], vp_sb[:, h2, kt, :],
                            e_sb[:, kt, QW:2 * QW], **st_flags)

                recip_mode = os.environ.get("ATTN_RECIP", "dve")

                def emit_epilogue_recip(item):
                    # 1/rowsum; runs while the next block's score
                    # matmuls keep the PE busy
                    pr, qt, e_sb, psE1, psE2, psA = item
                    rs = []
                    for hi, psE in ((0, psE1), (1, psE2)):
                        if recip_mode == "act":
                            lns = small.tile([1, QW], f32, tag="lns",
                                             name=f"ln{pr}_{qt}_{hi}")
                            nc.scalar.activation(
                                lns[:], psE[64:65, :],
                                mybir.ActivationFunctionType.Ln)
                            r16 = small.tile([1, QW], bf16, tag="r16",
                                             name=f"r16_{pr}_{qt}_{hi}")
                            nc.scalar.activation(
                                r16[:], lns[:],
                                mybir.ActivationFunctionType.Exp,
                                scale=-1.0)
                            rs.append(r16)
                            continue
                        # copy the rowsum to a partition-0 SBUF tile first:
                        # the custom-DVE recip mis-reads a partition-64
                        # PSUM operand (standard ops handle it fine)
                        rsum = small.tile([1, QW], f32, tag="rsum",
                                          name=f"rs_{pr}_{qt}_{hi}")
                        nc.vector.tensor_copy(rsum[:], psE[64:65, :])
                        r32 = small.tile([1, QW], f32, tag="r32",
                                         name=f"r32_{pr}_{qt}_{hi}")
                        nc.vector.reciprocal_approx_fast(r32[:], rsum[:])
                        if epi_mode == "gps":
                            rs.append(r32)
                        else:
                            r16 = small.tile([1, QW], bf16, tag="r16",
                                             name=f"r16_{pr}_{qt}_{hi}")
                            nc.vector.tensor_copy(r16[:], r32[:])
                            rs.append(r16)
                    return rs

                def emit_epilogue_apply(item, rs):
                    pr, qt, e_sb, psE1, psE2, psA = item
                    q0 = qt * QW
                    for hi, psE in ((0, psE1), (1, psE2)):
                        pa, pz = hi * 64, hi * 64 + 64
                        dst = attn_sb[pa:pz, pr, q0:q0 + QW]
                        if epi_mode == "gps":
                            rb = small.tile([64, QW], f32, tag="rb",
                                            name=f"rb{pr}_{qt}_{hi}")
                            nc.gpsimd.partition_broadcast(
                                rb[:], rs[hi][:], channels=64)
                            nc.vector.tensor_mul(dst, psE[0:64, :], rb[:])
                        else:
                            nc.tensor.matmul(psE[64:128, :],
                                             ones64_sb[0:1, :],
                                             rs[hi][:, :], start=True,
                                             stop=True,
                                             skip_group_check=True)
                            rb = small.tile([64, QW], f32, tag="rb",
                                            name=f"rb{pr}_{qt}_{hi}")
                            nc.vector.tensor_copy(rb[:], psE[64:128, :])
                            nc.vector.tensor_mul(dst, psE[0:64, :], rb[:])
                        nc.vector.tensor_add(dst, dst, psA[pa:pz, :])

                # qkT filler bursts inside the item loops: pairs 2..5,
                # each ct one-to-two items before its first use.  Item
                # (0,0) has no out-step work (pipeline fill), so it gets
                # two bursts.
                filler = {
                    (0, 0): [(2, 2, 0), (NPAIR + 2, 2, 1)],
                    (0, 1): [(3, 3, 0)],
                    (1, 0): [(NPAIR + 3, 3, 1)],
                    (1, 1): [(4, 4, 0)],
                    (2, 0): [(NPAIR + 4, 4, 1)],
                    (2, 1): [(5, 5, 0)],
                    (3, 0): [(NPAIR + 5, 5, 1)],
                }

                # software-pipelined emission: item i's ST/exp stream is
                # interleaved kt-by-kt with item i-1's E@v/A@v matmuls, so
                # the PE has dense work while ACT drains the score tiles
                items = [(pr, qt) for pr in range(NPAIR)
                         for qt in range(NQT)]
                prev = None        # item whose OUT runs in the current block
                pend = None        # pe-mode: (item, rs) awaiting PE/DVE apply
                e_dt = f8e4 if ev8 else bf16
                for pr, qt in items:
                    e_sb = epool.tile([P, NKT, 2 * QW], e_dt, tag="e",
                                      name=f"e{pr}_{qt}")
                    # two score steps up front cover the pending
                    # epilogue's DVE reciprocal latency
                    emit_st_step(pr, qt, e_sb, 0)
                    emit_st_step(pr, qt, e_sb, 1)
                    if pend is not None:
                        emit_epilogue_apply(*pend)
                        pend = None
                    psE1 = pp_ev.tile([P, QW], f32, tag="ev",
                                      name=f"ev1_{pr}_{qt}")
                    psE2 = pp_ev.tile([P, QW], f32, tag="ev",
                                      name=f"ev2_{pr}_{qt}")
                    psA = pp_av.tile([P, QW], f32, tag="av",
                                     name=f"av{pr}_{qt}")
                    cur = (pr, qt, e_sb, psE1, psE2, psA)
                    fill = list(filler.get((pr, qt), ()))
                    for kt in range(NKT):
                        if kt + 2 < NKT:
                            emit_st_step(pr, qt, e_sb, kt + 2)
                        if prev is not None:
                            emit_out_step(prev, kt)
                        if fill and kt in (2, 4):
                            qkt_ct(*fill.pop(0))
                    # item-boundary padding: the next item's first score
                    # steps wait on this item's last exps (ACT is the
                    # pacer once E@v runs DoubleRow); garbage written here
                    # is overwritten by this item's start=True A@v in the
                    # next block
                    i_next = items.index((pr, qt)) + 1
                    if i_next < len(items):
                        nwm = 2 if filler.get(items[i_next]) else itemwarm
                        for _ in range(nwm):
                            warm_into(psA)
                    if prev is not None:
                        rs = emit_epilogue_recip(prev)
                        if epi_mode == "gps":
                            emit_epilogue_apply(prev, rs)
                        else:
                            pend = (prev, rs)
                    prev = cur

                # ---- drain: last item's outs with proj partial bursts
                # interleaved, then the final epilogue and the output
                # projection ----
                def proj_partial(ps, ct, kcs, start, stop):
                    for kc in kcs:
                        for qh in range(NQT):
                            nc.tensor.matmul(
                                ps[:, qh * QW:(qh + 1) * QW],
                                pw_sb[:, kc, ct * P:(ct + 1) * P],
                                attn_sb[:, kc, qh * QW:(qh + 1) * QW],
                                start=(start and kc == kcs[0]),
                                stop=(stop and kc == kcs[-1]),
                                skip_group_check=True)

                with tc.tile_pool(name="ph3o", bufs=2) as ph3o:
                    out_r = out_ext.rearrange("(c p) t -> p c t", p=P)
                    pjs = {}

                    def proj_finish(ct, kcs, start):
                        ps = pjs[ct]
                        proj_partial(ps, ct, kcs, start, True)
                        o_sb = ph3o.tile([P, N], bf16, tag="o",
                                         name=f"o{ct}")
                        # alternate the bias-add between ACT and DVE so
                        # the six tail bias-adds run two-wide
                        if ct % 2 == 0:
                            nc.scalar.activation(
                                o_sb[:], ps[:],
                                mybir.ActivationFunctionType.Identity,
                                bias=pb_sb[:, ct:ct + 1])
                        else:
                            nc.vector.tensor_scalar_add(
                                o_sb[:], ps[:], pb_sb[:, ct:ct + 1])
                        nc.sync.dma_start(out_r[:, ct, :], o_sb[:])

                    for kt in range(NKT):
                        emit_out_step(prev, kt)
                        if kt == 0 and pend is not None:
                            emit_epilogue_apply(*pend)
                            pend = None
                        # the proj bursts recycle the st-score psum bufs;
                        # placed where the bufs actually free (exp kt6/kt7)
                        # so they don't stall the out-step stream
                        if kt == 5:
                            pjs[0] = pp_st.tile([P, N], f32, tag="st",
                                                name="proj0")
                            proj_partial(pjs[0], 0, list(range(NCIN - 1)),
                                         True, False)
                        if kt == 6:
                            pjs[1] = pp_st.tile([P, N], f32, tag="st",
                                                name="proj1")
                            proj_partial(pjs[1], 1, list(range(NCIN - 1)),
                                         True, False)
                    # const matmuls keep the PE busy (and the HAM clock
                    # warm) while the final epilogue chain runs on
                    # ACT/DVE; the proj finishes wait on it anyway
                    if tailwarm:
                        fav = pp_av.tile([P, QW], f32, tag="av",
                                         name="tailwarm")
                        for _ in range(tailwarm):
                            nc.tensor.matmul(
                                fav[0:64, :], ww_sb[:, 0:64],
                                wr_sb[:, :], start=True, stop=True,
                                skip_group_check=True)
                    rs = emit_epilogue_recip(prev)
                    emit_epilogue_apply(prev, rs)
                    proj_finish(0, [NCIN - 1], False)
                    proj_finish(1, [NCIN - 1], False)
                    for ct in range(2, NCIN):
                        pjs[ct] = pp_st.tile([P, N], f32, tag="st",
                                             name=f"proj{ct}")
                        proj_finish(ct, list(range(NCIN)), True)

    if os.environ.get("ATTN_DEDUP_LDW", "1") == "1":
        _dedup_ldweights(nc)
    if os.environ.get("ATTN_SPLIT_WAITS", "1") == "1":
        _split_excess_waits(nc)
    if not nc.is_finalized():
        nc.finalize()   # Bacc: move_matmul_waits + generate_event_semaphores
    return nc


def make_in_maps(x, qkv_w, qkv_b, static_a, proj_w, proj_b):
    """Host-side sharding / layout prep. One batch element per core."""
    x = np.asarray(x, dtype=np.float32)
    qkv_w = np.asarray(qkv_w, dtype=np.float32)
    qkv_b = np.asarray(qkv_b, dtype=np.float32)
    static_a = np.asarray(static_a, dtype=np.float32)
    proj_w = np.asarray(proj_w, dtype=np.float32)
    proj_b = np.asarray(proj_b, dtype=np.float32)

    import ml_dtypes
    bf16 = ml_dtypes.bfloat16

    qkwT = np.ascontiguousarray(qkv_w[0:2 * C].T).astype(bf16)  # [768, 1536]
    qkb = np.ascontiguousarray(
        qkv_b[0:2 * C].reshape(2 * C // P, P).T).astype(np.float32)
    vwT = np.ascontiguousarray(qkv_w[2 * C:3 * C].T).astype(bf16)
    vb = np.ascontiguousarray(
        qkv_b[2 * C:3 * C].reshape(1, C)).astype(bf16)
    # A^T strips, contiguous per (pair, qtile, ktile): [6, 2, 8, 128, 1024]
    # at[pr, qt, kt, :, 0:512] = A^T[2pr][kt tile, qt tile], [..., 512:] = head 2pr+1
    atT = static_a[0].transpose(0, 2, 1)                      # [H, k, q]
    at = np.ascontiguousarray(
        atT.reshape(NPAIR, 2, NKT, P, NQT, QW).transpose(0, 4, 2, 3, 1, 5)
        .reshape(NPAIR, NQT, NKT, P, 2 * QW)).astype(bf16)
    pwT = np.ascontiguousarray(proj_w.T).astype(bf16)
    pb = np.ascontiguousarray(
        proj_b.reshape(C // P, P).T).astype(np.float32)

    shared = {"qkwT": qkwT, "qkb": qkb, "vwT": vwT, "vb": vb,
              "at": at, "pwT": pwT, "pb": pb}
    in_maps = []
    for b in range(B):
        m = dict(shared)
        m["xT"] = np.ascontiguousarray(x[b].T).astype(bf16)
        in_maps.append(m)
    return in_maps


_NC_CACHE = {}


def _get_nc():
    if "nc" not in _NC_CACHE:
        _NC_CACHE["nc"] = build_nc()
    return _NC_CACHE["nc"]


def kernel(x, qkv_w, qkv_b, static_a, proj_w, proj_b):
    _ensure_paths()
    from concourse.bass_utils import run_bass_kernel_spmd

    nc = _get_nc()
    in_maps = make_in_maps(x, qkv_w, qkv_b, static_a, proj_w, proj_b)
    res = run_bass_kernel_spmd(nc, in_maps, core_ids=list(range(NCORES)))
    out = np.empty((B, N, C), dtype=np.float32)
    for b in range(B):
        out[b] = np.asarray(res.results[b]["out"], dtype=np.float32).T
    return out


# revision 30
# speedup vs baseline: 1.0696x; 1.0318x over previous
"""Trainium2 Bass kernel for nn_Attention_72438918414857.

Reference computation (B=8, N=1024, C=768, H=12, D=64):
    qkv = (x @ qkv_w.T + qkv_b) -> q, k, v per head
    attn = softmax(q @ k.T / sqrt(D)) + static_a   (bias added AFTER softmax)
    out = (attn @ v) merged-heads @ proj_w.T + proj_b

Sharding: data-parallel over batch -- one batch element per NeuronCore,
weights + static_a replicated. No collectives needed.

Math used on-chip (per batch, per head), everything transposed so each
matmul gets its contraction dim on partitions with no on-chip transposes:
    qkT = [Wq;Wk]^T-proj of x  ->  [cout, t] layout
    E^T = exp(K_h^T.T @ Q_h^T * D^-0.5)           [k, q] strips
    out_h^T = ([V_h|1].T @ E^T) -> rows 0..63 = E@v, row 64 = rowsum(E)
    attn_h^T = (E@v) * (1/rowsum) + V_h.T @ A_h^T
where static_a is pre-transposed on host to A^T[h, k, q].  The softmax
normalization is applied to the [64, q] output instead of the [k, q]
matrix; no max-subtraction is needed (|scores*scale| < ~3).

Matmuls run in bf16 (fp32 PE matmul is 4x slower); PSUM accumulation is
fp32.  bf16 rounding of operands keeps rel-err ~4e-3, well under the
2e-2 gate.

v2 scheduling changes (over the first working version):
  - warm-up matmuls on const data at t=0 so the PE HAM clock-gate
    reaches 8/8 before the real work starts (first ~20us of v1 ran at
    1.2 GHz)
  - V projection runs kc-outer in two tt-quads so the first x/vw DMA
    chunk immediately yields dense PE work
  - qkT projections for pairs 2..5 are emitted as per-item filler
    bursts inside the attention loop, filling the exp-gated PE bubbles
    at item boundaries
  - softmax reciprocal on DVE (reciprocal_approx_fast) instead of the
    ACT Ln/Exp chain (saves ~33us of ACT time; ACT runs only the big
    exps)
  - drain: proj partial bursts interleaved into the last item's
    out-steps; output stored/DMA'd as bf16
"""

import os
import sys

import numpy as np

B, N, C = 8, 1024, 768
H, D = 12, 64
NCORES = 8
P = 128
QW = 512          # q tile width (PSUM bank = 512 f32)
NQT = N // QW     # 2 q tiles
NKT = N // P      # 8 k tiles
NCIN = C // P     # 6 c_in chunks
NPAIR = H // 2    # 6 head pairs
SCALE = float(D) ** -0.5

_REPO = "/opt/trn_rl_repo"


def _ensure_paths():
    if _REPO not in sys.path:
        sys.path.insert(0, _REPO)


def _dedup_ldweights(nc):
    """Delete an Ldweights whose weights AP + tile geometry match the
    immediately preceding Ldweights on the PE stream (the weights are
    still resident in the array); its waits/updates move to the next
    instruction."""
    import concourse.mybir as mybir

    def sig(inst):
        ap = inst.ins[0]
        return (str(ap), str(getattr(inst, "tile_position", None)),
                str(getattr(inst, "tile_size", None)))

    for fn in nc.m.functions:
        for blk in fn.blocks:
            out = []
            last_sig = None
            pend_w, pend_u = [], []
            changed = False
            for inst in blk.instructions:
                op = str(inst.opcode)
                if op == "Ldweights":
                    s_ = sig(inst)
                    if s_ == last_sig:
                        si = inst.sync_info
                        if si:
                            pend_w.extend(si.on_wait or [])
                            pend_u.extend(si.on_update or [])
                        changed = True
                        continue
                    last_sig = s_
                elif op == "Matmult":
                    pass          # matmuls don't disturb loaded weights
                elif op in ("NoOp", "EventSemaphore"):
                    pass
                else:
                    last_sig = None
                if pend_w or pend_u:
                    si = inst.sync_info
                    ow = list(si.on_wait or []) if si else []
                    ou = list(si.on_update or []) if si else []
                    inst.sync_info = mybir.SyncInfo(
                        on_wait=pend_w + ow, on_update=pend_u + ou)
                    pend_w, pend_u = [], []
                out.append(inst)
            assert not pend_w and not pend_u
            if changed:
                blk.instructions = out


def _split_excess_waits(nc):
    """The TRN2 walrus codegen allows only 1 sem-wait command per
    instruction.  Tile's sem-assigner can emit more (one per logical
    proc a tile depends on).
    Move the excess onto freshly inserted same-engine NoOps placed just
    before the instruction -- engines execute in order, so waiting on a
    preceding NoOp is equivalent."""
    import concourse.mybir as mybir
    from bass_rust import InstNoOp

    nid = [0]
    for fn in nc.m.functions:
        for blk in fn.blocks:
            out = []
            changed = False
            for inst in blk.instructions:
                si = inst.sync_info
                waits = list(si.on_wait) if si and si.on_wait else []
                limit = 1
                if len(waits) > limit:
                    extra, keep = waits[:-limit], waits[-limit:]
                    inst.sync_info = si.__replace__(on_wait=keep)
                    for w in extra:
                        nop = InstNoOp(
                            name=f"{inst.name}-wsplit{nid[0]}", ins=[], outs=[])
                        nid[0] += 1
                        nop.engine = inst.engine
                        nop.sync_info = mybir.SyncInfo(
                            on_wait=[w], on_update=[])
                        out.append(nop)
                    changed = True
                out.append(inst)
            if changed:
                blk.instructions = out


def _patch_act_tables():
    """Force Bacc's activation-table chooser to the single set that
    contains every function this kernel uses (exp, identity, copy),
    so only one ACT_TABLE_LOAD (~2.7us each) is emitted."""
    import concourse.hw_specs as hw_specs
    import concourse.mybir as mybir
    if getattr(hw_specs.get_activation_tables, "_attn_patched", False):
        return
    orig = hw_specs.get_activation_tables
    keep = {mybir.ActivationFunctionType.Exp, mybir.ActivationFunctionType.Ln,
            mybir.ActivationFunctionType.Identity,
            mybir.ActivationFunctionType.Copy}

    import functools

    @functools.cache
    def patched(module_arch):
        tables = dict(orig(module_arch))
        out = {}
        for name, fns in tables.items():
            if name == "natural_log_exp_and_others":
                out[name] = fns
            else:
                out[name] = fns - keep
        return out

    patched._attn_patched = True
    hw_specs.get_activation_tables = patched
    import concourse.bacc as bacc_mod
    bacc_mod.get_activation_tables = patched


def build_nc():
    """Build the per-core Bass/Tile program."""
    _ensure_paths()
    _patch_act_tables()
    import concourse.bass as bass
    import concourse.mybir as mybir
    import concourse.tile as tile
    from concourse import bacc
    from contextlib import ExitStack

    f32 = mybir.dt.float32
    bf16 = mybir.dt.bfloat16
    f8e4 = mybir.dt.float8e4

    epi_mode = os.environ.get("ATTN_EPI", "pe")   # 'pe' | 'gps'
    nwarm = int(os.environ.get("ATTN_WARM", "16"))
    tailwarm = int(os.environ.get("ATTN_TAILWARM", "10"))
    itemwarm = int(os.environ.get("ATTN_ITEMWARM", "6"))
    at_bufs = int(os.environ.get("ATTN_AT_BUFS", "12"))
    ev8 = os.environ.get("ATTN_EV8", "1") == "1"   # DoubleRow fp8 E@v

    nc = bacc.Bacc("TRN2", target_bir_lowering=False, debug=False,
                   num_devices=NCORES)

    xT_ext = nc.declare_dram_parameter("xT", [C, N], bf16, isOutput=False)
    qkwT_ext = nc.declare_dram_parameter("qkwT", [C, 2 * C], bf16, isOutput=False)
    qkb_ext = nc.declare_dram_parameter("qkb", [P, 2 * C // P], f32, isOutput=False)
    vwT_ext = nc.declare_dram_parameter("vwT", [C, C], bf16, isOutput=False)
    vb_ext = nc.declare_dram_parameter("vb", [1, C], bf16, isOutput=False)
    at_ext = nc.declare_dram_parameter(
        "at", [NPAIR, NQT, NKT, P, 2 * QW], bf16, isOutput=False)
    pwT_ext = nc.declare_dram_parameter("pwT", [C, C], bf16, isOutput=False)
    pb_ext = nc.declare_dram_parameter("pb", [P, C // P], f32, isOutput=False)
    out_ext = nc.declare_dram_parameter("out", [C, N], bf16, isOutput=True)

    with tile.TileContext(nc, num_cores=NCORES) as tc, ExitStack() as ctx:
        consts = ctx.enter_context(tc.tile_pool(name="consts", bufs=1))
        persist = ctx.enter_context(tc.tile_pool(name="persist", bufs=1))
        attn_pool = ctx.enter_context(tc.tile_pool(name="attnout", bufs=1))
        epool = ctx.enter_context(tc.tile_pool(name="epool", bufs=2))
        atbf = ctx.enter_context(tc.tile_pool(name="atbf", bufs=at_bufs))
        small = ctx.enter_context(tc.tile_pool(name="small", bufs=2))

        qkb_sb = consts.tile([P, 2 * C // P], f32)
        pb_sb = consts.tile([P, NCIN], f32)
        vb_sb = consts.tile([1, C], bf16)
        # memsets for the warmup constants go on DVE: the gpsimd queue is
        # busy with the Tile prologue (sem clears) for the first ~3us and
        # would delay the HAM warmup matmuls
        ones_sb = consts.tile([1, P], bf16)
        nc.vector.memset(ones_sb[:], 1.0)
        ones64_sb = consts.tile([1, 64], bf16)
        nc.vector.memset(ones64_sb[:], 1.0)
        # warm matmuls must engage the full 128-row array: the HAM
        # activity monitor does not register K=1 matmuls as PE-busy
        ww_sb = consts.tile([P, 64], bf16)
        nc.vector.memset(ww_sb[:], 0.0)
        wr_sb = consts.tile([P, 512], bf16)
        nc.vector.memset(wr_sb[:], 0.0)

        # persistent activations (bf16 matmul operands)
        qkT_prs = [persist.tile([P, 2, N], bf16, tag=f"qkt{p}",
                                name=f"qkt{p}")
                   for p in range(NPAIR)]
        vp_sb = persist.tile([P, H, NKT, 65], bf16)   # [V_h | 1] stationary
        nc.any.memset(vp_sb[:, :, :, 64:65], 1.0)
        if ev8:
            # fp8 copy of [V_h | 1] with k-tile pairs interleaved along
            # the free axis for DoubleRow E@v (ko stride 80: %16 rule)
            vp8_sb = persist.tile([P, H, NKT // 2, 2, 80], f8e4)
            nc.any.memset(vp8_sb[:, :, :, :, 64:65], 1.0)
        pw_sb = persist.tile([P, NCIN, C], bf16)      # proj weights
        attn_sb = attn_pool.tile([P, NCIN, N], bf16)  # attention out^T

        if epi_mode == "gps":
            from concourse import library_config
            nc.gpsimd.load_library(library_config.attn)

        with tc.tile_pool(name="ph1", bufs=1) as ph1:
            xT_sb = ph1.tile([P, NCIN, N], bf16)
            qkw_sb = ph1.tile([P, NCIN, 2 * C], bf16)
            vw_sb = ph1.tile([P, NCIN, C], bf16)
            # direct bf16 DMA loads (host pre-casts).  Emission order on
            # the sync queue == descriptor order per DMA queue, so x/vw
            # stream first, then qkw; pw + at tiles are emitted later.
            xT_r = xT_ext.rearrange("(c p) t -> p c t", p=P)
            qkw_r = qkwT_ext.rearrange("(c p) n -> p c n", p=P)
            vw_r = vwT_ext.rearrange("(c p) n -> p c n", p=P)
            pw_r = pwT_ext.rearrange("(c p) n -> p c n", p=P)
            for kc in range(NCIN):
                nc.sync.dma_start(xT_sb[:, kc, :], xT_r[:, kc, :])
                nc.sync.dma_start(vw_sb[:, kc, :], vw_r[:, kc, :])
                if kc == 0:
                    nc.sync.dma_start(vb_sb[:], vb_ext[:])
                    nc.sync.dma_start(qkb_sb[:], qkb_ext[:])
                    nc.sync.dma_start(pb_sb[:], pb_ext[:])
            for kc in range(NCIN):
                nc.sync.dma_start(qkw_sb[:, kc, :], qkw_r[:, kc, :])

            # ---- HAM warm-up + V projection.  Const matmuls keep the PE
            # busy while the first x/vw chunks stream in, so the clock
            # gate opens to 8/8 (~3.4us sustained) before real work; a
            # few more const matmuls pad the DMA-paced first group so
            # the busy window stays unbroken. ----
            with tc.tile_pool(name="pp_w", bufs=1, space="PSUM") as pp_w, \
                 tc.tile_pool(name="pp_v", bufs=3, space="PSUM") as pp_v:
                wps = pp_w.tile([64, 512], f32)

                def warm_mm(n):
                    nc.tensor.matmul(
                        wps[:, 0:n], ww_sb[:, 0:64], wr_sb[:, 0:n],
                        start=True, stop=True, skip_group_check=True)

                for _ in range(nwarm):
                    warm_mm(256)

                for gi, tts in enumerate(((0, 1, 2), (3, 4, 5), (6, 7))):
                    pss = {tt: pp_v.tile([P, C], f32, tag="v",
                                         name=f"vps{tt}") for tt in tts}
                    if gi:
                        # pad the group seam (psum rotation wait on the
                        # previous group's DVE copies)
                        for _ in range(3):
                            warm_mm(512)
                    for kc in range(NCIN):
                        for tt in tts:
                            for (n0, nw) in ((0, QW), (QW, C - QW)):
                                nc.tensor.matmul(
                                    pss[tt][:, n0:n0 + nw],
                                    xT_sb[:, kc, tt * P:(tt + 1) * P],
                                    vw_sb[:, kc, n0:n0 + nw],
                                    start=(kc == 0), stop=False,
                                    skip_group_check=True)
                        if gi == 0 and kc < 5:
                            # absorb per-chunk DMA lateness so the HAM
                            # busy window is not broken
                            warm_mm(256)
                            warm_mm(256)
                    for tt in tts:
                        for (n0, nw) in ((0, QW), (QW, C - QW)):
                            nc.tensor.matmul(
                                pss[tt][:, n0:n0 + nw],
                                ones_sb[0:1, 0:P],
                                vb_sb[0:1, n0:n0 + nw],
                                start=False, stop=True,
                                skip_group_check=True)
                        nc.vector.tensor_copy(
                            vp_sb[:, :, tt, 0:64],
                            pss[tt].rearrange("p (h d) -> p h d", d=64))
                        if ev8:
                            # on ACT (idle in phase 1): the psum-pool
                            # rotation waits on these copies, and two
                            # serial DVE casts per tile stall the PE at
                            # every V-group seam
                            nc.scalar.copy(
                                vp8_sb[:, :, tt // 2, tt % 2, 0:64],
                                pss[tt].rearrange("p (h d) -> p h d", d=64))

            # ---- attention (+ interleaved qkT / proj work) ----
            with tc.tile_pool(name="pp_st", bufs=2, space="PSUM") as pp_st, \
                 tc.tile_pool(name="pp_ev", bufs=2, space="PSUM") as pp_ev, \
                     tc.tile_pool(name="pp_av", bufs=2, space="PSUM") as pp_av:

                def qkt_ct(ct, pr_dst, qki):
                    """One qkT output tile: 12 matmuls + DVE bias-add."""
                    ps = pp_st.tile([P, N], f32, tag="st", name=f"qk{ct}")
                    for kc in range(NCIN):
                        for qh in range(NQT):
                            nc.tensor.matmul(
                                ps[:, qh * QW:(qh + 1) * QW],
                                qkw_sb[:, kc, ct * P:(ct + 1) * P],
                                xT_sb[:, kc, qh * QW:(qh + 1) * QW],
                                start=(kc == 0), stop=(kc == NCIN - 1),
                                skip_group_check=True)
                    nc.vector.tensor_scalar_add(
                        qkT_prs[pr_dst][:, qki, :], ps[:, :],
                        qkb_sb[:, ct:ct + 1])

                def qkt_group(pr):
                    qkt_ct(pr, pr, 0)
                    qkt_ct(NPAIR + pr, pr, 1)

                def warm_into(ps, n=QW):
                    # full-K const matmul into a psum region whose next
                    # real matmul is start=True (overwrites the garbage):
                    # keeps the PE busy so the HAM clock stays at 8/8
                    nc.tensor.matmul(
                        ps[0:64, 0:n], ww_sb[:, 0:64],
                        wr_sb[:, 0:n], start=True, stop=True,
                        skip_group_check=True)

                # dummy av-tag tile: a scratch psum bank for padding the
                # qkT-upfront stretch (paced by the qkw DMA stream)
                avw = pp_av.tile([P, QW], f32, tag="av", name="phasewarm")
                for ct_i, (ct, prd, qki) in enumerate(
                        ((0, 0, 0), (NPAIR, 0, 1), (1, 1, 0),
                         (NPAIR + 1, 1, 1))):
                    qkt_ct(ct, prd, qki)
                    for _ in range(3):
                        warm_into(avw)

                # proj weights stream after x/qkw/vw, before the at tiles
                for kc in range(NCIN):
                    nc.sync.dma_start(pw_sb[:, kc, :], pw_r[:, kc, :])

                def emit_st_step(pr, qt, e_sb, kt):
                    q0 = qt * QW
                    st = pp_st.tile([P, 2 * QW], f32, tag="st",
                                    name=f"st{pr}_{qt}_{kt}")
                    k0 = kt * P
                    nc.tensor.matmul(
                        st[:, 0:QW],
                        qkT_prs[pr][0:64, 1, k0:k0 + P],
                        qkT_prs[pr][0:64, 0, q0:q0 + QW],
                        start=True, stop=True)
                    nc.tensor.matmul(
                        st[:, QW:2 * QW],
                        qkT_prs[pr][64:128, 1, k0:k0 + P],
                        qkT_prs[pr][64:128, 0, q0:q0 + QW],
                        start=True, stop=True)
                    nc.scalar.activation(
                        e_sb[:, kt, :], st[:, :],
                        mybir.ActivationFunctionType.Exp, scale=SCALE)

                def emit_out_step(item, kt):
                    pr, qt, e_sb, psE1, psE2, psA = item
                    h1, h2 = 2 * pr, 2 * pr + 1
                    at = atbf.tile([P, 2 * QW], bf16, tag="atb",
                                   name=f"atb{pr}_{qt}_{kt}")
                    nc.sync.dma_start(at[:], at_ext[pr, qt, kt])
                    st_flags = dict(start=(kt == 0), stop=(kt == NKT - 1),
                                    skip_group_check=True)
                    nc.tensor.matmul(
                        psA[0:64, :], vp_sb[:, h1, kt, 0:64],
                        at[:, 0:QW], **st_flags)
                    nc.tensor.matmul(
                        psA[64:128, :], vp_sb[:, h2, kt, 0:64],
                        at[:, QW:2 * QW], **st_flags)
                    if ev8:
                        # DoubleRow fp8: one matmul covers a k-tile pair
                        if kt % 2 == 1:
                            g = kt // 2
                            dr_flags = dict(
                                start=(g == 0), stop=(g == NKT // 2 - 1),
                                perf_mode=mybir.MatmulPerfMode.DoubleRow,
                                skip_group_check=True)
                            nc.tensor.matmul(
                                psE1[0:65, :],
                                vp8_sb[:, h1, g, :, 0:65],
                                e_sb[:, 2 * g:2 * g + 2, 0:QW], **dr_flags)
                            nc.tensor.matmul(
                                psE2[0:65, :],
                                vp8_sb[:, h2, g, :, 0:65],
                                e_sb[:, 2 * g:2 * g + 2, QW:2 * QW],
                                **dr_flags)
                    else:
                        nc.tensor.matmul(
                            psE1[0:65, :], vp_sb[:, h1, kt, :],
                            e_sb[:, kt, 0:QW], **st_flags)
                        nc.tensor.matmul(
                            psE2[0:65, :], vp_sb[:, h2, kt, :],
                            e_sb[:, kt, QW:2 * QW], **st_flags)

                recip_mode = os.environ.get("ATTN_RECIP", "dve")

                def emit_epilogue_recip(item, force_act=False):
                    # 1/rowsum; runs while the next block's score
                    # matmuls keep the PE busy
                    pr, qt, e_sb, psE1, psE2, psA = item
                    rs = []
                    for hi, psE in ((0, psE1), (1, psE2)):
                        if recip_mode == "act" or force_act:
                            lns = small.tile([1, QW], f32, tag="lns",
                                             name=f"ln{pr}_{qt}_{hi}")
                            nc.scalar.activation(
                                lns[:], psE[64:65, :],
                                mybir.ActivationFunctionType.Ln)
                            r16 = small.tile([1, QW], bf16, tag="r16",
                                             name=f"r16_{pr}_{qt}_{hi}")
                            nc.scalar.activation(
                                r16[:], lns[:],
                                mybir.ActivationFunctionType.Exp,
                                scale=-1.0)
                            rs.append(r16)
                            continue
                        # copy the rowsum to a partition-0 SBUF tile first:
                        # the custom-DVE recip mis-reads a partition-64
                        # PSUM operand (standard ops handle it fine)
                        rsum = small.tile([1, QW], f32, tag="rsum",
                                          name=f"rs_{pr}_{qt}_{hi}")
                        nc.vector.tensor_copy(rsum[:], psE[64:65, :])
                        r32 = small.tile([1, QW], f32, tag="r32",
                                         name=f"r32_{pr}_{qt}_{hi}")
                        nc.vector.reciprocal_approx_fast(r32[:], rsum[:])
                        if epi_mode == "gps":
                            rs.append(r32)
                        else:
                            r16 = small.tile([1, QW], bf16, tag="r16",
                                             name=f"r16_{pr}_{qt}_{hi}")
                            nc.vector.tensor_copy(r16[:], r32[:])
                            rs.append(r16)
                    return rs

                def emit_epilogue_apply(item, rs):
                    pr, qt, e_sb, psE1, psE2, psA = item
                    q0 = qt * QW
                    for hi, psE in ((0, psE1), (1, psE2)):
                        pa, pz = hi * 64, hi * 64 + 64
                        dst = attn_sb[pa:pz, pr, q0:q0 + QW]
                        if epi_mode == "gps":
                            rb = small.tile([64, QW], f32, tag="rb",
                                            name=f"rb{pr}_{qt}_{hi}")
                            nc.gpsimd.partition_broadcast(
                                rb[:], rs[hi][:], channels=64)
                            nc.vector.tensor_mul(dst, psE[0:64, :], rb[:])
                        else:
                            nc.tensor.matmul(psE[64:128, :],
                                             ones64_sb[0:1, :],
                                             rs[hi][:, :], start=True,
                                             stop=True,
                                             skip_group_check=True)
                            rb = small.tile([64, QW], f32, tag="rb",
                                            name=f"rb{pr}_{qt}_{hi}")
                            nc.vector.tensor_copy(rb[:], psE[64:128, :])
                            nc.vector.tensor_mul(dst, psE[0:64, :], rb[:])
                        nc.vector.tensor_add(dst, dst, psA[pa:pz, :])

                # qkT filler bursts inside the item loops: pairs 2..5,
                # each ct one-to-two items before its first use.  Item
                # (0,0) has no out-step work (pipeline fill), so it gets
                # two bursts.
                filler = {
                    (0, 0): [(2, 2, 0), (NPAIR + 2, 2, 1)],
                    (0, 1): [(3, 3, 0)],
                    (1, 0): [(NPAIR + 3, 3, 1)],
                    (1, 1): [(4, 4, 0)],
                    (2, 0): [(NPAIR + 4, 4, 1)],
                    (2, 1): [(5, 5, 0)],
                    (3, 0): [(NPAIR + 5, 5, 1)],
                }

                # software-pipelined emission: item i's ST/exp stream is
                # interleaved kt-by-kt with item i-1's E@v/A@v matmuls, so
                # the PE has dense work while ACT drains the score tiles
                items = [(pr, qt) for pr in range(NPAIR)
                         for qt in range(NQT)]
                prev = None        # item whose OUT runs in the current block
                pend = None        # pe-mode: (item, rs) awaiting PE/DVE apply
                e_dt = f8e4 if ev8 else bf16
                for pr, qt in items:
                    e_sb = epool.tile([P, NKT, 2 * QW], e_dt, tag="e",
                                      name=f"e{pr}_{qt}")
                    # two score steps up front cover the pending
                    # epilogue's DVE reciprocal latency
                    emit_st_step(pr, qt, e_sb, 0)
                    emit_st_step(pr, qt, e_sb, 1)
                    if pend is not None:
                        emit_epilogue_apply(*pend)
                        pend = None
                    psE1 = pp_ev.tile([P, QW], f32, tag="ev",
                                      name=f"ev1_{pr}_{qt}")
                    psE2 = pp_ev.tile([P, QW], f32, tag="ev",
                                      name=f"ev2_{pr}_{qt}")
                    psA = pp_av.tile([P, QW], f32, tag="av",
                                     name=f"av{pr}_{qt}")
                    cur = (pr, qt, e_sb, psE1, psE2, psA)
                    fill = list(filler.get((pr, qt), ()))
                    for kt in range(NKT):
                        if kt + 2 < NKT:
                            emit_st_step(pr, qt, e_sb, kt + 2)
                        if prev is not None:
                            emit_out_step(prev, kt)
                        if fill and kt in (2, 4):
                            qkt_ct(*fill.pop(0))
                    # item-boundary padding: the next item's first score
                    # steps wait on this item's last exps (ACT is the
                    # pacer once E@v runs DoubleRow); garbage written here
                    # is overwritten by this item's start=True A@v in the
                    # next block
                    i_next = items.index((pr, qt)) + 1
                    if i_next < len(items):
                        nwm = 2 if filler.get(items[i_next]) else itemwarm
                        for _ in range(nwm):
                            warm_into(psA)
                    if prev is not None:
                        rs = emit_epilogue_recip(prev)
                        if epi_mode == "gps":
                            emit_epilogue_apply(prev, rs)
                        else:
                            pend = (prev, rs)
                    prev = cur

                # ---- drain: last item's outs with proj partial bursts
                # interleaved, then the final epilogue and the output
                # projection ----
                def proj_partial(ps, ct, kcs, start, stop):
                    for kc in kcs:
                        for qh in range(NQT):
                            nc.tensor.matmul(
                                ps[:, qh * QW:(qh + 1) * QW],
                                pw_sb[:, kc, ct * P:(ct + 1) * P],
                                attn_sb[:, kc, qh * QW:(qh + 1) * QW],
                                start=(start and kc == kcs[0]),
                                stop=(stop and kc == kcs[-1]),
                                skip_group_check=True)

                with tc.tile_pool(name="ph3o", bufs=2) as ph3o:
                    out_r = out_ext.rearrange("(c p) t -> p c t", p=P)
                    pjs = {}

                    def proj_finish(ct, kcs, start):
                        ps = pjs[ct]
                        proj_partial(ps, ct, kcs, start, True)
                        o_sb = ph3o.tile([P, N], bf16, tag="o",
                                         name=f"o{ct}")
                        # alternate the bias-add between ACT and DVE so
                        # the six tail bias-adds run two-wide
                        if ct % 2 == 0:
                            nc.scalar.activation(
                                o_sb[:], ps[:],
                                mybir.ActivationFunctionType.Identity,
                                bias=pb_sb[:, ct:ct + 1])
                        else:
                            nc.vector.tensor_scalar_add(
                                o_sb[:], ps[:], pb_sb[:, ct:ct + 1])
                        nc.sync.dma_start(out_r[:, ct, :], o_sb[:])

                    for kt in range(NKT):
                        emit_out_step(prev, kt)
                        if kt == 0 and pend is not None:
                            emit_epilogue_apply(*pend)
                            pend = None
                        # the proj bursts recycle the st-score psum bufs;
                        # placed where the bufs actually free (exp kt6/kt7)
                        # so they don't stall the out-step stream
                        if kt == 5:
                            pjs[0] = pp_st.tile([P, N], f32, tag="st",
                                                name="proj0")
                            proj_partial(pjs[0], 0, list(range(NCIN - 1)),
                                         True, False)
                        if kt == 6:
                            pjs[1] = pp_st.tile([P, N], f32, tag="st",
                                                name="proj1")
                            proj_partial(pjs[1], 1, list(range(NCIN - 1)),
                                         True, False)
                    # const matmuls keep the PE busy (and the HAM clock
                    # warm) while the final epilogue chain runs on
                    # ACT/DVE; the proj finishes wait on it anyway
                    if tailwarm:
                        fav = pp_av.tile([P, QW], f32, tag="av",
                                         name="tailwarm")
                        for _ in range(tailwarm):
                            nc.tensor.matmul(
                                fav[0:64, :], ww_sb[:, 0:64],
                                wr_sb[:, :], start=True, stop=True,
                                skip_group_check=True)
                    # final epilogue: ACT recip chain — ACT is idle at
                    # the tail and the DVE chain (serial ~8us) was the
                    # dominant drain stall
                    rs = emit_epilogue_recip(prev, force_act=True)
                    emit_epilogue_apply(prev, rs)
                    proj_finish(0, [NCIN - 1], False)
                    proj_finish(1, [NCIN - 1], False)
                    for ct in range(2, NCIN):
                        pjs[ct] = pp_st.tile([P, N], f32, tag="st",
                                             name=f"proj{ct}")
                        proj_finish(ct, list(range(NCIN)), True)

    if os.environ.get("ATTN_DEDUP_LDW", "1") == "1":
        _dedup_ldweights(nc)
    if os.environ.get("ATTN_SPLIT_WAITS", "1") == "1":
        _split_excess_waits(nc)
    if not nc.is_finalized():
        nc.finalize()   # Bacc: move_matmul_waits + generate_event_semaphores
    return nc


def make_in_maps(x, qkv_w, qkv_b, static_a, proj_w, proj_b):
    """Host-side sharding / layout prep. One batch element per core."""
    x = np.asarray(x, dtype=np.float32)
    qkv_w = np.asarray(qkv_w, dtype=np.float32)
    qkv_b = np.asarray(qkv_b, dtype=np.float32)
    static_a = np.asarray(static_a, dtype=np.float32)
    proj_w = np.asarray(proj_w, dtype=np.float32)
    proj_b = np.asarray(proj_b, dtype=np.float32)

    import ml_dtypes
    bf16 = ml_dtypes.bfloat16

    qkwT = np.ascontiguousarray(qkv_w[0:2 * C].T).astype(bf16)  # [768, 1536]
    qkb = np.ascontiguousarray(
        qkv_b[0:2 * C].reshape(2 * C // P, P).T).astype(np.float32)
    vwT = np.ascontiguousarray(qkv_w[2 * C:3 * C].T).astype(bf16)
    vb = np.ascontiguousarray(
        qkv_b[2 * C:3 * C].reshape(1, C)).astype(bf16)
    # A^T strips, contiguous per (pair, qtile, ktile): [6, 2, 8, 128, 1024]
    # at[pr, qt, kt, :, 0:512] = A^T[2pr][kt tile, qt tile], [..., 512:] = head 2pr+1
    atT = static_a[0].transpose(0, 2, 1)                      # [H, k, q]
    at = np.ascontiguousarray(
        atT.reshape(NPAIR, 2, NKT, P, NQT, QW).transpose(0, 4, 2, 3, 1, 5)
        .reshape(NPAIR, NQT, NKT, P, 2 * QW)).astype(bf16)
    pwT = np.ascontiguousarray(proj_w.T).astype(bf16)
    pb = np.ascontiguousarray(
        proj_b.reshape(C // P, P).T).astype(np.float32)

    shared = {"qkwT": qkwT, "qkb": qkb, "vwT": vwT, "vb": vb,
              "at": at, "pwT": pwT, "pb": pb}
    in_maps = []
    for b in range(B):
        m = dict(shared)
        m["xT"] = np.ascontiguousarray(x[b].T).astype(bf16)
        in_maps.append(m)
    return in_maps


_NC_CACHE = {}


def _get_nc():
    if "nc" not in _NC_CACHE:
        _NC_CACHE["nc"] = build_nc()
    return _NC_CACHE["nc"]


def kernel(x, qkv_w, qkv_b, static_a, proj_w, proj_b):
    _ensure_paths()
    from concourse.bass_utils import run_bass_kernel_spmd

    nc = _get_nc()
    in_maps = make_in_maps(x, qkv_w, qkv_b, static_a, proj_w, proj_b)
    res = run_bass_kernel_spmd(nc, in_maps, core_ids=list(range(NCORES)))
    out = np.empty((B, N, C), dtype=np.float32)
    for b in range(B):
        out[b] = np.asarray(res.results[b]["out"], dtype=np.float32).T
    return out


# revision 37
# speedup vs baseline: 1.1089x; 1.0367x over previous
"""Trainium2 Bass kernel for nn_Attention_72438918414857.

Reference computation (B=8, N=1024, C=768, H=12, D=64):
    qkv = (x @ qkv_w.T + qkv_b) -> q, k, v per head
    attn = softmax(q @ k.T / sqrt(D)) + static_a   (bias added AFTER softmax)
    out = (attn @ v) merged-heads @ proj_w.T + proj_b

Sharding: data-parallel over batch -- one batch element per NeuronCore,
weights + static_a replicated. No collectives needed.

Math used on-chip (per batch, per head), everything transposed so each
matmul gets its contraction dim on partitions with no on-chip transposes:
    qkT = [Wq;Wk]^T-proj of x  ->  [cout, t] layout
    E^T = exp(K_h^T.T @ Q_h^T * D^-0.5)           [k, q] strips
    out_h^T = ([V_h|1].T @ E^T) -> rows 0..63 = E@v, row 64 = rowsum(E)
    attn_h^T = (E@v) * (1/rowsum) + V_h.T @ A_h^T
where static_a is pre-transposed on host to A^T[h, k, q].  The softmax
normalization is applied to the [64, q] output instead of the [k, q]
matrix; no max-subtraction is needed (|scores*scale| < ~3).

Matmuls run in bf16 (fp32 PE matmul is 4x slower); PSUM accumulation is
fp32.  bf16 rounding of operands keeps rel-err ~4e-3, well under the
2e-2 gate.

v2 scheduling changes (over the first working version):
  - warm-up matmuls on const data at t=0 so the PE HAM clock-gate
    reaches 8/8 before the real work starts (first ~20us of v1 ran at
    1.2 GHz)
  - V projection runs kc-outer in two tt-quads so the first x/vw DMA
    chunk immediately yields dense PE work
  - qkT projections for pairs 2..5 are emitted as per-item filler
    bursts inside the attention loop, filling the exp-gated PE bubbles
    at item boundaries
  - softmax reciprocal on DVE (reciprocal_approx_fast) instead of the
    ACT Ln/Exp chain (saves ~33us of ACT time; ACT runs only the big
    exps)
  - drain: proj partial bursts interleaved into the last item's
    out-steps; output stored/DMA'd as bf16
"""

import os
import sys

import numpy as np

B, N, C = 8, 1024, 768
H, D = 12, 64
NCORES = 8
P = 128
QW = 512          # q tile width (PSUM bank = 512 f32)
NQT = N // QW     # 2 q tiles
NKT = N // P      # 8 k tiles
NCIN = C // P     # 6 c_in chunks
NPAIR = H // 2    # 6 head pairs
SCALE = float(D) ** -0.5

_REPO = "/opt/trn_rl_repo"


def _ensure_paths():
    if _REPO not in sys.path:
        sys.path.insert(0, _REPO)


def _dedup_ldweights(nc):
    """Delete an Ldweights whose weights AP + tile geometry match the
    immediately preceding Ldweights on the PE stream (the weights are
    still resident in the array); its waits/updates move to the next
    instruction."""
    import concourse.mybir as mybir

    def sig(inst):
        ap = inst.ins[0]
        return (str(ap), str(getattr(inst, "tile_position", None)),
                str(getattr(inst, "tile_size", None)))

    for fn in nc.m.functions:
        for blk in fn.blocks:
            out = []
            last_sig = None
            pend_w, pend_u = [], []
            changed = False
            for inst in blk.instructions:
                op = str(inst.opcode)
                if op == "Ldweights":
                    s_ = sig(inst)
                    if s_ == last_sig:
                        si = inst.sync_info
                        if si:
                            pend_w.extend(si.on_wait or [])
                            pend_u.extend(si.on_update or [])
                        changed = True
                        continue
                    last_sig = s_
                elif op == "Matmult":
                    pass          # matmuls don't disturb loaded weights
                elif op in ("NoOp", "EventSemaphore"):
                    pass
                else:
                    last_sig = None
                if pend_w or pend_u:
                    si = inst.sync_info
                    ow = list(si.on_wait or []) if si else []
                    ou = list(si.on_update or []) if si else []
                    inst.sync_info = mybir.SyncInfo(
                        on_wait=pend_w + ow, on_update=pend_u + ou)
                    pend_w, pend_u = [], []
                out.append(inst)
            assert not pend_w and not pend_u
            if changed:
                blk.instructions = out


def _split_excess_waits(nc):
    """The TRN2 walrus codegen allows only 1 sem-wait command per
    instruction.  Tile's sem-assigner can emit more (one per logical
    proc a tile depends on).
    Move the excess onto freshly inserted same-engine NoOps placed just
    before the instruction -- engines execute in order, so waiting on a
    preceding NoOp is equivalent."""
    import concourse.mybir as mybir
    from bass_rust import InstNoOp

    nid = [0]
    for fn in nc.m.functions:
        for blk in fn.blocks:
            out = []
            changed = False
            for inst in blk.instructions:
                si = inst.sync_info
                waits = list(si.on_wait) if si and si.on_wait else []
                limit = 1
                if len(waits) > limit:
                    extra, keep = waits[:-limit], waits[-limit:]
                    inst.sync_info = si.__replace__(on_wait=keep)
                    for w in extra:
                        nop = InstNoOp(
                            name=f"{inst.name}-wsplit{nid[0]}", ins=[], outs=[])
                        nid[0] += 1
                        nop.engine = inst.engine
                        nop.sync_info = mybir.SyncInfo(
                            on_wait=[w], on_update=[])
                        out.append(nop)
                    changed = True
                out.append(inst)
            if changed:
                blk.instructions = out


def _patch_act_tables():
    """Force Bacc's activation-table chooser to the single set that
    contains every function this kernel uses (exp, identity, copy),
    so only one ACT_TABLE_LOAD (~2.7us each) is emitted."""
    import concourse.hw_specs as hw_specs
    import concourse.mybir as mybir
    if getattr(hw_specs.get_activation_tables, "_attn_patched", False):
        return
    orig = hw_specs.get_activation_tables
    keep = {mybir.ActivationFunctionType.Exp, mybir.ActivationFunctionType.Ln,
            mybir.ActivationFunctionType.Identity,
            mybir.ActivationFunctionType.Copy}

    import functools

    @functools.cache
    def patched(module_arch):
        tables = dict(orig(module_arch))
        out = {}
        for name, fns in tables.items():
            if name == "natural_log_exp_and_others":
                out[name] = fns
            else:
                out[name] = fns - keep
        return out

    patched._attn_patched = True
    hw_specs.get_activation_tables = patched
    import concourse.bacc as bacc_mod
    bacc_mod.get_activation_tables = patched


def build_nc():
    """Build the per-core Bass/Tile program."""
    _ensure_paths()
    _patch_act_tables()
    import concourse.bass as bass
    import concourse.mybir as mybir
    import concourse.tile as tile
    from concourse import bacc
    from contextlib import ExitStack

    f32 = mybir.dt.float32
    bf16 = mybir.dt.bfloat16
    f8e4 = mybir.dt.float8e4

    epi_mode = os.environ.get("ATTN_EPI", "pe")   # 'pe' | 'gps'
    nwarm = int(os.environ.get("ATTN_WARM", "16"))
    tailwarm = int(os.environ.get("ATTN_TAILWARM", "10"))
    itemwarm = int(os.environ.get("ATTN_ITEMWARM", "6"))
    at_bufs = int(os.environ.get("ATTN_AT_BUFS", "12"))
    ev8 = os.environ.get("ATTN_EV8", "1") == "1"   # DoubleRow fp8 E@v
    qk8 = os.environ.get("ATTN_QK8", "1") == "1"   # DoubleRow fp8 qkT proj

    nc = bacc.Bacc("TRN2", target_bir_lowering=False, debug=False,
                   num_devices=NCORES)

    xT_ext = nc.declare_dram_parameter("xT", [C, N], bf16, isOutput=False)
    if qk8:
        # cin-pair interleaved fp8 copies for DoubleRow (weights scaled
        # by 32 on host so w~N(0,0.02) lands in e4m3's normal range;
        # the extra 32*32 falls out of the exp scale)
        x8_ext = nc.declare_dram_parameter(
            "x8", [P, NCIN // 2, 2, N], f8e4, isOutput=False)
        qkw8_ext = nc.declare_dram_parameter(
            "qkw8", [P, NCIN // 2, 2, 2 * C], f8e4, isOutput=False)
    else:
        qkwT_ext = nc.declare_dram_parameter(
            "qkwT", [C, 2 * C], bf16, isOutput=False)
    qkb_ext = nc.declare_dram_parameter("qkb", [P, 2 * C // P], f32, isOutput=False)
    vwT_ext = nc.declare_dram_parameter("vwT", [C, C], bf16, isOutput=False)
    vb_ext = nc.declare_dram_parameter("vb", [1, C], bf16, isOutput=False)
    at_ext = nc.declare_dram_parameter(
        "at", [NPAIR, NQT, NKT, P, 2 * QW], bf16, isOutput=False)
    pwT_ext = nc.declare_dram_parameter("pwT", [C, C], bf16, isOutput=False)
    pb_ext = nc.declare_dram_parameter("pb", [P, C // P], f32, isOutput=False)
    out_ext = nc.declare_dram_parameter("out", [C, N], bf16, isOutput=True)

    with tile.TileContext(nc, num_cores=NCORES) as tc, ExitStack() as ctx:
        consts = ctx.enter_context(tc.tile_pool(name="consts", bufs=1))
        persist = ctx.enter_context(tc.tile_pool(name="persist", bufs=1))
        attn_pool = ctx.enter_context(tc.tile_pool(name="attnout", bufs=1))
        epool = ctx.enter_context(tc.tile_pool(name="epool", bufs=2))
        atbf = ctx.enter_context(tc.tile_pool(name="atbf", bufs=at_bufs))
        small = ctx.enter_context(tc.tile_pool(name="small", bufs=2))

        qkb_sb = consts.tile([P, 2 * C // P], f32)
        pb_sb = consts.tile([P, NCIN], f32)
        vb_sb = consts.tile([1, C], bf16)
        # memsets for the warmup constants go on DVE: the gpsimd queue is
        # busy with the Tile prologue (sem clears) for the first ~3us and
        # would delay the HAM warmup matmuls
        ones_sb = consts.tile([1, P], bf16)
        nc.vector.memset(ones_sb[:], 1.0)
        ones64_sb = consts.tile([1, 64], bf16)
        nc.vector.memset(ones64_sb[:], 1.0)
        # warm matmuls must engage the full 128-row array: the HAM
        # activity monitor does not register K=1 matmuls as PE-busy
        ww_sb = consts.tile([P, 64], bf16)
        nc.vector.memset(ww_sb[:], 0.0)
        wr_sb = consts.tile([P, 512], bf16)
        nc.vector.memset(wr_sb[:], 0.0)

        # persistent activations (bf16 matmul operands)
        qkT_prs = [persist.tile([P, 2, N], bf16, tag=f"qkt{p}",
                                name=f"qkt{p}")
                   for p in range(NPAIR)]
        vp_sb = persist.tile([P, H, NKT, 65], bf16)   # [V_h | 1] stationary
        nc.any.memset(vp_sb[:, :, :, 64:65], 1.0)
        if ev8:
            # fp8 copy of [V_h | 1] with k-tile pairs interleaved along
            # the free axis for DoubleRow E@v (ko stride 80: %16 rule)
            vp8_sb = persist.tile([P, H, NKT // 2, 2, 80], f8e4)
            nc.any.memset(vp8_sb[:, :, :, :, 64:65], 1.0)
        pw_sb = persist.tile([P, NCIN, C], bf16)      # proj weights
        attn_sb = attn_pool.tile([P, NCIN, N], bf16)  # attention out^T

        if epi_mode == "gps":
            from concourse import library_config
            nc.gpsimd.load_library(library_config.attn)

        with tc.tile_pool(name="ph1", bufs=1) as ph1:
            xT_sb = ph1.tile([P, NCIN, N], bf16)
            vw_sb = ph1.tile([P, NCIN, C], bf16)
            # direct DMA loads (host pre-casts).  Emission order on
            # the sync queue == descriptor order per DMA queue, so x/vw
            # stream first, then qkw; pw + at tiles are emitted later.
            xT_r = xT_ext.rearrange("(c p) t -> p c t", p=P)
            vw_r = vwT_ext.rearrange("(c p) n -> p c n", p=P)
            pw_r = pwT_ext.rearrange("(c p) n -> p c n", p=P)
            for kc in range(NCIN):
                nc.sync.dma_start(xT_sb[:, kc, :], xT_r[:, kc, :])
                nc.sync.dma_start(vw_sb[:, kc, :], vw_r[:, kc, :])
                if kc == 0:
                    nc.sync.dma_start(vb_sb[:], vb_ext[:])
                    nc.sync.dma_start(qkb_sb[:], qkb_ext[:])
                    nc.sync.dma_start(pb_sb[:], pb_ext[:])
            if qk8:
                x8_sb = ph1.tile([P, NCIN // 2, 2, N], f8e4)
                qkw8_sb = ph1.tile([P, NCIN // 2, 2, 2 * C], f8e4)
                for g in range(NCIN // 2):
                    nc.sync.dma_start(x8_sb[:, g, :, :], x8_ext[:, g, :, :])
                    nc.sync.dma_start(qkw8_sb[:, g, :, :],
                                      qkw8_ext[:, g, :, :])
            else:
                qkw_sb = ph1.tile([P, NCIN, 2 * C], bf16)
                qkw_r = qkwT_ext.rearrange("(c p) n -> p c n", p=P)
                for kc in range(NCIN):
                    nc.sync.dma_start(qkw_sb[:, kc, :], qkw_r[:, kc, :])

            # ---- HAM warm-up + V projection.  Const matmuls keep the PE
            # busy while the first x/vw chunks stream in, so the clock
            # gate opens to 8/8 (~3.4us sustained) before real work; a
            # few more const matmuls pad the DMA-paced first group so
            # the busy window stays unbroken. ----
            with tc.tile_pool(name="pp_w", bufs=1, space="PSUM") as pp_w, \
                 tc.tile_pool(name="pp_v", bufs=3, space="PSUM") as pp_v:
                wps = pp_w.tile([64, 512], f32)

                def warm_mm(n):
                    nc.tensor.matmul(
                        wps[:, 0:n], ww_sb[:, 0:64], wr_sb[:, 0:n],
                        start=True, stop=True, skip_group_check=True)

                for _ in range(nwarm):
                    warm_mm(256)

                for gi, tts in enumerate(((0, 1, 2), (3, 4, 5), (6, 7))):
                    pss = {tt: pp_v.tile([P, C], f32, tag="v",
                                         name=f"vps{tt}") for tt in tts}
                    if gi:
                        # pad the group seam (psum rotation wait on the
                        # previous group's DVE copies)
                        for _ in range(3):
                            warm_mm(512)
                    for kc in range(NCIN):
                        for tt in tts:
                            for (n0, nw) in ((0, QW), (QW, C - QW)):
                                nc.tensor.matmul(
                                    pss[tt][:, n0:n0 + nw],
                                    xT_sb[:, kc, tt * P:(tt + 1) * P],
                                    vw_sb[:, kc, n0:n0 + nw],
                                    start=(kc == 0), stop=False,
                                    skip_group_check=True)
                        if gi == 0 and kc < 5:
                            # absorb per-chunk DMA lateness so the HAM
                            # busy window is not broken
                            warm_mm(256)
                            warm_mm(256)
                    for tt in tts:
                        for (n0, nw) in ((0, QW), (QW, C - QW)):
                            nc.tensor.matmul(
                                pss[tt][:, n0:n0 + nw],
                                ones_sb[0:1, 0:P],
                                vb_sb[0:1, n0:n0 + nw],
                                start=False, stop=True,
                                skip_group_check=True)
                        nc.vector.tensor_copy(
                            vp_sb[:, :, tt, 0:64],
                            pss[tt].rearrange("p (h d) -> p h d", d=64))
                        if ev8:
                            # on ACT (idle in phase 1): the psum-pool
                            # rotation waits on these copies, and two
                            # serial DVE casts per tile stall the PE at
                            # every V-group seam
                            nc.scalar.copy(
                                vp8_sb[:, :, tt // 2, tt % 2, 0:64],
                                pss[tt].rearrange("p (h d) -> p h d", d=64))

            # ---- attention (+ interleaved qkT / proj work) ----
            with tc.tile_pool(name="pp_st", bufs=2, space="PSUM") as pp_st, \
                 tc.tile_pool(name="pp_ev", bufs=2, space="PSUM") as pp_ev, \
                     tc.tile_pool(name="pp_av", bufs=2, space="PSUM") as pp_av:

                def qkt_ct(ct, pr_dst, qki):
                    """One qkT output tile (6 or 3 matmuls) + bias-add."""
                    ps = pp_st.tile([P, N], f32, tag="st", name=f"qk{ct}")
                    if qk8:
                        for g in range(NCIN // 2):
                            for qh in range(NQT):
                                nc.tensor.matmul(
                                    ps[:, qh * QW:(qh + 1) * QW],
                                    qkw8_sb[:, g, :, ct * P:(ct + 1) * P],
                                    x8_sb[:, g, :, qh * QW:(qh + 1) * QW],
                                    start=(g == 0),
                                    stop=(g == NCIN // 2 - 1),
                                    perf_mode=mybir.MatmulPerfMode.DoubleRow,
                                    skip_group_check=True)
                    else:
                        for kc in range(NCIN):
                            for qh in range(NQT):
                                nc.tensor.matmul(
                                    ps[:, qh * QW:(qh + 1) * QW],
                                    qkw_sb[:, kc, ct * P:(ct + 1) * P],
                                    xT_sb[:, kc, qh * QW:(qh + 1) * QW],
                                    start=(kc == 0), stop=(kc == NCIN - 1),
                                    skip_group_check=True)
                    nc.vector.tensor_scalar_add(
                        qkT_prs[pr_dst][:, qki, :], ps[:, :],
                        qkb_sb[:, ct:ct + 1])

                def qkt_group(pr):
                    qkt_ct(pr, pr, 0)
                    qkt_ct(NPAIR + pr, pr, 1)

                def warm_into(ps, n=QW):
                    # full-K const matmul into a psum region whose next
                    # real matmul is start=True (overwrites the garbage):
                    # keeps the PE busy so the HAM clock stays at 8/8
                    nc.tensor.matmul(
                        ps[0:64, 0:n], ww_sb[:, 0:64],
                        wr_sb[:, 0:n], start=True, stop=True,
                        skip_group_check=True)

                # dummy av-tag tile: a scratch psum bank for padding the
                # qkT-upfront stretch (paced by the qkw DMA stream)
                avw = pp_av.tile([P, QW], f32, tag="av", name="phasewarm")
                for ct_i, (ct, prd, qki) in enumerate(
                        ((0, 0, 0), (NPAIR, 0, 1), (1, 1, 0),
                         (NPAIR + 1, 1, 1))):
                    qkt_ct(ct, prd, qki)
                    for _ in range(3):
                        warm_into(avw)

                # proj weights stream after x/qkw/vw, before the at tiles
                for kc in range(NCIN):
                    nc.sync.dma_start(pw_sb[:, kc, :], pw_r[:, kc, :])

                # with fp8 qkT, qkT_prs holds 32*(q+b) per side -> the
                # score psum carries an extra 1024x, folded into the exp
                exp_scale = SCALE / 1024.0 if qk8 else SCALE

                def emit_st_step(pr, qt, e_sb, kt):
                    q0 = qt * QW
                    st = pp_st.tile([P, 2 * QW], f32, tag="st",
                                    name=f"st{pr}_{qt}_{kt}")
                    k0 = kt * P
                    nc.tensor.matmul(
                        st[:, 0:QW],
                        qkT_prs[pr][0:64, 1, k0:k0 + P],
                        qkT_prs[pr][0:64, 0, q0:q0 + QW],
                        start=True, stop=True)
                    nc.tensor.matmul(
                        st[:, QW:2 * QW],
                        qkT_prs[pr][64:128, 1, k0:k0 + P],
                        qkT_prs[pr][64:128, 0, q0:q0 + QW],
                        start=True, stop=True)
                    nc.scalar.activation(
                        e_sb[:, kt, :], st[:, :],
                        mybir.ActivationFunctionType.Exp, scale=exp_scale)

                def emit_out_step(item, kt):
                    pr, qt, e_sb, psE1, psE2, psA = item
                    h1, h2 = 2 * pr, 2 * pr + 1
                    at = atbf.tile([P, 2 * QW], bf16, tag="atb",
                                   name=f"atb{pr}_{qt}_{kt}")
                    nc.sync.dma_start(at[:], at_ext[pr, qt, kt])
                    st_flags = dict(start=(kt == 0), stop=(kt == NKT - 1),
                                    skip_group_check=True)
                    nc.tensor.matmul(
                        psA[0:64, :], vp_sb[:, h1, kt, 0:64],
                        at[:, 0:QW], **st_flags)
                    nc.tensor.matmul(
                        psA[64:128, :], vp_sb[:, h2, kt, 0:64],
                        at[:, QW:2 * QW], **st_flags)
                    if ev8:
                        # DoubleRow fp8: one matmul covers a k-tile pair
                        if kt % 2 == 1:
                            g = kt // 2
                            dr_flags = dict(
                                start=(g == 0), stop=(g == NKT // 2 - 1),
                                perf_mode=mybir.MatmulPerfMode.DoubleRow,
                                skip_group_check=True)
                            nc.tensor.matmul(
                                psE1[0:65, :],
                                vp8_sb[:, h1, g, :, 0:65],
                                e_sb[:, 2 * g:2 * g + 2, 0:QW], **dr_flags)
                            nc.tensor.matmul(
                                psE2[0:65, :],
                                vp8_sb[:, h2, g, :, 0:65],
                                e_sb[:, 2 * g:2 * g + 2, QW:2 * QW],
                                **dr_flags)
                    else:
                        nc.tensor.matmul(
                            psE1[0:65, :], vp_sb[:, h1, kt, :],
                            e_sb[:, kt, 0:QW], **st_flags)
                        nc.tensor.matmul(
                            psE2[0:65, :], vp_sb[:, h2, kt, :],
                            e_sb[:, kt, QW:2 * QW], **st_flags)

                recip_mode = os.environ.get("ATTN_RECIP", "dve")

                def emit_epilogue_recip(item, force_act=False):
                    # 1/rowsum; runs while the next block's score
                    # matmuls keep the PE busy
                    pr, qt, e_sb, psE1, psE2, psA = item
                    rs = []
                    for hi, psE in ((0, psE1), (1, psE2)):
                        if recip_mode == "act" or force_act:
                            lns = small.tile([1, QW], f32, tag="lns",
                                             name=f"ln{pr}_{qt}_{hi}")
                            nc.scalar.activation(
                                lns[:], psE[64:65, :],
                                mybir.ActivationFunctionType.Ln)
                            r16 = small.tile([1, QW], bf16, tag="r16",
                                             name=f"r16_{pr}_{qt}_{hi}")
                            nc.scalar.activation(
                                r16[:], lns[:],
                                mybir.ActivationFunctionType.Exp,
                                scale=-1.0)
                            rs.append(r16)
                            continue
                        # copy the rowsum to a partition-0 SBUF tile first:
                        # the custom-DVE recip mis-reads a partition-64
                        # PSUM operand (standard ops handle it fine)
                        rsum = small.tile([1, QW], f32, tag="rsum",
                                          name=f"rs_{pr}_{qt}_{hi}")
                        nc.vector.tensor_copy(rsum[:], psE[64:65, :])
                        r32 = small.tile([1, QW], f32, tag="r32",
                                         name=f"r32_{pr}_{qt}_{hi}")
                        nc.vector.reciprocal_approx_fast(r32[:], rsum[:])
                        if epi_mode == "gps":
                            rs.append(r32)
                        else:
                            r16 = small.tile([1, QW], bf16, tag="r16",
                                             name=f"r16_{pr}_{qt}_{hi}")
                            nc.vector.tensor_copy(r16[:], r32[:])
                            rs.append(r16)
                    return rs

                def emit_epilogue_apply(item, rs):
                    pr, qt, e_sb, psE1, psE2, psA = item
                    q0 = qt * QW
                    for hi, psE in ((0, psE1), (1, psE2)):
                        pa, pz = hi * 64, hi * 64 + 64
                        dst = attn_sb[pa:pz, pr, q0:q0 + QW]
                        if epi_mode == "gps":
                            rb = small.tile([64, QW], f32, tag="rb",
                                            name=f"rb{pr}_{qt}_{hi}")
                            nc.gpsimd.partition_broadcast(
                                rb[:], rs[hi][:], channels=64)
                            nc.vector.tensor_mul(dst, psE[0:64, :], rb[:])
                        else:
                            nc.tensor.matmul(psE[64:128, :],
                                             ones64_sb[0:1, :],
                                             rs[hi][:, :], start=True,
                                             stop=True,
                                             skip_group_check=True)
                            rb = small.tile([64, QW], f32, tag="rb",
                                            name=f"rb{pr}_{qt}_{hi}")
                            nc.vector.tensor_copy(rb[:], psE[64:128, :])
                            nc.vector.tensor_mul(dst, psE[0:64, :], rb[:])
                        nc.vector.tensor_add(dst, dst, psA[pa:pz, :])

                # qkT filler bursts inside the item loops: pairs 2..5,
                # each ct one-to-two items before its first use.  Item
                # (0,0) has no out-step work (pipeline fill), so it gets
                # two bursts.
                filler = {
                    (0, 0): [(2, 2, 0), (NPAIR + 2, 2, 1)],
                    (0, 1): [(3, 3, 0)],
                    (1, 0): [(NPAIR + 3, 3, 1)],
                    (1, 1): [(4, 4, 0)],
                    (2, 0): [(NPAIR + 4, 4, 1)],
                    (2, 1): [(5, 5, 0)],
                    (3, 0): [(NPAIR + 5, 5, 1)],
                }

                # software-pipelined emission: item i's ST/exp stream is
                # interleaved kt-by-kt with item i-1's E@v/A@v matmuls, so
                # the PE has dense work while ACT drains the score tiles
                items = [(pr, qt) for pr in range(NPAIR)
                         for qt in range(NQT)]
                prev = None        # item whose OUT runs in the current block
                pend = None        # pe-mode: (item, rs) awaiting PE/DVE apply
                e_dt = f8e4 if ev8 else bf16
                for pr, qt in items:
                    e_sb = epool.tile([P, NKT, 2 * QW], e_dt, tag="e",
                                      name=f"e{pr}_{qt}")
                    # two score steps up front cover the pending
                    # epilogue's DVE reciprocal latency
                    emit_st_step(pr, qt, e_sb, 0)
                    emit_st_step(pr, qt, e_sb, 1)
                    if pend is not None:
                        emit_epilogue_apply(*pend)
                        pend = None
                    psE1 = pp_ev.tile([P, QW], f32, tag="ev",
                                      name=f"ev1_{pr}_{qt}")
                    psE2 = pp_ev.tile([P, QW], f32, tag="ev",
                                      name=f"ev2_{pr}_{qt}")
                    psA = pp_av.tile([P, QW], f32, tag="av",
                                     name=f"av{pr}_{qt}")
                    cur = (pr, qt, e_sb, psE1, psE2, psA)
                    fill = list(filler.get((pr, qt), ()))
                    for kt in range(NKT):
                        if kt + 2 < NKT:
                            emit_st_step(pr, qt, e_sb, kt + 2)
                        if prev is not None:
                            emit_out_step(prev, kt)
                        if fill and kt in (2, 4):
                            qkt_ct(*fill.pop(0))
                    # item-boundary padding: the next item's first score
                    # steps wait on this item's last exps (ACT is the
                    # pacer once E@v runs DoubleRow); garbage written here
                    # is overwritten by this item's start=True A@v in the
                    # next block
                    i_next = items.index((pr, qt)) + 1
                    if i_next < len(items):
                        nwm = 2 if filler.get(items[i_next]) else itemwarm
                        for _ in range(nwm):
                            warm_into(psA)
                    if prev is not None:
                        rs = emit_epilogue_recip(prev)
                        if epi_mode == "gps":
                            emit_epilogue_apply(prev, rs)
                        else:
                            pend = (prev, rs)
                    prev = cur

                # ---- drain: last item's outs with proj partial bursts
                # interleaved, then the final epilogue and the output
                # projection ----
                def proj_partial(ps, ct, kcs, start, stop):
                    for kc in kcs:
                        for qh in range(NQT):
                            nc.tensor.matmul(
                                ps[:, qh * QW:(qh + 1) * QW],
                                pw_sb[:, kc, ct * P:(ct + 1) * P],
                                attn_sb[:, kc, qh * QW:(qh + 1) * QW],
                                start=(start and kc == kcs[0]),
                                stop=(stop and kc == kcs[-1]),
                                skip_group_check=True)

                with tc.tile_pool(name="ph3o", bufs=2) as ph3o:
                    out_r = out_ext.rearrange("(c p) t -> p c t", p=P)
                    pjs = {}

                    def proj_finish(ct, kcs, start):
                        ps = pjs[ct]
                        proj_partial(ps, ct, kcs, start, True)
                        o_sb = ph3o.tile([P, N], bf16, tag="o",
                                         name=f"o{ct}")
                        # alternate the bias-add between ACT and DVE so
                        # the six tail bias-adds run two-wide
                        if ct % 2 == 0:
                            nc.scalar.activation(
                                o_sb[:], ps[:],
                                mybir.ActivationFunctionType.Identity,
                                bias=pb_sb[:, ct:ct + 1])
                        else:
                            nc.vector.tensor_scalar_add(
                                o_sb[:], ps[:], pb_sb[:, ct:ct + 1])
                        nc.sync.dma_start(out_r[:, ct, :], o_sb[:])

                    for kt in range(NKT):
                        emit_out_step(prev, kt)
                        if kt == 0 and pend is not None:
                            emit_epilogue_apply(*pend)
                            pend = None
                        # the proj bursts recycle the st-score psum bufs;
                        # placed where the bufs actually free (exp kt6/kt7)
                        # so they don't stall the out-step stream
                        if kt == 5:
                            pjs[0] = pp_st.tile([P, N], f32, tag="st",
                                                name="proj0")
                            proj_partial(pjs[0], 0, list(range(NCIN - 1)),
                                         True, False)
                        if kt == 6:
                            pjs[1] = pp_st.tile([P, N], f32, tag="st",
                                                name="proj1")
                            proj_partial(pjs[1], 1, list(range(NCIN - 1)),
                                         True, False)
                    # const matmuls keep the PE busy (and the HAM clock
                    # warm) while the final epilogue chain runs on
                    # ACT/DVE; the proj finishes wait on it anyway
                    if tailwarm:
                        fav = pp_av.tile([P, QW], f32, tag="av",
                                         name="tailwarm")
                        for _ in range(tailwarm):
                            nc.tensor.matmul(
                                fav[0:64, :], ww_sb[:, 0:64],
                                wr_sb[:, :], start=True, stop=True,
                                skip_group_check=True)
                    # final epilogue: ACT recip chain — ACT is idle at
                    # the tail and the DVE chain (serial ~8us) was the
                    # dominant drain stall
                    rs = emit_epilogue_recip(prev, force_act=True)
                    emit_epilogue_apply(prev, rs)
                    proj_finish(0, [NCIN - 1], False)
                    proj_finish(1, [NCIN - 1], False)
                    for ct in range(2, NCIN):
                        pjs[ct] = pp_st.tile([P, N], f32, tag="st",
                                             name=f"proj{ct}")
                        proj_finish(ct, list(range(NCIN)), True)

    if os.environ.get("ATTN_DEDUP_LDW", "1") == "1":
        _dedup_ldweights(nc)
    if os.environ.get("ATTN_SPLIT_WAITS", "1") == "1":
        _split_excess_waits(nc)
    if not nc.is_finalized():
        nc.finalize()   # Bacc: move_matmul_waits + generate_event_semaphores
    return nc


def make_in_maps(x, qkv_w, qkv_b, static_a, proj_w, proj_b):
    """Host-side sharding / layout prep. One batch element per core."""
    x = np.asarray(x, dtype=np.float32)
    qkv_w = np.asarray(qkv_w, dtype=np.float32)
    qkv_b = np.asarray(qkv_b, dtype=np.float32)
    static_a = np.asarray(static_a, dtype=np.float32)
    proj_w = np.asarray(proj_w, dtype=np.float32)
    proj_b = np.asarray(proj_b, dtype=np.float32)

    import ml_dtypes
    bf16 = ml_dtypes.bfloat16
    f8 = ml_dtypes.float8_e4m3fn
    qk8 = os.environ.get("ATTN_QK8", "1") == "1"

    qkb_scale = 32.0 if qk8 else 1.0
    qkb = np.ascontiguousarray(
        qkv_b[0:2 * C].reshape(2 * C // P, P).T
        * qkb_scale).astype(np.float32)
    vwT = np.ascontiguousarray(qkv_w[2 * C:3 * C].T).astype(bf16)
    vb = np.ascontiguousarray(
        qkv_b[2 * C:3 * C].reshape(1, C)).astype(bf16)
    # A^T strips, contiguous per (pair, qtile, ktile): [6, 2, 8, 128, 1024]
    # at[pr, qt, kt, :, 0:512] = A^T[2pr][kt tile, qt tile], [..., 512:] = head 2pr+1
    atT = static_a[0].transpose(0, 2, 1)                      # [H, k, q]
    at = np.ascontiguousarray(
        atT.reshape(NPAIR, 2, NKT, P, NQT, QW).transpose(0, 4, 2, 3, 1, 5)
        .reshape(NPAIR, NQT, NKT, P, 2 * QW)).astype(bf16)
    pwT = np.ascontiguousarray(proj_w.T).astype(bf16)
    pb = np.ascontiguousarray(
        proj_b.reshape(C // P, P).T).astype(np.float32)

    shared = {"qkb": qkb, "vwT": vwT, "vb": vb,
              "at": at, "pwT": pwT, "pb": pb}
    if qk8:
        qkw32 = qkv_w[0:2 * C].T * 32.0                       # [768, 1536]
        shared["qkw8"] = np.ascontiguousarray(
            qkw32.reshape(NCIN // 2, 2, P, 2 * C)
            .transpose(2, 0, 1, 3)).astype(f8)
    else:
        shared["qkwT"] = np.ascontiguousarray(
            qkv_w[0:2 * C].T).astype(bf16)
    in_maps = []
    for b in range(B):
        m = dict(shared)
        xT = x[b].T
        m["xT"] = np.ascontiguousarray(xT).astype(bf16)
        if qk8:
            m["x8"] = np.ascontiguousarray(
                xT.reshape(NCIN // 2, 2, P, N)
                .transpose(2, 0, 1, 3)).astype(f8)
        in_maps.append(m)
    return in_maps


_NC_CACHE = {}


def _get_nc():
    if "nc" not in _NC_CACHE:
        _NC_CACHE["nc"] = build_nc()
    return _NC_CACHE["nc"]


def kernel(x, qkv_w, qkv_b, static_a, proj_w, proj_b):
    _ensure_paths()
    from concourse.bass_utils import run_bass_kernel_spmd

    nc = _get_nc()
    in_maps = make_in_maps(x, qkv_w, qkv_b, static_a, proj_w, proj_b)
    res = run_bass_kernel_spmd(nc, in_maps, core_ids=list(range(NCORES)))
    out = np.empty((B, N, C), dtype=np.float32)
    for b in range(B):
        out[b] = np.asarray(res.results[b]["out"], dtype=np.float32).T
    return out


# revision 38
# speedup vs baseline: 1.1131x; 1.0037x over previous
"""Trainium2 Bass kernel for nn_Attention_72438918414857.

Reference computation (B=8, N=1024, C=768, H=12, D=64):
    qkv = (x @ qkv_w.T + qkv_b) -> q, k, v per head
    attn = softmax(q @ k.T / sqrt(D)) + static_a   (bias added AFTER softmax)
    out = (attn @ v) merged-heads @ proj_w.T + proj_b

Sharding: data-parallel over batch -- one batch element per NeuronCore,
weights + static_a replicated. No collectives needed.

Math used on-chip (per batch, per head), everything transposed so each
matmul gets its contraction dim on partitions with no on-chip transposes:
    qkT = [Wq;Wk]^T-proj of x  ->  [cout, t] layout
    E^T = exp(K_h^T.T @ Q_h^T * D^-0.5)           [k, q] strips
    out_h^T = ([V_h|1].T @ E^T) -> rows 0..63 = E@v, row 64 = rowsum(E)
    attn_h^T = (E@v) * (1/rowsum) + V_h.T @ A_h^T
where static_a is pre-transposed on host to A^T[h, k, q].  The softmax
normalization is applied to the [64, q] output instead of the [k, q]
matrix; no max-subtraction is needed (|scores*scale| < ~3).

Matmuls run in bf16 (fp32 PE matmul is 4x slower); PSUM accumulation is
fp32.  bf16 rounding of operands keeps rel-err ~4e-3, well under the
2e-2 gate.

v2 scheduling changes (over the first working version):
  - warm-up matmuls on const data at t=0 so the PE HAM clock-gate
    reaches 8/8 before the real work starts (first ~20us of v1 ran at
    1.2 GHz)
  - V projection runs kc-outer in two tt-quads so the first x/vw DMA
    chunk immediately yields dense PE work
  - qkT projections for pairs 2..5 are emitted as per-item filler
    bursts inside the attention loop, filling the exp-gated PE bubbles
    at item boundaries
  - softmax reciprocal on DVE (reciprocal_approx_fast) instead of the
    ACT Ln/Exp chain (saves ~33us of ACT time; ACT runs only the big
    exps)
  - drain: proj partial bursts interleaved into the last item's
    out-steps; output stored/DMA'd as bf16
"""

import os
import sys

import numpy as np

B, N, C = 8, 1024, 768
H, D = 12, 64
NCORES = 8
P = 128
QW = 512          # q tile width (PSUM bank = 512 f32)
NQT = N // QW     # 2 q tiles
NKT = N // P      # 8 k tiles
NCIN = C // P     # 6 c_in chunks
NPAIR = H // 2    # 6 head pairs
SCALE = float(D) ** -0.5

_REPO = "/opt/trn_rl_repo"


def _ensure_paths():
    if _REPO not in sys.path:
        sys.path.insert(0, _REPO)


def _dedup_ldweights(nc):
    """Delete an Ldweights whose weights AP + tile geometry match the
    immediately preceding Ldweights on the PE stream (the weights are
    still resident in the array); its waits/updates move to the next
    instruction."""
    import concourse.mybir as mybir

    def sig(inst):
        ap = inst.ins[0]
        return (str(ap), str(getattr(inst, "tile_position", None)),
                str(getattr(inst, "tile_size", None)))

    for fn in nc.m.functions:
        for blk in fn.blocks:
            out = []
            last_sig = None
            pend_w, pend_u = [], []
            changed = False
            for inst in blk.instructions:
                op = str(inst.opcode)
                if op == "Ldweights":
                    s_ = sig(inst)
                    if s_ == last_sig:
                        si = inst.sync_info
                        if si:
                            pend_w.extend(si.on_wait or [])
                            pend_u.extend(si.on_update or [])
                        changed = True
                        continue
                    last_sig = s_
                elif op == "Matmult":
                    pass          # matmuls don't disturb loaded weights
                elif op in ("NoOp", "EventSemaphore"):
                    pass
                else:
                    last_sig = None
                if pend_w or pend_u:
                    si = inst.sync_info
                    ow = list(si.on_wait or []) if si else []
                    ou = list(si.on_update or []) if si else []
                    inst.sync_info = mybir.SyncInfo(
                        on_wait=pend_w + ow, on_update=pend_u + ou)
                    pend_w, pend_u = [], []
                out.append(inst)
            assert not pend_w and not pend_u
            if changed:
                blk.instructions = out


def _split_excess_waits(nc):
    """The TRN2 walrus codegen allows only 1 sem-wait command per
    instruction.  Tile's sem-assigner can emit more (one per logical
    proc a tile depends on).
    Move the excess onto freshly inserted same-engine NoOps placed just
    before the instruction -- engines execute in order, so waiting on a
    preceding NoOp is equivalent."""
    import concourse.mybir as mybir
    from bass_rust import InstNoOp

    nid = [0]
    for fn in nc.m.functions:
        for blk in fn.blocks:
            out = []
            changed = False
            for inst in blk.instructions:
                si = inst.sync_info
                waits = list(si.on_wait) if si and si.on_wait else []
                limit = 1
                if len(waits) > limit:
                    extra, keep = waits[:-limit], waits[-limit:]
                    inst.sync_info = si.__replace__(on_wait=keep)
                    for w in extra:
                        nop = InstNoOp(
                            name=f"{inst.name}-wsplit{nid[0]}", ins=[], outs=[])
                        nid[0] += 1
                        nop.engine = inst.engine
                        nop.sync_info = mybir.SyncInfo(
                            on_wait=[w], on_update=[])
                        out.append(nop)
                    changed = True
                out.append(inst)
            if changed:
                blk.instructions = out


def _patch_act_tables():
    """Force Bacc's activation-table chooser to the single set that
    contains every function this kernel uses (exp, identity, copy),
    so only one ACT_TABLE_LOAD (~2.7us each) is emitted."""
    import concourse.hw_specs as hw_specs
    import concourse.mybir as mybir
    if getattr(hw_specs.get_activation_tables, "_attn_patched", False):
        return
    orig = hw_specs.get_activation_tables
    keep = {mybir.ActivationFunctionType.Exp, mybir.ActivationFunctionType.Ln,
            mybir.ActivationFunctionType.Identity,
            mybir.ActivationFunctionType.Copy}

    import functools

    @functools.cache
    def patched(module_arch):
        tables = dict(orig(module_arch))
        out = {}
        for name, fns in tables.items():
            if name == "natural_log_exp_and_others":
                out[name] = fns
            else:
                out[name] = fns - keep
        return out

    patched._attn_patched = True
    hw_specs.get_activation_tables = patched
    import concourse.bacc as bacc_mod
    bacc_mod.get_activation_tables = patched


def build_nc():
    """Build the per-core Bass/Tile program."""
    _ensure_paths()
    _patch_act_tables()
    import concourse.bass as bass
    import concourse.mybir as mybir
    import concourse.tile as tile
    from concourse import bacc
    from contextlib import ExitStack

    f32 = mybir.dt.float32
    bf16 = mybir.dt.bfloat16
    f8e4 = mybir.dt.float8e4

    epi_mode = os.environ.get("ATTN_EPI", "pe")   # 'pe' | 'gps'
    nwarm = int(os.environ.get("ATTN_WARM", "16"))
    tailwarm = int(os.environ.get("ATTN_TAILWARM", "10"))
    itemwarm = int(os.environ.get("ATTN_ITEMWARM", "6"))
    at_bufs = int(os.environ.get("ATTN_AT_BUFS", "12"))
    ev8 = os.environ.get("ATTN_EV8", "1") == "1"   # DoubleRow fp8 E@v
    qk8 = os.environ.get("ATTN_QK8", "1") == "1"   # DoubleRow fp8 qkT proj

    nc = bacc.Bacc("TRN2", target_bir_lowering=False, debug=False,
                   num_devices=NCORES)

    xT_ext = nc.declare_dram_parameter("xT", [C, N], bf16, isOutput=False)
    if qk8:
        # cin-pair interleaved fp8 copies for DoubleRow (weights scaled
        # by 32 on host so w~N(0,0.02) lands in e4m3's normal range;
        # the extra 32*32 falls out of the exp scale)
        x8_ext = nc.declare_dram_parameter(
            "x8", [P, NCIN // 2, 2, N], f8e4, isOutput=False)
        qkw8_ext = nc.declare_dram_parameter(
            "qkw8", [P, NCIN // 2, 2, 2 * C], f8e4, isOutput=False)
    else:
        qkwT_ext = nc.declare_dram_parameter(
            "qkwT", [C, 2 * C], bf16, isOutput=False)
    qkb_ext = nc.declare_dram_parameter("qkb", [P, 2 * C // P], f32, isOutput=False)
    vwT_ext = nc.declare_dram_parameter("vwT", [C, C], bf16, isOutput=False)
    vb_ext = nc.declare_dram_parameter("vb", [1, C], bf16, isOutput=False)
    at_ext = nc.declare_dram_parameter(
        "at", [NPAIR, NQT, NKT, P, 2 * QW], bf16, isOutput=False)
    pwT_ext = nc.declare_dram_parameter("pwT", [C, C], bf16, isOutput=False)
    pb_ext = nc.declare_dram_parameter("pb", [P, C // P], f32, isOutput=False)
    out_ext = nc.declare_dram_parameter("out", [C, N], bf16, isOutput=True)

    with tile.TileContext(nc, num_cores=NCORES) as tc, ExitStack() as ctx:
        consts = ctx.enter_context(tc.tile_pool(name="consts", bufs=1))
        persist = ctx.enter_context(tc.tile_pool(name="persist", bufs=1))
        attn_pool = ctx.enter_context(tc.tile_pool(name="attnout", bufs=1))
        epool = ctx.enter_context(tc.tile_pool(name="epool", bufs=2))
        atbf = ctx.enter_context(tc.tile_pool(name="atbf", bufs=at_bufs))
        small = ctx.enter_context(tc.tile_pool(name="small", bufs=2))

        qkb_sb = consts.tile([P, 2 * C // P], f32)
        pb_sb = consts.tile([P, NCIN], f32)
        vb_sb = consts.tile([1, C], bf16)
        # memsets for the warmup constants go on DVE: the gpsimd queue is
        # busy with the Tile prologue (sem clears) for the first ~3us and
        # would delay the HAM warmup matmuls
        ones_sb = consts.tile([1, P], bf16)
        nc.vector.memset(ones_sb[:], 1.0)
        ones64_sb = consts.tile([1, 64], bf16)
        nc.vector.memset(ones64_sb[:], 1.0)
        # warm matmuls must engage the full 128-row array: the HAM
        # activity monitor does not register K=1 matmuls as PE-busy
        ww_sb = consts.tile([P, 64], bf16)
        nc.vector.memset(ww_sb[:], 0.0)
        wr_sb = consts.tile([P, 512], bf16)
        nc.vector.memset(wr_sb[:], 0.0)

        # persistent activations (bf16 matmul operands)
        qkT_prs = [persist.tile([P, 2, N], bf16, tag=f"qkt{p}",
                                name=f"qkt{p}")
                   for p in range(NPAIR)]
        vp_sb = persist.tile([P, H, NKT, 65], bf16)   # [V_h | 1] stationary
        nc.any.memset(vp_sb[:, :, :, 64:65], 1.0)
        if ev8:
            # fp8 copy of [V_h | 1] with k-tile pairs interleaved along
            # the free axis for DoubleRow E@v (ko stride 80: %16 rule)
            vp8_sb = persist.tile([P, H, NKT // 2, 2, 80], f8e4)
            nc.any.memset(vp8_sb[:, :, :, :, 64:65], 1.0)
        pw_sb = persist.tile([P, NCIN, C], bf16)      # proj weights
        attn_sb = attn_pool.tile([P, NCIN, N], bf16)  # attention out^T

        if epi_mode == "gps":
            from concourse import library_config
            nc.gpsimd.load_library(library_config.attn)

        with tc.tile_pool(name="ph1", bufs=1) as ph1:
            xT_sb = ph1.tile([P, NCIN, N], bf16)
            vw_sb = ph1.tile([P, NCIN, C], bf16)
            # direct DMA loads (host pre-casts).  Emission order on
            # the sync queue == descriptor order per DMA queue, so x/vw
            # stream first, then qkw; pw + at tiles are emitted later.
            xT_r = xT_ext.rearrange("(c p) t -> p c t", p=P)
            vw_r = vwT_ext.rearrange("(c p) n -> p c n", p=P)
            pw_r = pwT_ext.rearrange("(c p) n -> p c n", p=P)
            for kc in range(NCIN):
                nc.sync.dma_start(xT_sb[:, kc, :], xT_r[:, kc, :])
                nc.sync.dma_start(vw_sb[:, kc, :], vw_r[:, kc, :])
                if kc == 0:
                    nc.sync.dma_start(vb_sb[:], vb_ext[:])
                    nc.sync.dma_start(qkb_sb[:], qkb_ext[:])
                    nc.sync.dma_start(pb_sb[:], pb_ext[:])
            if qk8:
                x8_sb = ph1.tile([P, NCIN // 2, 2, N], f8e4)
                qkw8_sb = ph1.tile([P, NCIN // 2, 2, 2 * C], f8e4)
                for g in range(NCIN // 2):
                    nc.sync.dma_start(x8_sb[:, g, :, :], x8_ext[:, g, :, :])
                    nc.sync.dma_start(qkw8_sb[:, g, :, :],
                                      qkw8_ext[:, g, :, :])
            else:
                qkw_sb = ph1.tile([P, NCIN, 2 * C], bf16)
                qkw_r = qkwT_ext.rearrange("(c p) n -> p c n", p=P)
                for kc in range(NCIN):
                    nc.sync.dma_start(qkw_sb[:, kc, :], qkw_r[:, kc, :])

            # ---- HAM warm-up + V projection.  Const matmuls keep the PE
            # busy while the first x/vw chunks stream in, so the clock
            # gate opens to 8/8 (~3.4us sustained) before real work; a
            # few more const matmuls pad the DMA-paced first group so
            # the busy window stays unbroken. ----
            with tc.tile_pool(name="pp_w", bufs=1, space="PSUM") as pp_w, \
                 tc.tile_pool(name="pp_v", bufs=3, space="PSUM") as pp_v:
                wps = pp_w.tile([64, 512], f32)

                def warm_mm(n):
                    nc.tensor.matmul(
                        wps[:, 0:n], ww_sb[:, 0:64], wr_sb[:, 0:n],
                        start=True, stop=True, skip_group_check=True)

                for _ in range(nwarm):
                    warm_mm(256)

                for gi, tts in enumerate(((0, 1, 2), (3, 4, 5), (6, 7))):
                    pss = {tt: pp_v.tile([P, C], f32, tag="v",
                                         name=f"vps{tt}") for tt in tts}
                    if gi:
                        # pad the group seam (psum rotation wait on the
                        # previous group's DVE copies)
                        for _ in range(3):
                            warm_mm(512)
                    for kc in range(NCIN):
                        for tt in tts:
                            for (n0, nw) in ((0, QW), (QW, C - QW)):
                                nc.tensor.matmul(
                                    pss[tt][:, n0:n0 + nw],
                                    xT_sb[:, kc, tt * P:(tt + 1) * P],
                                    vw_sb[:, kc, n0:n0 + nw],
                                    start=(kc == 0), stop=False,
                                    skip_group_check=True)
                        if gi == 0 and kc < 5:
                            # absorb per-chunk DMA lateness so the HAM
                            # busy window is not broken
                            warm_mm(256)
                            warm_mm(256)
                    for tt in tts:
                        for (n0, nw) in ((0, QW), (QW, C - QW)):
                            nc.tensor.matmul(
                                pss[tt][:, n0:n0 + nw],
                                ones_sb[0:1, 0:P],
                                vb_sb[0:1, n0:n0 + nw],
                                start=False, stop=True,
                                skip_group_check=True)
                        nc.vector.tensor_copy(
                            vp_sb[:, :, tt, 0:64],
                            pss[tt].rearrange("p (h d) -> p h d", d=64))
                        if ev8:
                            # on ACT (idle in phase 1): the psum-pool
                            # rotation waits on these copies, and two
                            # serial DVE casts per tile stall the PE at
                            # every V-group seam
                            nc.scalar.copy(
                                vp8_sb[:, :, tt // 2, tt % 2, 0:64],
                                pss[tt].rearrange("p (h d) -> p h d", d=64))

            # ---- attention (+ interleaved qkT / proj work) ----
            with tc.tile_pool(name="pp_st", bufs=2, space="PSUM") as pp_st, \
                 tc.tile_pool(name="pp_ev", bufs=2, space="PSUM") as pp_ev, \
                     tc.tile_pool(name="pp_av", bufs=2, space="PSUM") as pp_av:

                def qkt_ct(ct, pr_dst, qki):
                    """One qkT output tile (6 or 3 matmuls) + bias-add."""
                    ps = pp_st.tile([P, N], f32, tag="st", name=f"qk{ct}")
                    if qk8:
                        for g in range(NCIN // 2):
                            for qh in range(NQT):
                                nc.tensor.matmul(
                                    ps[:, qh * QW:(qh + 1) * QW],
                                    qkw8_sb[:, g, :, ct * P:(ct + 1) * P],
                                    x8_sb[:, g, :, qh * QW:(qh + 1) * QW],
                                    start=(g == 0),
                                    stop=(g == NCIN // 2 - 1),
                                    perf_mode=mybir.MatmulPerfMode.DoubleRow,
                                    skip_group_check=True)
                    else:
                        for kc in range(NCIN):
                            for qh in range(NQT):
                                nc.tensor.matmul(
                                    ps[:, qh * QW:(qh + 1) * QW],
                                    qkw_sb[:, kc, ct * P:(ct + 1) * P],
                                    xT_sb[:, kc, qh * QW:(qh + 1) * QW],
                                    start=(kc == 0), stop=(kc == NCIN - 1),
                                    skip_group_check=True)
                    nc.vector.tensor_scalar_add(
                        qkT_prs[pr_dst][:, qki, :], ps[:, :],
                        qkb_sb[:, ct:ct + 1])

                def qkt_group(pr):
                    qkt_ct(pr, pr, 0)
                    qkt_ct(NPAIR + pr, pr, 1)

                def warm_into(ps, n=QW):
                    # full-K const matmul into a psum region whose next
                    # real matmul is start=True (overwrites the garbage):
                    # keeps the PE busy so the HAM clock stays at 8/8
                    nc.tensor.matmul(
                        ps[0:64, 0:n], ww_sb[:, 0:64],
                        wr_sb[:, 0:n], start=True, stop=True,
                        skip_group_check=True)

                # dummy av-tag tile: a scratch psum bank for padding the
                # qkT-upfront stretch (paced by the qkw DMA stream)
                avw = pp_av.tile([P, QW], f32, tag="av", name="phasewarm")
                for ct_i, (ct, prd, qki) in enumerate(
                        ((0, 0, 0), (NPAIR, 0, 1), (1, 1, 0),
                         (NPAIR + 1, 1, 1))):
                    qkt_ct(ct, prd, qki)
                    for _ in range(3):
                        warm_into(avw)

                # proj weights stream after x/qkw/vw, before the at tiles
                for kc in range(NCIN):
                    nc.sync.dma_start(pw_sb[:, kc, :], pw_r[:, kc, :])

                # with fp8 qkT, qkT_prs holds 32*(q+b) per side -> the
                # score psum carries an extra 1024x, folded into the exp
                exp_scale = SCALE / 1024.0 if qk8 else SCALE

                def emit_st_step(pr, qt, e_sb, kt):
                    q0 = qt * QW
                    st = pp_st.tile([P, 2 * QW], f32, tag="st",
                                    name=f"st{pr}_{qt}_{kt}")
                    k0 = kt * P
                    nc.tensor.matmul(
                        st[:, 0:QW],
                        qkT_prs[pr][0:64, 1, k0:k0 + P],
                        qkT_prs[pr][0:64, 0, q0:q0 + QW],
                        start=True, stop=True)
                    nc.tensor.matmul(
                        st[:, QW:2 * QW],
                        qkT_prs[pr][64:128, 1, k0:k0 + P],
                        qkT_prs[pr][64:128, 0, q0:q0 + QW],
                        start=True, stop=True)
                    nc.scalar.activation(
                        e_sb[:, kt, :], st[:, :],
                        mybir.ActivationFunctionType.Exp, scale=exp_scale)

                def emit_out_step(item, kt):
                    pr, qt, e_sb, psE1, psE2, psA = item
                    h1, h2 = 2 * pr, 2 * pr + 1
                    at = atbf.tile([P, 2 * QW], bf16, tag="atb",
                                   name=f"atb{pr}_{qt}_{kt}")
                    nc.sync.dma_start(at[:], at_ext[pr, qt, kt])
                    st_flags = dict(start=(kt == 0), stop=(kt == NKT - 1),
                                    skip_group_check=True)
                    nc.tensor.matmul(
                        psA[0:64, :], vp_sb[:, h1, kt, 0:64],
                        at[:, 0:QW], **st_flags)
                    nc.tensor.matmul(
                        psA[64:128, :], vp_sb[:, h2, kt, 0:64],
                        at[:, QW:2 * QW], **st_flags)
                    if ev8:
                        # DoubleRow fp8: one matmul covers a k-tile pair
                        if kt % 2 == 1:
                            g = kt // 2
                            dr_flags = dict(
                                start=(g == 0), stop=(g == NKT // 2 - 1),
                                perf_mode=mybir.MatmulPerfMode.DoubleRow,
                                skip_group_check=True)
                            nc.tensor.matmul(
                                psE1[0:65, :],
                                vp8_sb[:, h1, g, :, 0:65],
                                e_sb[:, 2 * g:2 * g + 2, 0:QW], **dr_flags)
                            nc.tensor.matmul(
                                psE2[0:65, :],
                                vp8_sb[:, h2, g, :, 0:65],
                                e_sb[:, 2 * g:2 * g + 2, QW:2 * QW],
                                **dr_flags)
                    else:
                        nc.tensor.matmul(
                            psE1[0:65, :], vp_sb[:, h1, kt, :],
                            e_sb[:, kt, 0:QW], **st_flags)
                        nc.tensor.matmul(
                            psE2[0:65, :], vp_sb[:, h2, kt, :],
                            e_sb[:, kt, QW:2 * QW], **st_flags)

                recip_mode = os.environ.get("ATTN_RECIP", "dve")

                def emit_epilogue_recip(item, force_act=False):
                    # 1/rowsum; runs while the next block's score
                    # matmuls keep the PE busy
                    pr, qt, e_sb, psE1, psE2, psA = item
                    rs = []
                    for hi, psE in ((0, psE1), (1, psE2)):
                        if recip_mode == "act" or force_act:
                            lns = small.tile([1, QW], f32, tag="lns",
                                             name=f"ln{pr}_{qt}_{hi}")
                            nc.scalar.activation(
                                lns[:], psE[64:65, :],
                                mybir.ActivationFunctionType.Ln)
                            r16 = small.tile([1, QW], bf16, tag="r16",
                                             name=f"r16_{pr}_{qt}_{hi}")
                            nc.scalar.activation(
                                r16[:], lns[:],
                                mybir.ActivationFunctionType.Exp,
                                scale=-1.0)
                            rs.append(r16)
                            continue
                        # copy the rowsum to a partition-0 SBUF tile first:
                        # the custom-DVE recip mis-reads a partition-64
                        # PSUM operand (standard ops handle it fine)
                        rsum = small.tile([1, QW], f32, tag="rsum",
                                          name=f"rs_{pr}_{qt}_{hi}")
                        nc.vector.tensor_copy(rsum[:], psE[64:65, :])
                        r32 = small.tile([1, QW], f32, tag="r32",
                                         name=f"r32_{pr}_{qt}_{hi}")
                        nc.vector.reciprocal_approx_fast(r32[:], rsum[:])
                        if epi_mode == "gps":
                            rs.append(r32)
                        else:
                            r16 = small.tile([1, QW], bf16, tag="r16",
                                             name=f"r16_{pr}_{qt}_{hi}")
                            nc.vector.tensor_copy(r16[:], r32[:])
                            rs.append(r16)
                    return rs

                def emit_epilogue_apply(item, rs):
                    pr, qt, e_sb, psE1, psE2, psA = item
                    q0 = qt * QW
                    for hi, psE in ((0, psE1), (1, psE2)):
                        pa, pz = hi * 64, hi * 64 + 64
                        dst = attn_sb[pa:pz, pr, q0:q0 + QW]
                        if epi_mode == "gps":
                            rb = small.tile([64, QW], f32, tag="rb",
                                            name=f"rb{pr}_{qt}_{hi}")
                            nc.gpsimd.partition_broadcast(
                                rb[:], rs[hi][:], channels=64)
                            nc.vector.tensor_mul(dst, psE[0:64, :], rb[:])
                        else:
                            nc.tensor.matmul(psE[64:128, :],
                                             ones64_sb[0:1, :],
                                             rs[hi][:, :], start=True,
                                             stop=True,
                                             skip_group_check=True)
                            rb = small.tile([64, QW], f32, tag="rb",
                                            name=f"rb{pr}_{qt}_{hi}")
                            nc.vector.tensor_copy(rb[:], psE[64:128, :])
                            nc.vector.tensor_mul(dst, psE[0:64, :], rb[:])
                        nc.vector.tensor_add(dst, dst, psA[pa:pz, :])

                # qkT filler bursts inside the item loops: pairs 2..5,
                # each ct one-to-two items before its first use.  Item
                # (0,0) has no out-step work (pipeline fill), so it gets
                # two bursts.
                filler = {
                    (0, 0): [(2, 2, 0), (NPAIR + 2, 2, 1)],
                    (0, 1): [(3, 3, 0)],
                    (1, 0): [(NPAIR + 3, 3, 1)],
                    (1, 1): [(4, 4, 0)],
                    (2, 0): [(NPAIR + 4, 4, 1)],
                    (2, 1): [(5, 5, 0)],
                    (3, 0): [(NPAIR + 5, 5, 1)],
                }

                # software-pipelined emission: item i's ST/exp stream is
                # interleaved kt-by-kt with item i-1's E@v/A@v matmuls, so
                # the PE has dense work while ACT drains the score tiles
                items = [(pr, qt) for pr in range(NPAIR)
                         for qt in range(NQT)]
                prev = None        # item whose OUT runs in the current block
                pend = None        # pe-mode: (item, rs) awaiting PE/DVE apply
                e_dt = f8e4 if ev8 else bf16
                for pr, qt in items:
                    e_sb = epool.tile([P, NKT, 2 * QW], e_dt, tag="e",
                                      name=f"e{pr}_{qt}")
                    # two score steps up front cover the pending
                    # epilogue's DVE reciprocal latency
                    emit_st_step(pr, qt, e_sb, 0)
                    emit_st_step(pr, qt, e_sb, 1)
                    if pend is not None:
                        emit_epilogue_apply(*pend)
                        pend = None
                    psE1 = pp_ev.tile([P, QW], f32, tag="ev",
                                      name=f"ev1_{pr}_{qt}")
                    psE2 = pp_ev.tile([P, QW], f32, tag="ev",
                                      name=f"ev2_{pr}_{qt}")
                    psA = pp_av.tile([P, QW], f32, tag="av",
                                     name=f"av{pr}_{qt}")
                    cur = (pr, qt, e_sb, psE1, psE2, psA)
                    fill = list(filler.get((pr, qt), ()))
                    for kt in range(NKT):
                        if kt + 2 < NKT:
                            emit_st_step(pr, qt, e_sb, kt + 2)
                        if prev is not None:
                            emit_out_step(prev, kt)
                        if fill and kt in (2, 4):
                            qkt_ct(*fill.pop(0))
                    # item-boundary padding: the next item's first score
                    # steps wait on this item's last exps (ACT is the
                    # pacer once E@v runs DoubleRow); garbage written here
                    # is overwritten by this item's start=True A@v in the
                    # next block
                    i_next = items.index((pr, qt)) + 1
                    if i_next < len(items):
                        nwm = 2 if filler.get(items[i_next]) else itemwarm
                        for _ in range(nwm):
                            warm_into(psA)
                    if prev is not None:
                        rs = emit_epilogue_recip(prev)
                        if epi_mode == "gps":
                            emit_epilogue_apply(prev, rs)
                        else:
                            pend = (prev, rs)
                    prev = cur

                # ---- drain: last item's outs with proj partial bursts
                # interleaved, then the final epilogue and the output
                # projection ----
                def proj_partial(ps, ct, kcs, start, stop):
                    for kc in kcs:
                        for qh in range(NQT):
                            nc.tensor.matmul(
                                ps[:, qh * QW:(qh + 1) * QW],
                                pw_sb[:, kc, ct * P:(ct + 1) * P],
                                attn_sb[:, kc, qh * QW:(qh + 1) * QW],
                                start=(start and kc == kcs[0]),
                                stop=(stop and kc == kcs[-1]),
                                skip_group_check=True)

                with tc.tile_pool(name="ph3o", bufs=2) as ph3o:
                    out_r = out_ext.rearrange("(c p) t -> p c t", p=P)
                    pjs = {}

                    def proj_finish(ct, kcs, start):
                        ps = pjs[ct]
                        proj_partial(ps, ct, kcs, start, True)
                        o_sb = ph3o.tile([P, N], bf16, tag="o",
                                         name=f"o{ct}")
                        # alternate the bias-add between ACT and DVE so
                        # the six tail bias-adds run two-wide
                        if ct % 2 == 0:
                            nc.scalar.activation(
                                o_sb[:], ps[:],
                                mybir.ActivationFunctionType.Identity,
                                bias=pb_sb[:, ct:ct + 1])
                        else:
                            nc.vector.tensor_scalar_add(
                                o_sb[:], ps[:], pb_sb[:, ct:ct + 1])
                        nc.sync.dma_start(out_r[:, ct, :], o_sb[:])

                    for kt in range(NKT):
                        emit_out_step(prev, kt)
                        if kt == 0 and pend is not None:
                            emit_epilogue_apply(*pend)
                            pend = None
                        # the proj bursts recycle the st-score psum bufs;
                        # placed where the bufs actually free (exp kt6/kt7)
                        # so they don't stall the out-step stream
                        if kt == 5:
                            pjs[0] = pp_st.tile([P, N], f32, tag="st",
                                                name="proj0")
                            for _ in range(3):
                                warm_into(pjs[0])
                            proj_partial(pjs[0], 0, list(range(NCIN - 1)),
                                         True, False)
                        if kt == 6:
                            pjs[1] = pp_st.tile([P, N], f32, tag="st",
                                                name="proj1")
                            for _ in range(3):
                                warm_into(pjs[1])
                            proj_partial(pjs[1], 1, list(range(NCIN - 1)),
                                         True, False)
                    # const matmuls keep the PE busy (and the HAM clock
                    # warm) while the final epilogue chain runs on
                    # ACT/DVE; the proj finishes wait on it anyway
                    if tailwarm:
                        fav = pp_av.tile([P, QW], f32, tag="av",
                                         name="tailwarm")
                        for _ in range(tailwarm):
                            nc.tensor.matmul(
                                fav[0:64, :], ww_sb[:, 0:64],
                                wr_sb[:, :], start=True, stop=True,
                                skip_group_check=True)
                    # final epilogue: ACT recip chain — ACT is idle at
                    # the tail and the DVE chain (serial ~8us) was the
                    # dominant drain stall
                    rs = emit_epilogue_recip(prev, force_act=True)
                    emit_epilogue_apply(prev, rs)
                    proj_finish(0, [NCIN - 1], False)
                    proj_finish(1, [NCIN - 1], False)
                    for ct in range(2, NCIN):
                        pjs[ct] = pp_st.tile([P, N], f32, tag="st",
                                             name=f"proj{ct}")
                        proj_finish(ct, list(range(NCIN)), True)

    if os.environ.get("ATTN_DEDUP_LDW", "1") == "1":
        _dedup_ldweights(nc)
    if os.environ.get("ATTN_SPLIT_WAITS", "1") == "1":
        _split_excess_waits(nc)
    if not nc.is_finalized():
        nc.finalize()   # Bacc: move_matmul_waits + generate_event_semaphores
    return nc


def make_in_maps(x, qkv_w, qkv_b, static_a, proj_w, proj_b):
    """Host-side sharding / layout prep. One batch element per core."""
    x = np.asarray(x, dtype=np.float32)
    qkv_w = np.asarray(qkv_w, dtype=np.float32)
    qkv_b = np.asarray(qkv_b, dtype=np.float32)
    static_a = np.asarray(static_a, dtype=np.float32)
    proj_w = np.asarray(proj_w, dtype=np.float32)
    proj_b = np.asarray(proj_b, dtype=np.float32)

    import ml_dtypes
    bf16 = ml_dtypes.bfloat16
    f8 = ml_dtypes.float8_e4m3fn
    qk8 = os.environ.get("ATTN_QK8", "1") == "1"

    qkb_scale = 32.0 if qk8 else 1.0
    qkb = np.ascontiguousarray(
        qkv_b[0:2 * C].reshape(2 * C // P, P).T
        * qkb_scale).astype(np.float32)
    vwT = np.ascontiguousarray(qkv_w[2 * C:3 * C].T).astype(bf16)
    vb = np.ascontiguousarray(
        qkv_b[2 * C:3 * C].reshape(1, C)).astype(bf16)
    # A^T strips, contiguous per (pair, qtile, ktile): [6, 2, 8, 128, 1024]
    # at[pr, qt, kt, :, 0:512] = A^T[2pr][kt tile, qt tile], [..., 512:] = head 2pr+1
    atT = static_a[0].transpose(0, 2, 1)                      # [H, k, q]
    at = np.ascontiguousarray(
        atT.reshape(NPAIR, 2, NKT, P, NQT, QW).transpose(0, 4, 2, 3, 1, 5)
        .reshape(NPAIR, NQT, NKT, P, 2 * QW)).astype(bf16)
    pwT = np.ascontiguousarray(proj_w.T).astype(bf16)
    pb = np.ascontiguousarray(
        proj_b.reshape(C // P, P).T).astype(np.float32)

    shared = {"qkb": qkb, "vwT": vwT, "vb": vb,
              "at": at, "pwT": pwT, "pb": pb}
    if qk8:
        qkw32 = qkv_w[0:2 * C].T * 32.0                       # [768, 1536]
        shared["qkw8"] = np.ascontiguousarray(
            qkw32.reshape(NCIN // 2, 2, P, 2 * C)
            .transpose(2, 0, 1, 3)).astype(f8)
    else:
        shared["qkwT"] = np.ascontiguousarray(
            qkv_w[0:2 * C].T).astype(bf16)
    in_maps = []
    for b in range(B):
        m = dict(shared)
        xT = x[b].T
        m["xT"] = np.ascontiguousarray(xT).astype(bf16)
        if qk8:
            m["x8"] = np.ascontiguousarray(
                xT.reshape(NCIN // 2, 2, P, N)
                .transpose(2, 0, 1, 3)).astype(f8)
        in_maps.append(m)
    return in_maps


_NC_CACHE = {}


def _get_nc():
    if "nc" not in _NC_CACHE:
        _NC_CACHE["nc"] = build_nc()
    return _NC_CACHE["nc"]


def kernel(x, qkv_w, qkv_b, static_a, proj_w, proj_b):
    _ensure_paths()
    from concourse.bass_utils import run_bass_kernel_spmd

    nc = _get_nc()
    in_maps = make_in_maps(x, qkv_w, qkv_b, static_a, proj_w, proj_b)
    res = run_bass_kernel_spmd(nc, in_maps, core_ids=list(range(NCORES)))
    out = np.empty((B, N, C), dtype=np.float32)
    for b in range(B):
        out[b] = np.asarray(res.results[b]["out"], dtype=np.float32).T
    return out


# revision 40
# speedup vs baseline: 1.1135x; 1.0004x over previous
"""Trainium2 Bass kernel for nn_Attention_72438918414857.

Reference computation (B=8, N=1024, C=768, H=12, D=64):
    qkv = (x @ qkv_w.T + qkv_b) -> q, k, v per head
    attn = softmax(q @ k.T / sqrt(D)) + static_a   (bias added AFTER softmax)
    out = (attn @ v) merged-heads @ proj_w.T + proj_b

Sharding: data-parallel over batch -- one batch element per NeuronCore,
weights + static_a replicated. No collectives needed.

Math used on-chip (per batch, per head), everything transposed so each
matmul gets its contraction dim on partitions with no on-chip transposes:
    qkT = [Wq;Wk]^T-proj of x  ->  [cout, t] layout
    E^T = exp(K_h^T.T @ Q_h^T * D^-0.5)           [k, q] strips
    out_h^T = ([V_h|1].T @ E^T) -> rows 0..63 = E@v, row 64 = rowsum(E)
    attn_h^T = (E@v) * (1/rowsum) + V_h.T @ A_h^T
where static_a is pre-transposed on host to A^T[h, k, q].  The softmax
normalization is applied to the [64, q] output instead of the [k, q]
matrix; no max-subtraction is needed (|scores*scale| < ~3).

Matmuls run in bf16 (fp32 PE matmul is 4x slower); PSUM accumulation is
fp32.  bf16 rounding of operands keeps rel-err ~4e-3, well under the
2e-2 gate.

v2 scheduling changes (over the first working version):
  - warm-up matmuls on const data at t=0 so the PE HAM clock-gate
    reaches 8/8 before the real work starts (first ~20us of v1 ran at
    1.2 GHz)
  - V projection runs kc-outer in two tt-quads so the first x/vw DMA
    chunk immediately yields dense PE work
  - qkT projections for pairs 2..5 are emitted as per-item filler
    bursts inside the attention loop, filling the exp-gated PE bubbles
    at item boundaries
  - softmax reciprocal on DVE (reciprocal_approx_fast) instead of the
    ACT Ln/Exp chain (saves ~33us of ACT time; ACT runs only the big
    exps)
  - drain: proj partial bursts interleaved into the last item's
    out-steps; output stored/DMA'd as bf16
"""

import os
import sys

import numpy as np

B, N, C = 8, 1024, 768
H, D = 12, 64
NCORES = 8
P = 128
QW = 512          # q tile width (PSUM bank = 512 f32)
NQT = N // QW     # 2 q tiles
NKT = N // P      # 8 k tiles
NCIN = C // P     # 6 c_in chunks
NPAIR = H // 2    # 6 head pairs
SCALE = float(D) ** -0.5

_REPO = "/opt/trn_rl_repo"


def _ensure_paths():
    if _REPO not in sys.path:
        sys.path.insert(0, _REPO)


def _dedup_ldweights(nc):
    """Delete an Ldweights whose weights AP + tile geometry match the
    immediately preceding Ldweights on the PE stream (the weights are
    still resident in the array); its waits/updates move to the next
    instruction."""
    import concourse.mybir as mybir

    def sig(inst):
        ap = inst.ins[0]
        return (str(ap), str(getattr(inst, "tile_position", None)),
                str(getattr(inst, "tile_size", None)))

    for fn in nc.m.functions:
        for blk in fn.blocks:
            out = []
            last_sig = None
            pend_w, pend_u = [], []
            changed = False
            for inst in blk.instructions:
                op = str(inst.opcode)
                if op == "Ldweights":
                    s_ = sig(inst)
                    if s_ == last_sig:
                        si = inst.sync_info
                        if si:
                            pend_w.extend(si.on_wait or [])
                            pend_u.extend(si.on_update or [])
                        changed = True
                        continue
                    last_sig = s_
                elif op == "Matmult":
                    pass          # matmuls don't disturb loaded weights
                elif op in ("NoOp", "EventSemaphore"):
                    pass
                else:
                    last_sig = None
                if pend_w or pend_u:
                    si = inst.sync_info
                    ow = list(si.on_wait or []) if si else []
                    ou = list(si.on_update or []) if si else []
                    inst.sync_info = mybir.SyncInfo(
                        on_wait=pend_w + ow, on_update=pend_u + ou)
                    pend_w, pend_u = [], []
                out.append(inst)
            assert not pend_w and not pend_u
            if changed:
                blk.instructions = out


def _split_excess_waits(nc):
    """The TRN2 walrus codegen allows only 1 sem-wait command per
    instruction.  Tile's sem-assigner can emit more (one per logical
    proc a tile depends on).
    Move the excess onto freshly inserted same-engine NoOps placed just
    before the instruction -- engines execute in order, so waiting on a
    preceding NoOp is equivalent."""
    import concourse.mybir as mybir
    from bass_rust import InstNoOp

    nid = [0]
    for fn in nc.m.functions:
        for blk in fn.blocks:
            out = []
            changed = False
            for inst in blk.instructions:
                si = inst.sync_info
                waits = list(si.on_wait) if si and si.on_wait else []
                limit = 1
                if len(waits) > limit:
                    extra, keep = waits[:-limit], waits[-limit:]
                    inst.sync_info = si.__replace__(on_wait=keep)
                    for w in extra:
                        nop = InstNoOp(
                            name=f"{inst.name}-wsplit{nid[0]}", ins=[], outs=[])
                        nid[0] += 1
                        nop.engine = inst.engine
                        nop.sync_info = mybir.SyncInfo(
                            on_wait=[w], on_update=[])
                        out.append(nop)
                    changed = True
                out.append(inst)
            if changed:
                blk.instructions = out


def _patch_act_tables():
    """Force Bacc's activation-table chooser to the single set that
    contains every function this kernel uses (exp, identity, copy),
    so only one ACT_TABLE_LOAD (~2.7us each) is emitted."""
    import concourse.hw_specs as hw_specs
    import concourse.mybir as mybir
    if getattr(hw_specs.get_activation_tables, "_attn_patched", False):
        return
    orig = hw_specs.get_activation_tables
    keep = {mybir.ActivationFunctionType.Exp, mybir.ActivationFunctionType.Ln,
            mybir.ActivationFunctionType.Identity,
            mybir.ActivationFunctionType.Copy}

    import functools

    @functools.cache
    def patched(module_arch):
        tables = dict(orig(module_arch))
        out = {}
        for name, fns in tables.items():
            if name == "natural_log_exp_and_others":
                out[name] = fns
            else:
                out[name] = fns - keep
        return out

    patched._attn_patched = True
    hw_specs.get_activation_tables = patched
    import concourse.bacc as bacc_mod
    bacc_mod.get_activation_tables = patched


def build_nc():
    """Build the per-core Bass/Tile program."""
    _ensure_paths()
    _patch_act_tables()
    import concourse.bass as bass
    import concourse.mybir as mybir
    import concourse.tile as tile
    from concourse import bacc
    from contextlib import ExitStack

    f32 = mybir.dt.float32
    bf16 = mybir.dt.bfloat16
    f8e4 = mybir.dt.float8e4

    epi_mode = os.environ.get("ATTN_EPI", "pe")   # 'pe' | 'gps'
    nwarm = int(os.environ.get("ATTN_WARM", "16"))
    tailwarm = int(os.environ.get("ATTN_TAILWARM", "10"))
    itemwarm = int(os.environ.get("ATTN_ITEMWARM", "6"))
    at_bufs = int(os.environ.get("ATTN_AT_BUFS", "12"))
    ev8 = os.environ.get("ATTN_EV8", "1") == "1"   # DoubleRow fp8 E@v
    qk8 = os.environ.get("ATTN_QK8", "1") == "1"   # DoubleRow fp8 qkT proj

    nc = bacc.Bacc("TRN2", target_bir_lowering=False, debug=False,
                   num_devices=NCORES)

    xT_ext = nc.declare_dram_parameter("xT", [C, N], bf16, isOutput=False)
    if qk8:
        # cin-pair interleaved fp8 copies for DoubleRow (weights scaled
        # by 32 on host so w~N(0,0.02) lands in e4m3's normal range;
        # the extra 32*32 falls out of the exp scale)
        x8_ext = nc.declare_dram_parameter(
            "x8", [P, NCIN // 2, 2, N], f8e4, isOutput=False)
        qkw8_ext = nc.declare_dram_parameter(
            "qkw8", [P, NCIN // 2, 2, 2 * C], f8e4, isOutput=False)
    else:
        qkwT_ext = nc.declare_dram_parameter(
            "qkwT", [C, 2 * C], bf16, isOutput=False)
    qkb_ext = nc.declare_dram_parameter("qkb", [P, 2 * C // P], f32, isOutput=False)
    vwT_ext = nc.declare_dram_parameter("vwT", [C, C], bf16, isOutput=False)
    vb_ext = nc.declare_dram_parameter("vb", [1, C], bf16, isOutput=False)
    at_ext = nc.declare_dram_parameter(
        "at", [NPAIR, NQT, NKT, P, 2 * QW], bf16, isOutput=False)
    pwT_ext = nc.declare_dram_parameter("pwT", [C, C], bf16, isOutput=False)
    pb_ext = nc.declare_dram_parameter("pb", [P, C // P], f32, isOutput=False)
    out_ext = nc.declare_dram_parameter("out", [C, N], bf16, isOutput=True)

    with tile.TileContext(nc, num_cores=NCORES) as tc, ExitStack() as ctx:
        consts = ctx.enter_context(tc.tile_pool(name="consts", bufs=1))
        persist = ctx.enter_context(tc.tile_pool(name="persist", bufs=1))
        attn_pool = ctx.enter_context(tc.tile_pool(name="attnout", bufs=1))
        epool = ctx.enter_context(tc.tile_pool(name="epool", bufs=2))
        atbf = ctx.enter_context(tc.tile_pool(name="atbf", bufs=at_bufs))
        small = ctx.enter_context(tc.tile_pool(name="small", bufs=2))

        qkb_sb = consts.tile([P, 2 * C // P], f32)
        pb_sb = consts.tile([P, NCIN], f32)
        vb_sb = consts.tile([1, C], bf16)
        # memsets for the warmup constants go on DVE: the gpsimd queue is
        # busy with the Tile prologue (sem clears) for the first ~3us and
        # would delay the HAM warmup matmuls
        ones_sb = consts.tile([1, P], bf16)
        nc.vector.memset(ones_sb[:], 1.0)
        ones64_sb = consts.tile([1, 64], bf16)
        nc.vector.memset(ones64_sb[:], 1.0)
        # warm matmuls must engage the full 128-row array: the HAM
        # activity monitor does not register K=1 matmuls as PE-busy
        ww_sb = consts.tile([P, 64], bf16)
        nc.vector.memset(ww_sb[:], 0.0)
        wr_sb = consts.tile([P, 512], bf16)
        nc.vector.memset(wr_sb[:], 0.0)

        # persistent activations (bf16 matmul operands)
        qkT_prs = [persist.tile([P, 2, N], bf16, tag=f"qkt{p}",
                                name=f"qkt{p}")
                   for p in range(NPAIR)]
        vp_sb = persist.tile([P, H, NKT, 65], bf16)   # [V_h | 1] stationary
        nc.any.memset(vp_sb[:, :, :, 64:65], 1.0)
        if ev8:
            # fp8 copy of [V_h | 1] with k-tile pairs interleaved along
            # the free axis for DoubleRow E@v (ko stride 80: %16 rule)
            vp8_sb = persist.tile([P, H, NKT // 2, 2, 80], f8e4)
            nc.any.memset(vp8_sb[:, :, :, :, 64:65], 1.0)
        pw_sb = persist.tile([P, NCIN, C], bf16)      # proj weights
        attn_sb = attn_pool.tile([P, NCIN, N], bf16)  # attention out^T

        if epi_mode == "gps":
            from concourse import library_config
            nc.gpsimd.load_library(library_config.attn)

        with tc.tile_pool(name="ph1", bufs=1) as ph1:
            xT_sb = ph1.tile([P, NCIN, N], bf16)
            vw_sb = ph1.tile([P, NCIN, C], bf16)
            # direct DMA loads (host pre-casts).  Emission order on
            # the sync queue == descriptor order per DMA queue, so x/vw
            # stream first, then qkw; pw + at tiles are emitted later.
            xT_r = xT_ext.rearrange("(c p) t -> p c t", p=P)
            vw_r = vwT_ext.rearrange("(c p) n -> p c n", p=P)
            pw_r = pwT_ext.rearrange("(c p) n -> p c n", p=P)
            for kc in range(NCIN):
                nc.sync.dma_start(xT_sb[:, kc, :], xT_r[:, kc, :])
                nc.sync.dma_start(vw_sb[:, kc, :], vw_r[:, kc, :])
                if kc == 0:
                    nc.sync.dma_start(vb_sb[:], vb_ext[:])
                    nc.sync.dma_start(qkb_sb[:], qkb_ext[:])
                    nc.sync.dma_start(pb_sb[:], pb_ext[:])
            if qk8:
                x8_sb = ph1.tile([P, NCIN // 2, 2, N], f8e4)
                qkw8_sb = ph1.tile([P, NCIN // 2, 2, 2 * C], f8e4)
                for g in range(NCIN // 2):
                    nc.sync.dma_start(x8_sb[:, g, :, :], x8_ext[:, g, :, :])
                    nc.sync.dma_start(qkw8_sb[:, g, :, :],
                                      qkw8_ext[:, g, :, :])
            else:
                qkw_sb = ph1.tile([P, NCIN, 2 * C], bf16)
                qkw_r = qkwT_ext.rearrange("(c p) n -> p c n", p=P)
                for kc in range(NCIN):
                    nc.sync.dma_start(qkw_sb[:, kc, :], qkw_r[:, kc, :])

            # ---- HAM warm-up + V projection.  Const matmuls keep the PE
            # busy while the first x/vw chunks stream in, so the clock
            # gate opens to 8/8 (~3.4us sustained) before real work; a
            # few more const matmuls pad the DMA-paced first group so
            # the busy window stays unbroken. ----
            with tc.tile_pool(name="pp_w", bufs=1, space="PSUM") as pp_w, \
                 tc.tile_pool(name="pp_v", bufs=3, space="PSUM") as pp_v:
                wps = pp_w.tile([64, 512], f32)

                def warm_mm(n):
                    nc.tensor.matmul(
                        wps[:, 0:n], ww_sb[:, 0:64], wr_sb[:, 0:n],
                        start=True, stop=True, skip_group_check=True)

                for _ in range(nwarm):
                    warm_mm(256)

                # groups of 2 tiles over a 3-deep psum rotation: the
                # first tile of each group never waits on the previous
                # group's vp copies, so the seams pipeline away
                for gi, tts in enumerate(((0, 1), (2, 3), (4, 5), (6, 7))):
                    pss = {tt: pp_v.tile([P, C], f32, tag="v",
                                         name=f"vps{tt}") for tt in tts}
                    for kc in range(NCIN):
                        for tt in tts:
                            for (n0, nw) in ((0, QW), (QW, C - QW)):
                                nc.tensor.matmul(
                                    pss[tt][:, n0:n0 + nw],
                                    xT_sb[:, kc, tt * P:(tt + 1) * P],
                                    vw_sb[:, kc, n0:n0 + nw],
                                    start=(kc == 0), stop=False,
                                    skip_group_check=True)
                        if gi <= 1 and kc < 5:
                            # absorb per-chunk DMA lateness so the HAM
                            # busy window is not broken
                            warm_mm(256)
                            warm_mm(256)
                    for tt in tts:
                        for (n0, nw) in ((0, QW), (QW, C - QW)):
                            nc.tensor.matmul(
                                pss[tt][:, n0:n0 + nw],
                                ones_sb[0:1, 0:P],
                                vb_sb[0:1, n0:n0 + nw],
                                start=False, stop=True,
                                skip_group_check=True)
                        nc.vector.tensor_copy(
                            vp_sb[:, :, tt, 0:64],
                            pss[tt].rearrange("p (h d) -> p h d", d=64))
                        if ev8:
                            # on ACT (idle in phase 1): the psum-pool
                            # rotation waits on these copies, and two
                            # serial DVE casts per tile stall the PE at
                            # every V-group seam
                            nc.scalar.copy(
                                vp8_sb[:, :, tt // 2, tt % 2, 0:64],
                                pss[tt].rearrange("p (h d) -> p h d", d=64))

            # ---- attention (+ interleaved qkT / proj work) ----
            with tc.tile_pool(name="pp_st", bufs=2, space="PSUM") as pp_st, \
                 tc.tile_pool(name="pp_ev", bufs=2, space="PSUM") as pp_ev, \
                     tc.tile_pool(name="pp_av", bufs=2, space="PSUM") as pp_av:

                def qkt_ct(ct, pr_dst, qki):
                    """One qkT output tile (6 or 3 matmuls) + bias-add."""
                    ps = pp_st.tile([P, N], f32, tag="st", name=f"qk{ct}")
                    if qk8:
                        for g in range(NCIN // 2):
                            for qh in range(NQT):
                                nc.tensor.matmul(
                                    ps[:, qh * QW:(qh + 1) * QW],
                                    qkw8_sb[:, g, :, ct * P:(ct + 1) * P],
                                    x8_sb[:, g, :, qh * QW:(qh + 1) * QW],
                                    start=(g == 0),
                                    stop=(g == NCIN // 2 - 1),
                                    perf_mode=mybir.MatmulPerfMode.DoubleRow,
                                    skip_group_check=True)
                    else:
                        for kc in range(NCIN):
                            for qh in range(NQT):
                                nc.tensor.matmul(
                                    ps[:, qh * QW:(qh + 1) * QW],
                                    qkw_sb[:, kc, ct * P:(ct + 1) * P],
                                    xT_sb[:, kc, qh * QW:(qh + 1) * QW],
                                    start=(kc == 0), stop=(kc == NCIN - 1),
                                    skip_group_check=True)
                    nc.vector.tensor_scalar_add(
                        qkT_prs[pr_dst][:, qki, :], ps[:, :],
                        qkb_sb[:, ct:ct + 1])

                def qkt_group(pr):
                    qkt_ct(pr, pr, 0)
                    qkt_ct(NPAIR + pr, pr, 1)

                def warm_into(ps, n=QW):
                    # full-K const matmul into a psum region whose next
                    # real matmul is start=True (overwrites the garbage):
                    # keeps the PE busy so the HAM clock stays at 8/8
                    nc.tensor.matmul(
                        ps[0:64, 0:n], ww_sb[:, 0:64],
                        wr_sb[:, 0:n], start=True, stop=True,
                        skip_group_check=True)

                # dummy av-tag tile: a scratch psum bank for padding the
                # qkT-upfront stretch (paced by the qkw DMA stream)
                avw = pp_av.tile([P, QW], f32, tag="av", name="phasewarm")
                for ct_i, (ct, prd, qki) in enumerate(
                        ((0, 0, 0), (NPAIR, 0, 1), (1, 1, 0),
                         (NPAIR + 1, 1, 1))):
                    qkt_ct(ct, prd, qki)
                    for _ in range(5):
                        warm_into(avw)

                # proj weights stream after x/qkw/vw, before the at tiles
                for kc in range(NCIN):
                    nc.sync.dma_start(pw_sb[:, kc, :], pw_r[:, kc, :])

                # with fp8 qkT, qkT_prs holds 32*(q+b) per side -> the
                # score psum carries an extra 1024x, folded into the exp
                exp_scale = SCALE / 1024.0 if qk8 else SCALE

                def emit_st_step(pr, qt, e_sb, kt):
                    q0 = qt * QW
                    st = pp_st.tile([P, 2 * QW], f32, tag="st",
                                    name=f"st{pr}_{qt}_{kt}")
                    k0 = kt * P
                    nc.tensor.matmul(
                        st[:, 0:QW],
                        qkT_prs[pr][0:64, 1, k0:k0 + P],
                        qkT_prs[pr][0:64, 0, q0:q0 + QW],
                        start=True, stop=True)
                    nc.tensor.matmul(
                        st[:, QW:2 * QW],
                        qkT_prs[pr][64:128, 1, k0:k0 + P],
                        qkT_prs[pr][64:128, 0, q0:q0 + QW],
                        start=True, stop=True)
                    nc.scalar.activation(
                        e_sb[:, kt, :], st[:, :],
                        mybir.ActivationFunctionType.Exp, scale=exp_scale)

                def emit_out_step(item, kt):
                    pr, qt, e_sb, psE1, psE2, psA = item
                    h1, h2 = 2 * pr, 2 * pr + 1
                    at = atbf.tile([P, 2 * QW], bf16, tag="atb",
                                   name=f"atb{pr}_{qt}_{kt}")
                    nc.sync.dma_start(at[:], at_ext[pr, qt, kt])
                    st_flags = dict(start=(kt == 0), stop=(kt == NKT - 1),
                                    skip_group_check=True)
                    nc.tensor.matmul(
                        psA[0:64, :], vp_sb[:, h1, kt, 0:64],
                        at[:, 0:QW], **st_flags)
                    nc.tensor.matmul(
                        psA[64:128, :], vp_sb[:, h2, kt, 0:64],
                        at[:, QW:2 * QW], **st_flags)
                    if ev8:
                        # DoubleRow fp8: one matmul covers a k-tile pair
                        if kt % 2 == 1:
                            g = kt // 2
                            dr_flags = dict(
                                start=(g == 0), stop=(g == NKT // 2 - 1),
                                perf_mode=mybir.MatmulPerfMode.DoubleRow,
                                skip_group_check=True)
                            nc.tensor.matmul(
                                psE1[0:65, :],
                                vp8_sb[:, h1, g, :, 0:65],
                                e_sb[:, 2 * g:2 * g + 2, 0:QW], **dr_flags)
                            nc.tensor.matmul(
                                psE2[0:65, :],
                                vp8_sb[:, h2, g, :, 0:65],
                                e_sb[:, 2 * g:2 * g + 2, QW:2 * QW],
                                **dr_flags)
                    else:
                        nc.tensor.matmul(
                            psE1[0:65, :], vp_sb[:, h1, kt, :],
                            e_sb[:, kt, 0:QW], **st_flags)
                        nc.tensor.matmul(
                            psE2[0:65, :], vp_sb[:, h2, kt, :],
                            e_sb[:, kt, QW:2 * QW], **st_flags)

                recip_mode = os.environ.get("ATTN_RECIP", "dve")

                def emit_epilogue_recip(item, force_act=False):
                    # 1/rowsum; runs while the next block's score
                    # matmuls keep the PE busy
                    pr, qt, e_sb, psE1, psE2, psA = item
                    rs = []
                    for hi, psE in ((0, psE1), (1, psE2)):
                        if recip_mode == "act" or force_act:
                            lns = small.tile([1, QW], f32, tag="lns",
                                             name=f"ln{pr}_{qt}_{hi}")
                            nc.scalar.activation(
                                lns[:], psE[64:65, :],
                                mybir.ActivationFunctionType.Ln)
                            r16 = small.tile([1, QW], bf16, tag="r16",
                                             name=f"r16_{pr}_{qt}_{hi}")
                            nc.scalar.activation(
                                r16[:], lns[:],
                                mybir.ActivationFunctionType.Exp,
                                scale=-1.0)
                            rs.append(r16)
                            continue
                        # copy the rowsum to a partition-0 SBUF tile first:
                        # the custom-DVE recip mis-reads a partition-64
                        # PSUM operand (standard ops handle it fine)
                        rsum = small.tile([1, QW], f32, tag="rsum",
                                          name=f"rs_{pr}_{qt}_{hi}")
                        nc.vector.tensor_copy(rsum[:], psE[64:65, :])
                        r32 = small.tile([1, QW], f32, tag="r32",
                                         name=f"r32_{pr}_{qt}_{hi}")
                        nc.vector.reciprocal_approx_fast(r32[:], rsum[:])
                        if epi_mode == "gps":
                            rs.append(r32)
                        else:
                            r16 = small.tile([1, QW], bf16, tag="r16",
                                             name=f"r16_{pr}_{qt}_{hi}")
                            nc.vector.tensor_copy(r16[:], r32[:])
                            rs.append(r16)
                    return rs

                def emit_epilogue_apply(item, rs):
                    pr, qt, e_sb, psE1, psE2, psA = item
                    q0 = qt * QW
                    for hi, psE in ((0, psE1), (1, psE2)):
                        pa, pz = hi * 64, hi * 64 + 64
                        dst = attn_sb[pa:pz, pr, q0:q0 + QW]
                        if epi_mode == "gps":
                            rb = small.tile([64, QW], f32, tag="rb",
                                            name=f"rb{pr}_{qt}_{hi}")
                            nc.gpsimd.partition_broadcast(
                                rb[:], rs[hi][:], channels=64)
                            nc.vector.tensor_mul(dst, psE[0:64, :], rb[:])
                        else:
                            nc.tensor.matmul(psE[64:128, :],
                                             ones64_sb[0:1, :],
                                             rs[hi][:, :], start=True,
                                             stop=True,
                                             skip_group_check=True)
                            rb = small.tile([64, QW], f32, tag="rb",
                                            name=f"rb{pr}_{qt}_{hi}")
                            nc.vector.tensor_copy(rb[:], psE[64:128, :])
                            nc.vector.tensor_mul(dst, psE[0:64, :], rb[:])
                        nc.vector.tensor_add(dst, dst, psA[pa:pz, :])

                # qkT filler bursts inside the item loops: pairs 2..5,
                # each ct one-to-two items before its first use.  Item
                # (0,0) has no out-step work (pipeline fill), so it gets
                # two bursts.
                filler = {
                    (0, 0): [(2, 2, 0), (NPAIR + 2, 2, 1)],
                    (0, 1): [(3, 3, 0)],
                    (1, 0): [(NPAIR + 3, 3, 1)],
                    (1, 1): [(4, 4, 0)],
                    (2, 0): [(NPAIR + 4, 4, 1)],
                    (2, 1): [(5, 5, 0)],
                    (3, 0): [(NPAIR + 5, 5, 1)],
                }

                # software-pipelined emission: item i's ST/exp stream is
                # interleaved kt-by-kt with item i-1's E@v/A@v matmuls, so
                # the PE has dense work while ACT drains the score tiles
                items = [(pr, qt) for pr in range(NPAIR)
                         for qt in range(NQT)]
                prev = None        # item whose OUT runs in the current block
                pend = None        # pe-mode: (item, rs) awaiting PE/DVE apply
                e_dt = f8e4 if ev8 else bf16
                for pr, qt in items:
                    e_sb = epool.tile([P, NKT, 2 * QW], e_dt, tag="e",
                                      name=f"e{pr}_{qt}")
                    # two score steps up front cover the pending
                    # epilogue's DVE reciprocal latency
                    emit_st_step(pr, qt, e_sb, 0)
                    emit_st_step(pr, qt, e_sb, 1)
                    if pend is not None:
                        emit_epilogue_apply(*pend)
                        pend = None
                    psE1 = pp_ev.tile([P, QW], f32, tag="ev",
                                      name=f"ev1_{pr}_{qt}")
                    psE2 = pp_ev.tile([P, QW], f32, tag="ev",
                                      name=f"ev2_{pr}_{qt}")
                    psA = pp_av.tile([P, QW], f32, tag="av",
                                     name=f"av{pr}_{qt}")
                    cur = (pr, qt, e_sb, psE1, psE2, psA)
                    fill = list(filler.get((pr, qt), ()))
                    for kt in range(NKT):
                        if kt + 2 < NKT:
                            emit_st_step(pr, qt, e_sb, kt + 2)
                        if prev is not None:
                            emit_out_step(prev, kt)
                        if fill and kt in (2, 4):
                            qkt_ct(*fill.pop(0))
                    # item-boundary padding: the next item's first score
                    # steps wait on this item's last exps (ACT is the
                    # pacer once E@v runs DoubleRow); garbage written here
                    # is overwritten by this item's start=True A@v in the
                    # next block
                    i_next = items.index((pr, qt)) + 1
                    if i_next < len(items):
                        nwm = 2 if filler.get(items[i_next]) else itemwarm
                        for _ in range(nwm):
                            warm_into(psA)
                    if prev is not None:
                        rs = emit_epilogue_recip(prev)
                        if epi_mode == "gps":
                            emit_epilogue_apply(prev, rs)
                        else:
                            pend = (prev, rs)
                    prev = cur

                # ---- drain: last item's outs with proj partial bursts
                # interleaved, then the final epilogue and the output
                # projection ----
                def proj_partial(ps, ct, kcs, start, stop):
                    for kc in kcs:
                        for qh in range(NQT):
                            nc.tensor.matmul(
                                ps[:, qh * QW:(qh + 1) * QW],
                                pw_sb[:, kc, ct * P:(ct + 1) * P],
                                attn_sb[:, kc, qh * QW:(qh + 1) * QW],
                                start=(start and kc == kcs[0]),
                                stop=(stop and kc == kcs[-1]),
                                skip_group_check=True)

                with tc.tile_pool(name="ph3o", bufs=2) as ph3o:
                    out_r = out_ext.rearrange("(c p) t -> p c t", p=P)
                    pjs = {}

                    def proj_finish(ct, kcs, start):
                        ps = pjs[ct]
                        proj_partial(ps, ct, kcs, start, True)
                        o_sb = ph3o.tile([P, N], bf16, tag="o",
                                         name=f"o{ct}")
                        # alternate the bias-add between ACT and DVE so
                        # the six tail bias-adds run two-wide
                        if ct % 2 == 0:
                            nc.scalar.activation(
                                o_sb[:], ps[:],
                                mybir.ActivationFunctionType.Identity,
                                bias=pb_sb[:, ct:ct + 1])
                        else:
                            nc.vector.tensor_scalar_add(
                                o_sb[:], ps[:], pb_sb[:, ct:ct + 1])
                        nc.sync.dma_start(out_r[:, ct, :], o_sb[:])

                    for kt in range(NKT):
                        emit_out_step(prev, kt)
                        if kt == 0 and pend is not None:
                            emit_epilogue_apply(*pend)
                            pend = None
                        # the proj bursts recycle the st-score psum bufs;
                        # placed where the bufs actually free (exp kt6/kt7)
                        # so they don't stall the out-step stream
                        if kt == 5:
                            pjs[0] = pp_st.tile([P, N], f32, tag="st",
                                                name="proj0")
                            for _ in range(3):
                                warm_into(pjs[0])
                            proj_partial(pjs[0], 0, list(range(NCIN - 1)),
                                         True, False)
                        if kt == 6:
                            pjs[1] = pp_st.tile([P, N], f32, tag="st",
                                                name="proj1")
                            for _ in range(3):
                                warm_into(pjs[1])
                            proj_partial(pjs[1], 1, list(range(NCIN - 1)),
                                         True, False)
                    # const matmuls keep the PE busy (and the HAM clock
                    # warm) while the final epilogue chain runs on
                    # ACT/DVE; the proj finishes wait on it anyway
                    if tailwarm:
                        fav = pp_av.tile([P, QW], f32, tag="av",
                                         name="tailwarm")
                        for _ in range(tailwarm):
                            nc.tensor.matmul(
                                fav[0:64, :], ww_sb[:, 0:64],
                                wr_sb[:, :], start=True, stop=True,
                                skip_group_check=True)
                    # final epilogue: ACT recip chain — ACT is idle at
                    # the tail and the DVE chain (serial ~8us) was the
                    # dominant drain stall
                    rs = emit_epilogue_recip(prev, force_act=True)
                    emit_epilogue_apply(prev, rs)
                    proj_finish(0, [NCIN - 1], False)
                    proj_finish(1, [NCIN - 1], False)
                    for ct in range(2, NCIN):
                        pjs[ct] = pp_st.tile([P, N], f32, tag="st",
                                             name=f"proj{ct}")
                        proj_finish(ct, list(range(NCIN)), True)

    if os.environ.get("ATTN_DEDUP_LDW", "1") == "1":
        _dedup_ldweights(nc)
    if os.environ.get("ATTN_SPLIT_WAITS", "1") == "1":
        _split_excess_waits(nc)
    if not nc.is_finalized():
        nc.finalize()   # Bacc: move_matmul_waits + generate_event_semaphores
    return nc


def make_in_maps(x, qkv_w, qkv_b, static_a, proj_w, proj_b):
    """Host-side sharding / layout prep. One batch element per core."""
    x = np.asarray(x, dtype=np.float32)
    qkv_w = np.asarray(qkv_w, dtype=np.float32)
    qkv_b = np.asarray(qkv_b, dtype=np.float32)
    static_a = np.asarray(static_a, dtype=np.float32)
    proj_w = np.asarray(proj_w, dtype=np.float32)
    proj_b = np.asarray(proj_b, dtype=np.float32)

    import ml_dtypes
    bf16 = ml_dtypes.bfloat16
    f8 = ml_dtypes.float8_e4m3fn
    qk8 = os.environ.get("ATTN_QK8", "1") == "1"

    qkb_scale = 32.0 if qk8 else 1.0
    qkb = np.ascontiguousarray(
        qkv_b[0:2 * C].reshape(2 * C // P, P).T
        * qkb_scale).astype(np.float32)
    vwT = np.ascontiguousarray(qkv_w[2 * C:3 * C].T).astype(bf16)
    vb = np.ascontiguousarray(
        qkv_b[2 * C:3 * C].reshape(1, C)).astype(bf16)
    # A^T strips, contiguous per (pair, qtile, ktile): [6, 2, 8, 128, 1024]
    # at[pr, qt, kt, :, 0:512] = A^T[2pr][kt tile, qt tile], [..., 512:] = head 2pr+1
    atT = static_a[0].transpose(0, 2, 1)                      # [H, k, q]
    at = np.ascontiguousarray(
        atT.reshape(NPAIR, 2, NKT, P, NQT, QW).transpose(0, 4, 2, 3, 1, 5)
        .reshape(NPAIR, NQT, NKT, P, 2 * QW)).astype(bf16)
    pwT = np.ascontiguousarray(proj_w.T).astype(bf16)
    pb = np.ascontiguousarray(
        proj_b.reshape(C // P, P).T).astype(np.float32)

    shared = {"qkb": qkb, "vwT": vwT, "vb": vb,
              "at": at, "pwT": pwT, "pb": pb}
    if qk8:
        qkw32 = qkv_w[0:2 * C].T * 32.0                       # [768, 1536]
        shared["qkw8"] = np.ascontiguousarray(
            qkw32.reshape(NCIN // 2, 2, P, 2 * C)
            .transpose(2, 0, 1, 3)).astype(f8)
    else:
        shared["qkwT"] = np.ascontiguousarray(
            qkv_w[0:2 * C].T).astype(bf16)
    in_maps = []
    for b in range(B):
        m = dict(shared)
        xT = x[b].T
        m["xT"] = np.ascontiguousarray(xT).astype(bf16)
        if qk8:
            m["x8"] = np.ascontiguousarray(
                xT.reshape(NCIN // 2, 2, P, N)
                .transpose(2, 0, 1, 3)).astype(f8)
        in_maps.append(m)
    return in_maps


_NC_CACHE = {}


def _get_nc():
    if "nc" not in _NC_CACHE:
        _NC_CACHE["nc"] = build_nc()
    return _NC_CACHE["nc"]


def kernel(x, qkv_w, qkv_b, static_a, proj_w, proj_b):
    _ensure_paths()
    from concourse.bass_utils import run_bass_kernel_spmd

    nc = _get_nc()
    in_maps = make_in_maps(x, qkv_w, qkv_b, static_a, proj_w, proj_b)
    res = run_bass_kernel_spmd(nc, in_maps, core_ids=list(range(NCORES)))
    out = np.empty((B, N, C), dtype=np.float32)
    for b in range(B):
        out[b] = np.asarray(res.results[b]["out"], dtype=np.float32).T
    return out


# revision 41
# speedup vs baseline: 1.1295x; 1.0144x over previous
"""Trainium2 Bass kernel for nn_Attention_72438918414857.

Reference computation (B=8, N=1024, C=768, H=12, D=64):
    qkv = (x @ qkv_w.T + qkv_b) -> q, k, v per head
    attn = softmax(q @ k.T / sqrt(D)) + static_a   (bias added AFTER softmax)
    out = (attn @ v) merged-heads @ proj_w.T + proj_b

Sharding: data-parallel over batch -- one batch element per NeuronCore,
weights + static_a replicated. No collectives needed.

Math used on-chip (per batch, per head), everything transposed so each
matmul gets its contraction dim on partitions with no on-chip transposes:
    qkT = [Wq;Wk]^T-proj of x  ->  [cout, t] layout
    E^T = exp(K_h^T.T @ Q_h^T * D^-0.5)           [k, q] strips
    out_h^T = ([V_h|1].T @ E^T) -> rows 0..63 = E@v, row 64 = rowsum(E)
    attn_h^T = (E@v) * (1/rowsum) + V_h.T @ A_h^T
where static_a is pre-transposed on host to A^T[h, k, q].  The softmax
normalization is applied to the [64, q] output instead of the [k, q]
matrix; no max-subtraction is needed (|scores*scale| < ~3).

Matmuls run in bf16 (fp32 PE matmul is 4x slower); PSUM accumulation is
fp32.  bf16 rounding of operands keeps rel-err ~4e-3, well under the
2e-2 gate.

v2 scheduling changes (over the first working version):
  - warm-up matmuls on const data at t=0 so the PE HAM clock-gate
    reaches 8/8 before the real work starts (first ~20us of v1 ran at
    1.2 GHz)
  - V projection runs kc-outer in two tt-quads so the first x/vw DMA
    chunk immediately yields dense PE work
  - qkT projections for pairs 2..5 are emitted as per-item filler
    bursts inside the attention loop, filling the exp-gated PE bubbles
    at item boundaries
  - softmax reciprocal on DVE (reciprocal_approx_fast) instead of the
    ACT Ln/Exp chain (saves ~33us of ACT time; ACT runs only the big
    exps)
  - drain: proj partial bursts interleaved into the last item's
    out-steps; output stored/DMA'd as bf16
"""

import os
import sys

import numpy as np

B, N, C = 8, 1024, 768
H, D = 12, 64
NCORES = 8
P = 128
QW = 512          # q tile width (PSUM bank = 512 f32)
NQT = N // QW     # 2 q tiles
NKT = N // P      # 8 k tiles
NCIN = C // P     # 6 c_in chunks
NPAIR = H // 2    # 6 head pairs
SCALE = float(D) ** -0.5

_REPO = "/opt/trn_rl_repo"


def _ensure_paths():
    if _REPO not in sys.path:
        sys.path.insert(0, _REPO)


def _dedup_ldweights(nc):
    """Delete an Ldweights whose weights AP + tile geometry match the
    immediately preceding Ldweights on the PE stream (the weights are
    still resident in the array); its waits/updates move to the next
    instruction."""
    import concourse.mybir as mybir

    def sig(inst):
        ap = inst.ins[0]
        return (str(ap), str(getattr(inst, "tile_position", None)),
                str(getattr(inst, "tile_size", None)))

    for fn in nc.m.functions:
        for blk in fn.blocks:
            out = []
            last_sig = None
            pend_w, pend_u = [], []
            changed = False
            for inst in blk.instructions:
                op = str(inst.opcode)
                if op == "Ldweights":
                    s_ = sig(inst)
                    if s_ == last_sig:
                        si = inst.sync_info
                        if si:
                            pend_w.extend(si.on_wait or [])
                            pend_u.extend(si.on_update or [])
                        changed = True
                        continue
                    last_sig = s_
                elif op == "Matmult":
                    pass          # matmuls don't disturb loaded weights
                elif op in ("NoOp", "EventSemaphore"):
                    pass
                else:
                    last_sig = None
                if pend_w or pend_u:
                    si = inst.sync_info
                    ow = list(si.on_wait or []) if si else []
                    ou = list(si.on_update or []) if si else []
                    inst.sync_info = mybir.SyncInfo(
                        on_wait=pend_w + ow, on_update=pend_u + ou)
                    pend_w, pend_u = [], []
                out.append(inst)
            assert not pend_w and not pend_u
            if changed:
                blk.instructions = out


def _split_excess_waits(nc):
    """The TRN2 walrus codegen allows only 1 sem-wait command per
    instruction.  Tile's sem-assigner can emit more (one per logical
    proc a tile depends on).
    Move the excess onto freshly inserted same-engine NoOps placed just
    before the instruction -- engines execute in order, so waiting on a
    preceding NoOp is equivalent."""
    import concourse.mybir as mybir
    from bass_rust import InstNoOp

    nid = [0]
    for fn in nc.m.functions:
        for blk in fn.blocks:
            out = []
            changed = False
            for inst in blk.instructions:
                si = inst.sync_info
                waits = list(si.on_wait) if si and si.on_wait else []
                limit = 1
                if len(waits) > limit:
                    extra, keep = waits[:-limit], waits[-limit:]
                    inst.sync_info = si.__replace__(on_wait=keep)
                    for w in extra:
                        nop = InstNoOp(
                            name=f"{inst.name}-wsplit{nid[0]}", ins=[], outs=[])
                        nid[0] += 1
                        nop.engine = inst.engine
                        nop.sync_info = mybir.SyncInfo(
                            on_wait=[w], on_update=[])
                        out.append(nop)
                    changed = True
                out.append(inst)
            if changed:
                blk.instructions = out


def _patch_act_tables():
    """Force Bacc's activation-table chooser to the single set that
    contains every function this kernel uses (exp, identity, copy),
    so only one ACT_TABLE_LOAD (~2.7us each) is emitted."""
    import concourse.hw_specs as hw_specs
    import concourse.mybir as mybir
    if getattr(hw_specs.get_activation_tables, "_attn_patched", False):
        return
    orig = hw_specs.get_activation_tables
    keep = {mybir.ActivationFunctionType.Exp, mybir.ActivationFunctionType.Ln,
            mybir.ActivationFunctionType.Identity,
            mybir.ActivationFunctionType.Copy}

    import functools

    @functools.cache
    def patched(module_arch):
        tables = dict(orig(module_arch))
        out = {}
        for name, fns in tables.items():
            if name == "natural_log_exp_and_others":
                out[name] = fns
            else:
                out[name] = fns - keep
        return out

    patched._attn_patched = True
    hw_specs.get_activation_tables = patched
    import concourse.bacc as bacc_mod
    bacc_mod.get_activation_tables = patched


def build_nc():
    """Build the per-core Bass/Tile program."""
    _ensure_paths()
    _patch_act_tables()
    import concourse.bass as bass
    import concourse.mybir as mybir
    import concourse.tile as tile
    from concourse import bacc
    from contextlib import ExitStack

    f32 = mybir.dt.float32
    bf16 = mybir.dt.bfloat16
    f8e4 = mybir.dt.float8e4

    epi_mode = os.environ.get("ATTN_EPI", "pe")   # 'pe' | 'gps'
    nwarm = int(os.environ.get("ATTN_WARM", "16"))
    tailwarm = int(os.environ.get("ATTN_TAILWARM", "16"))
    itemwarm = int(os.environ.get("ATTN_ITEMWARM", "4"))
    at_bufs = int(os.environ.get("ATTN_AT_BUFS", "12"))
    ev8 = os.environ.get("ATTN_EV8", "1") == "1"   # DoubleRow fp8 E@v
    qk8 = os.environ.get("ATTN_QK8", "1") == "1"   # DoubleRow fp8 qkT proj

    nc = bacc.Bacc("TRN2", target_bir_lowering=False, debug=False,
                   num_devices=NCORES)

    xT_ext = nc.declare_dram_parameter("xT", [C, N], bf16, isOutput=False)
    if qk8:
        # cin-pair interleaved fp8 copies for DoubleRow (weights scaled
        # by 32 on host so w~N(0,0.02) lands in e4m3's normal range;
        # the extra 32*32 falls out of the exp scale)
        x8_ext = nc.declare_dram_parameter(
            "x8", [P, NCIN // 2, 2, N], f8e4, isOutput=False)
        qkw8_ext = nc.declare_dram_parameter(
            "qkw8", [P, NCIN // 2, 2, 2 * C], f8e4, isOutput=False)
    else:
        qkwT_ext = nc.declare_dram_parameter(
            "qkwT", [C, 2 * C], bf16, isOutput=False)
    qkb_ext = nc.declare_dram_parameter("qkb", [P, 2 * C // P], f32, isOutput=False)
    vwT_ext = nc.declare_dram_parameter("vwT", [C, C], bf16, isOutput=False)
    vb_ext = nc.declare_dram_parameter("vb", [1, C], bf16, isOutput=False)
    at_ext = nc.declare_dram_parameter(
        "at", [NPAIR, NQT, NKT, P, 2 * QW], bf16, isOutput=False)
    pwT_ext = nc.declare_dram_parameter("pwT", [C, C], bf16, isOutput=False)
    pb_ext = nc.declare_dram_parameter("pb", [P, C // P], f32, isOutput=False)
    out_ext = nc.declare_dram_parameter("out", [C, N], bf16, isOutput=True)

    with tile.TileContext(nc, num_cores=NCORES) as tc, ExitStack() as ctx:
        consts = ctx.enter_context(tc.tile_pool(name="consts", bufs=1))
        persist = ctx.enter_context(tc.tile_pool(name="persist", bufs=1))
        attn_pool = ctx.enter_context(tc.tile_pool(name="attnout", bufs=1))
        epool = ctx.enter_context(tc.tile_pool(name="epool", bufs=2))
        atbf = ctx.enter_context(tc.tile_pool(name="atbf", bufs=at_bufs))
        small = ctx.enter_context(tc.tile_pool(name="small", bufs=2))

        qkb_sb = consts.tile([P, 2 * C // P], f32)
        pb_sb = consts.tile([P, NCIN], f32)
        vb_sb = consts.tile([1, C], bf16)
        # memsets for the warmup constants go on DVE: the gpsimd queue is
        # busy with the Tile prologue (sem clears) for the first ~3us and
        # would delay the HAM warmup matmuls
        ones_sb = consts.tile([1, P], bf16)
        nc.vector.memset(ones_sb[:], 1.0)
        ones64_sb = consts.tile([1, 64], bf16)
        nc.vector.memset(ones64_sb[:], 1.0)
        # warm matmuls must engage the full 128-row array: the HAM
        # activity monitor does not register K=1 matmuls as PE-busy
        ww_sb = consts.tile([P, 64], bf16)
        nc.vector.memset(ww_sb[:], 0.0)
        wr_sb = consts.tile([P, 512], bf16)
        nc.vector.memset(wr_sb[:], 0.0)

        # persistent activations (bf16 matmul operands)
        qkT_prs = [persist.tile([P, 2, N], bf16, tag=f"qkt{p}",
                                name=f"qkt{p}")
                   for p in range(NPAIR)]
        vp_sb = persist.tile([P, H, NKT, 65], bf16)   # [V_h | 1] stationary
        nc.any.memset(vp_sb[:, :, :, 64:65], 1.0)
        if ev8:
            # fp8 copy of [V_h | 1] with k-tile pairs interleaved along
            # the free axis for DoubleRow E@v (ko stride 80: %16 rule)
            vp8_sb = persist.tile([P, H, NKT // 2, 2, 80], f8e4)
            nc.any.memset(vp8_sb[:, :, :, :, 64:65], 1.0)
        pw_sb = persist.tile([P, NCIN, C], bf16)      # proj weights
        attn_sb = attn_pool.tile([P, NCIN, N], bf16)  # attention out^T

        if epi_mode == "gps":
            from concourse import library_config
            nc.gpsimd.load_library(library_config.attn)

        with tc.tile_pool(name="ph1", bufs=1) as ph1:
            xT_sb = ph1.tile([P, NCIN, N], bf16)
            vw_sb = ph1.tile([P, NCIN, C], bf16)
            # direct DMA loads (host pre-casts).  Emission order on
            # the sync queue == descriptor order per DMA queue, so x/vw
            # stream first, then qkw; pw + at tiles are emitted later.
            xT_r = xT_ext.rearrange("(c p) t -> p c t", p=P)
            vw_r = vwT_ext.rearrange("(c p) n -> p c n", p=P)
            pw_r = pwT_ext.rearrange("(c p) n -> p c n", p=P)
            for kc in range(NCIN):
                nc.sync.dma_start(xT_sb[:, kc, :], xT_r[:, kc, :])
                nc.sync.dma_start(vw_sb[:, kc, :], vw_r[:, kc, :])
                if kc == 0:
                    nc.sync.dma_start(vb_sb[:], vb_ext[:])
                    nc.sync.dma_start(qkb_sb[:], qkb_ext[:])
                    nc.sync.dma_start(pb_sb[:], pb_ext[:])
            if qk8:
                x8_sb = ph1.tile([P, NCIN // 2, 2, N], f8e4)
                qkw8_sb = ph1.tile([P, NCIN // 2, 2, 2 * C], f8e4)
                for g in range(NCIN // 2):
                    nc.sync.dma_start(x8_sb[:, g, :, :], x8_ext[:, g, :, :])
                    nc.sync.dma_start(qkw8_sb[:, g, :, :],
                                      qkw8_ext[:, g, :, :])
            else:
                qkw_sb = ph1.tile([P, NCIN, 2 * C], bf16)
                qkw_r = qkwT_ext.rearrange("(c p) n -> p c n", p=P)
                for kc in range(NCIN):
                    nc.sync.dma_start(qkw_sb[:, kc, :], qkw_r[:, kc, :])

            # ---- HAM warm-up + V projection.  Const matmuls keep the PE
            # busy while the first x/vw chunks stream in, so the clock
            # gate opens to 8/8 (~3.4us sustained) before real work; a
            # few more const matmuls pad the DMA-paced first group so
            # the busy window stays unbroken. ----
            with tc.tile_pool(name="pp_w", bufs=1, space="PSUM") as pp_w, \
                 tc.tile_pool(name="pp_v", bufs=3, space="PSUM") as pp_v:
                wps = pp_w.tile([64, 512], f32)

                def warm_mm(n):
                    nc.tensor.matmul(
                        wps[:, 0:n], ww_sb[:, 0:64], wr_sb[:, 0:n],
                        start=True, stop=True, skip_group_check=True)

                for _ in range(nwarm):
                    warm_mm(256)

                # groups of 2 tiles over a 3-deep psum rotation: the
                # first tile of each group never waits on the previous
                # group's vp copies, so the seams pipeline away
                for gi, tts in enumerate(((0, 1), (2, 3), (4, 5), (6, 7))):
                    pss = {tt: pp_v.tile([P, C], f32, tag="v",
                                         name=f"vps{tt}") for tt in tts}
                    for kc in range(NCIN):
                        for tt in tts:
                            for (n0, nw) in ((0, QW), (QW, C - QW)):
                                nc.tensor.matmul(
                                    pss[tt][:, n0:n0 + nw],
                                    xT_sb[:, kc, tt * P:(tt + 1) * P],
                                    vw_sb[:, kc, n0:n0 + nw],
                                    start=(kc == 0), stop=False,
                                    skip_group_check=True)
                        if gi <= 1 and kc < 5:
                            # absorb per-chunk DMA lateness so the HAM
                            # busy window is not broken
                            warm_mm(256)
                            warm_mm(256)
                    for tt in tts:
                        for (n0, nw) in ((0, QW), (QW, C - QW)):
                            nc.tensor.matmul(
                                pss[tt][:, n0:n0 + nw],
                                ones_sb[0:1, 0:P],
                                vb_sb[0:1, n0:n0 + nw],
                                start=False, stop=True,
                                skip_group_check=True)
                        nc.vector.tensor_copy(
                            vp_sb[:, :, tt, 0:64],
                            pss[tt].rearrange("p (h d) -> p h d", d=64))
                        if ev8:
                            # on ACT (idle in phase 1): the psum-pool
                            # rotation waits on these copies, and two
                            # serial DVE casts per tile stall the PE at
                            # every V-group seam
                            nc.scalar.copy(
                                vp8_sb[:, :, tt // 2, tt % 2, 0:64],
                                pss[tt].rearrange("p (h d) -> p h d", d=64))

            # ---- attention (+ interleaved qkT / proj work) ----
            with tc.tile_pool(name="pp_st", bufs=2, space="PSUM") as pp_st, \
                 tc.tile_pool(name="pp_ev", bufs=2, space="PSUM") as pp_ev, \
                     tc.tile_pool(name="pp_av", bufs=2, space="PSUM") as pp_av:

                def qkt_ct(ct, pr_dst, qki):
                    """One qkT output tile (6 or 3 matmuls) + bias-add."""
                    ps = pp_st.tile([P, N], f32, tag="st", name=f"qk{ct}")
                    if qk8:
                        for g in range(NCIN // 2):
                            for qh in range(NQT):
                                nc.tensor.matmul(
                                    ps[:, qh * QW:(qh + 1) * QW],
                                    qkw8_sb[:, g, :, ct * P:(ct + 1) * P],
                                    x8_sb[:, g, :, qh * QW:(qh + 1) * QW],
                                    start=(g == 0),
                                    stop=(g == NCIN // 2 - 1),
                                    perf_mode=mybir.MatmulPerfMode.DoubleRow,
                                    skip_group_check=True)
                    else:
                        for kc in range(NCIN):
                            for qh in range(NQT):
                                nc.tensor.matmul(
                                    ps[:, qh * QW:(qh + 1) * QW],
                                    qkw_sb[:, kc, ct * P:(ct + 1) * P],
                                    xT_sb[:, kc, qh * QW:(qh + 1) * QW],
                                    start=(kc == 0), stop=(kc == NCIN - 1),
                                    skip_group_check=True)
                    nc.vector.tensor_scalar_add(
                        qkT_prs[pr_dst][:, qki, :], ps[:, :],
                        qkb_sb[:, ct:ct + 1])

                def qkt_group(pr):
                    qkt_ct(pr, pr, 0)
                    qkt_ct(NPAIR + pr, pr, 1)

                def warm_into(ps, n=QW):
                    # full-K const matmul into a psum region whose next
                    # real matmul is start=True (overwrites the garbage):
                    # keeps the PE busy so the HAM clock stays at 8/8
                    nc.tensor.matmul(
                        ps[0:64, 0:n], ww_sb[:, 0:64],
                        wr_sb[:, 0:n], start=True, stop=True,
                        skip_group_check=True)

                # dummy av-tag tile: a scratch psum bank for padding the
                # qkT-upfront stretch (paced by the qkw DMA stream)
                avw = pp_av.tile([P, QW], f32, tag="av", name="phasewarm")
                for ct_i, (ct, prd, qki) in enumerate(
                        ((0, 0, 0), (NPAIR, 0, 1), (1, 1, 0),
                         (NPAIR + 1, 1, 1))):
                    qkt_ct(ct, prd, qki)
                    for _ in range(5):
                        warm_into(avw)

                # proj weights stream after x/qkw/vw, before the at tiles
                for kc in range(NCIN):
                    nc.sync.dma_start(pw_sb[:, kc, :], pw_r[:, kc, :])

                # with fp8 qkT, qkT_prs holds 32*(q+b) per side -> the
                # score psum carries an extra 1024x, folded into the exp
                exp_scale = SCALE / 1024.0 if qk8 else SCALE

                def emit_st_step(pr, qt, e_sb, kt):
                    q0 = qt * QW
                    st = pp_st.tile([P, 2 * QW], f32, tag="st",
                                    name=f"st{pr}_{qt}_{kt}")
                    k0 = kt * P
                    nc.tensor.matmul(
                        st[:, 0:QW],
                        qkT_prs[pr][0:64, 1, k0:k0 + P],
                        qkT_prs[pr][0:64, 0, q0:q0 + QW],
                        start=True, stop=True)
                    nc.tensor.matmul(
                        st[:, QW:2 * QW],
                        qkT_prs[pr][64:128, 1, k0:k0 + P],
                        qkT_prs[pr][64:128, 0, q0:q0 + QW],
                        start=True, stop=True)
                    nc.scalar.activation(
                        e_sb[:, kt, :], st[:, :],
                        mybir.ActivationFunctionType.Exp, scale=exp_scale)

                def emit_out_step(item, kt):
                    pr, qt, e_sb, psE1, psE2, psA = item
                    h1, h2 = 2 * pr, 2 * pr + 1
                    at = atbf.tile([P, 2 * QW], bf16, tag="atb",
                                   name=f"atb{pr}_{qt}_{kt}")
                    nc.sync.dma_start(at[:], at_ext[pr, qt, kt])
                    st_flags = dict(start=(kt == 0), stop=(kt == NKT - 1),
                                    skip_group_check=True)
                    nc.tensor.matmul(
                        psA[0:64, :], vp_sb[:, h1, kt, 0:64],
                        at[:, 0:QW], **st_flags)
                    nc.tensor.matmul(
                        psA[64:128, :], vp_sb[:, h2, kt, 0:64],
                        at[:, QW:2 * QW], **st_flags)
                    if ev8:
                        # DoubleRow fp8: one matmul covers a k-tile pair
                        if kt % 2 == 1:
                            g = kt // 2
                            dr_flags = dict(
                                start=(g == 0), stop=(g == NKT // 2 - 1),
                                perf_mode=mybir.MatmulPerfMode.DoubleRow,
                                skip_group_check=True)
                            nc.tensor.matmul(
                                psE1[0:65, :],
                                vp8_sb[:, h1, g, :, 0:65],
                                e_sb[:, 2 * g:2 * g + 2, 0:QW], **dr_flags)
                            nc.tensor.matmul(
                                psE2[0:65, :],
                                vp8_sb[:, h2, g, :, 0:65],
                                e_sb[:, 2 * g:2 * g + 2, QW:2 * QW],
                                **dr_flags)
                    else:
                        nc.tensor.matmul(
                            psE1[0:65, :], vp_sb[:, h1, kt, :],
                            e_sb[:, kt, 0:QW], **st_flags)
                        nc.tensor.matmul(
                            psE2[0:65, :], vp_sb[:, h2, kt, :],
                            e_sb[:, kt, QW:2 * QW], **st_flags)

                recip_mode = os.environ.get("ATTN_RECIP", "dve")

                def emit_epilogue_recip(item, force_act=False):
                    # 1/rowsum; runs while the next block's score
                    # matmuls keep the PE busy
                    pr, qt, e_sb, psE1, psE2, psA = item
                    rs = []
                    for hi, psE in ((0, psE1), (1, psE2)):
                        if recip_mode == "act" or force_act:
                            lns = small.tile([1, QW], f32, tag="lns",
                                             name=f"ln{pr}_{qt}_{hi}")
                            nc.scalar.activation(
                                lns[:], psE[64:65, :],
                                mybir.ActivationFunctionType.Ln)
                            r16 = small.tile([1, QW], bf16, tag="r16",
                                             name=f"r16_{pr}_{qt}_{hi}")
                            nc.scalar.activation(
                                r16[:], lns[:],
                                mybir.ActivationFunctionType.Exp,
                                scale=-1.0)
                            rs.append(r16)
                            continue
                        # copy the rowsum to a partition-0 SBUF tile first:
                        # the custom-DVE recip mis-reads a partition-64
                        # PSUM operand (standard ops handle it fine)
                        rsum = small.tile([1, QW], f32, tag="rsum",
                                          name=f"rs_{pr}_{qt}_{hi}")
                        nc.vector.tensor_copy(rsum[:], psE[64:65, :])
                        r32 = small.tile([1, QW], f32, tag="r32",
                                         name=f"r32_{pr}_{qt}_{hi}")
                        nc.vector.reciprocal_approx_fast(r32[:], rsum[:])
                        if epi_mode == "gps":
                            rs.append(r32)
                        else:
                            r16 = small.tile([1, QW], bf16, tag="r16",
                                             name=f"r16_{pr}_{qt}_{hi}")
                            nc.vector.tensor_copy(r16[:], r32[:])
                            rs.append(r16)
                    return rs

                def emit_epilogue_apply(item, rs):
                    pr, qt, e_sb, psE1, psE2, psA = item
                    q0 = qt * QW
                    for hi, psE in ((0, psE1), (1, psE2)):
                        pa, pz = hi * 64, hi * 64 + 64
                        dst = attn_sb[pa:pz, pr, q0:q0 + QW]
                        if epi_mode == "gps":
                            rb = small.tile([64, QW], f32, tag="rb",
                                            name=f"rb{pr}_{qt}_{hi}")
                            nc.gpsimd.partition_broadcast(
                                rb[:], rs[hi][:], channels=64)
                            nc.vector.tensor_mul(dst, psE[0:64, :], rb[:])
                        else:
                            nc.tensor.matmul(psE[64:128, :],
                                             ones64_sb[0:1, :],
                                             rs[hi][:, :], start=True,
                                             stop=True,
                                             skip_group_check=True)
                            rb = small.tile([64, QW], f32, tag="rb",
                                            name=f"rb{pr}_{qt}_{hi}")
                            nc.vector.tensor_copy(rb[:], psE[64:128, :])
                            nc.vector.tensor_mul(dst, psE[0:64, :], rb[:])
                        nc.vector.tensor_add(dst, dst, psA[pa:pz, :])

                # qkT filler bursts inside the item loops: pairs 2..5,
                # each ct one-to-two items before its first use.  Item
                # (0,0) has no out-step work (pipeline fill), so it gets
                # two bursts.
                filler = {
                    (0, 0): [(2, 2, 0), (NPAIR + 2, 2, 1)],
                    (0, 1): [(3, 3, 0)],
                    (1, 0): [(NPAIR + 3, 3, 1)],
                    (1, 1): [(4, 4, 0)],
                    (2, 0): [(NPAIR + 4, 4, 1)],
                    (2, 1): [(5, 5, 0)],
                    (3, 0): [(NPAIR + 5, 5, 1)],
                }

                # software-pipelined emission: item i's ST/exp stream is
                # interleaved kt-by-kt with item i-1's E@v/A@v matmuls, so
                # the PE has dense work while ACT drains the score tiles
                items = [(pr, qt) for pr in range(NPAIR)
                         for qt in range(NQT)]
                prev = None        # item whose OUT runs in the current block
                pend = None        # pe-mode: (item, rs) awaiting PE/DVE apply
                e_dt = f8e4 if ev8 else bf16
                for pr, qt in items:
                    e_sb = epool.tile([P, NKT, 2 * QW], e_dt, tag="e",
                                      name=f"e{pr}_{qt}")
                    # two score steps up front cover the pending
                    # epilogue's DVE reciprocal latency
                    emit_st_step(pr, qt, e_sb, 0)
                    emit_st_step(pr, qt, e_sb, 1)
                    if pend is not None:
                        emit_epilogue_apply(*pend)
                        pend = None
                    psE1 = pp_ev.tile([P, QW], f32, tag="ev",
                                      name=f"ev1_{pr}_{qt}")
                    psE2 = pp_ev.tile([P, QW], f32, tag="ev",
                                      name=f"ev2_{pr}_{qt}")
                    psA = pp_av.tile([P, QW], f32, tag="av",
                                     name=f"av{pr}_{qt}")
                    cur = (pr, qt, e_sb, psE1, psE2, psA)
                    fill = list(filler.get((pr, qt), ()))
                    for kt in range(NKT):
                        if kt + 2 < NKT:
                            emit_st_step(pr, qt, e_sb, kt + 2)
                        if prev is not None:
                            emit_out_step(prev, kt)
                        if fill and kt in (2, 4):
                            qkt_ct(*fill.pop(0))
                    # item-boundary padding: the next item's first score
                    # steps wait on this item's last exps (ACT is the
                    # pacer once E@v runs DoubleRow); garbage written here
                    # is overwritten by this item's start=True A@v in the
                    # next block
                    i_next = items.index((pr, qt)) + 1
                    if i_next < len(items):
                        nwm = 2 if filler.get(items[i_next]) else itemwarm
                        for _ in range(nwm):
                            warm_into(psA)
                    if prev is not None:
                        rs = emit_epilogue_recip(prev)
                        if epi_mode == "gps":
                            emit_epilogue_apply(prev, rs)
                        else:
                            pend = (prev, rs)
                    prev = cur

                # ---- drain: last item's outs with proj partial bursts
                # interleaved, then the final epilogue and the output
                # projection ----
                def proj_partial(ps, ct, kcs, start, stop):
                    for kc in kcs:
                        for qh in range(NQT):
                            nc.tensor.matmul(
                                ps[:, qh * QW:(qh + 1) * QW],
                                pw_sb[:, kc, ct * P:(ct + 1) * P],
                                attn_sb[:, kc, qh * QW:(qh + 1) * QW],
                                start=(start and kc == kcs[0]),
                                stop=(stop and kc == kcs[-1]),
                                skip_group_check=True)

                with tc.tile_pool(name="ph3o", bufs=2) as ph3o:
                    out_r = out_ext.rearrange("(c p) t -> p c t", p=P)
                    pjs = {}

                    def proj_finish(ct, kcs, start):
                        ps = pjs[ct]
                        proj_partial(ps, ct, kcs, start, True)
                        o_sb = ph3o.tile([P, N], bf16, tag="o",
                                         name=f"o{ct}")
                        # alternate the bias-add between ACT and DVE so
                        # the six tail bias-adds run two-wide
                        if ct % 2 == 0:
                            nc.scalar.activation(
                                o_sb[:], ps[:],
                                mybir.ActivationFunctionType.Identity,
                                bias=pb_sb[:, ct:ct + 1])
                        else:
                            nc.vector.tensor_scalar_add(
                                o_sb[:], ps[:], pb_sb[:, ct:ct + 1])
                        nc.sync.dma_start(out_r[:, ct, :], o_sb[:])

                    for kt in range(NKT):
                        emit_out_step(prev, kt)
                        if kt == 0 and pend is not None:
                            emit_epilogue_apply(*pend)
                            pend = None
                        # the proj bursts recycle the st-score psum bufs;
                        # placed where the bufs actually free (exp kt6/kt7)
                        # so they don't stall the out-step stream
                        if kt == 5:
                            pjs[0] = pp_st.tile([P, N], f32, tag="st",
                                                name="proj0")
                            for _ in range(3):
                                warm_into(pjs[0])
                            proj_partial(pjs[0], 0, list(range(NCIN - 1)),
                                         True, False)
                        if kt == 6:
                            pjs[1] = pp_st.tile([P, N], f32, tag="st",
                                                name="proj1")
                            for _ in range(3):
                                warm_into(pjs[1])
                            proj_partial(pjs[1], 1, list(range(NCIN - 1)),
                                         True, False)
                    # const matmuls keep the PE busy (and the HAM clock
                    # warm) while the final epilogue chain runs on
                    # ACT/DVE; the proj finishes wait on it anyway
                    if tailwarm:
                        fav = pp_av.tile([P, QW], f32, tag="av",
                                         name="tailwarm")
                        for _ in range(tailwarm):
                            nc.tensor.matmul(
                                fav[0:64, :], ww_sb[:, 0:64],
                                wr_sb[:, :], start=True, stop=True,
                                skip_group_check=True)
                    # final epilogue: ACT recip chain — ACT is idle at
                    # the tail and the DVE chain (serial ~8us) was the
                    # dominant drain stall
                    rs = emit_epilogue_recip(prev, force_act=True)
                    emit_epilogue_apply(prev, rs)
                    proj_finish(0, [NCIN - 1], False)
                    proj_finish(1, [NCIN - 1], False)
                    for ct in range(2, NCIN):
                        pjs[ct] = pp_st.tile([P, N], f32, tag="st",
                                             name=f"proj{ct}")
                        proj_finish(ct, list(range(NCIN)), True)

    if os.environ.get("ATTN_DEDUP_LDW", "1") == "1":
        _dedup_ldweights(nc)
    if os.environ.get("ATTN_SPLIT_WAITS", "1") == "1":
        _split_excess_waits(nc)
    if not nc.is_finalized():
        nc.finalize()   # Bacc: move_matmul_waits + generate_event_semaphores
    return nc


def make_in_maps(x, qkv_w, qkv_b, static_a, proj_w, proj_b):
    """Host-side sharding / layout prep. One batch element per core."""
    x = np.asarray(x, dtype=np.float32)
    qkv_w = np.asarray(qkv_w, dtype=np.float32)
    qkv_b = np.asarray(qkv_b, dtype=np.float32)
    static_a = np.asarray(static_a, dtype=np.float32)
    proj_w = np.asarray(proj_w, dtype=np.float32)
    proj_b = np.asarray(proj_b, dtype=np.float32)

    import ml_dtypes
    bf16 = ml_dtypes.bfloat16
    f8 = ml_dtypes.float8_e4m3fn
    qk8 = os.environ.get("ATTN_QK8", "1") == "1"

    qkb_scale = 32.0 if qk8 else 1.0
    qkb = np.ascontiguousarray(
        qkv_b[0:2 * C].reshape(2 * C // P, P).T
        * qkb_scale).astype(np.float32)
    vwT = np.ascontiguousarray(qkv_w[2 * C:3 * C].T).astype(bf16)
    vb = np.ascontiguousarray(
        qkv_b[2 * C:3 * C].reshape(1, C)).astype(bf16)
    # A^T strips, contiguous per (pair, qtile, ktile): [6, 2, 8, 128, 1024]
    # at[pr, qt, kt, :, 0:512] = A^T[2pr][kt tile, qt tile], [..., 512:] = head 2pr+1
    atT = static_a[0].transpose(0, 2, 1)                      # [H, k, q]
    at = np.ascontiguousarray(
        atT.reshape(NPAIR, 2, NKT, P, NQT, QW).transpose(0, 4, 2, 3, 1, 5)
        .reshape(NPAIR, NQT, NKT, P, 2 * QW)).astype(bf16)
    pwT = np.ascontiguousarray(proj_w.T).astype(bf16)
    pb = np.ascontiguousarray(
        proj_b.reshape(C // P, P).T).astype(np.float32)

    shared = {"qkb": qkb, "vwT": vwT, "vb": vb,
              "at": at, "pwT": pwT, "pb": pb}
    if qk8:
        qkw32 = qkv_w[0:2 * C].T * 32.0                       # [768, 1536]
        shared["qkw8"] = np.ascontiguousarray(
            qkw32.reshape(NCIN // 2, 2, P, 2 * C)
            .transpose(2, 0, 1, 3)).astype(f8)
    else:
        shared["qkwT"] = np.ascontiguousarray(
            qkv_w[0:2 * C].T).astype(bf16)
    in_maps = []
    for b in range(B):
        m = dict(shared)
        xT = x[b].T
        m["xT"] = np.ascontiguousarray(xT).astype(bf16)
        if qk8:
            m["x8"] = np.ascontiguousarray(
                xT.reshape(NCIN // 2, 2, P, N)
                .transpose(2, 0, 1, 3)).astype(f8)
        in_maps.append(m)
    return in_maps


_NC_CACHE = {}


def _get_nc():
    if "nc" not in _NC_CACHE:
        _NC_CACHE["nc"] = build_nc()
    return _NC_CACHE["nc"]


def kernel(x, qkv_w, qkv_b, static_a, proj_w, proj_b):
    _ensure_paths()
    from concourse.bass_utils import run_bass_kernel_spmd

    nc = _get_nc()
    in_maps = make_in_maps(x, qkv_w, qkv_b, static_a, proj_w, proj_b)
    res = run_bass_kernel_spmd(nc, in_maps, core_ids=list(range(NCORES)))
    out = np.empty((B, N, C), dtype=np.float32)
    for b in range(B):
        out[b] = np.asarray(res.results[b]["out"], dtype=np.float32).T
    return out
